# revision 28
# baseline (speedup 1.0000x reference)
"""Trainium2 Bass kernel for nn_DecoderAttention (Bahdanau attention + LSTM decoder).

Data-parallel over batch: B=128 split across 8 NeuronCores (16 batches/core).
All FLOPs run on device; the host only reshuffles layouts (transpose / dtype
cast / fp8 DoubleRow packing / weight concat with bias rows folded in).

Per-core device pipeline (cost-model-aware layout):
  phase 0: ONE packed DMA for all small attention weights (+one fp32 qb DMA),
           qprojT = Wa @ q^T (+ ba + bua) on PE
  phase 1: per batch b: kprojT = Ua @ enc_b^T as fp8 DoubleRow matmuls
           (K=200 packed 2/partition, one pass, 0.5 cyc/row),
           tanh(kproj/64 + qprojT[:, b]) on ACT -> e tiles [h, t] bf16;
           scores as FLIPPED matmuls: e chunk stationary (K=h, M=128 t's of
           one stride-16 class), Va moving (N=1) -> scores accumulate into
           one [128, 256] PSUM tile, column 16*b+c;
           per WAVE of 4 batches: exp slice [128, 64] -> p (bf16,
           unnormalized), per-batch colsum matmuls -> zc, context via FLIPPED
           matmuls (encN chunk stationary K=t, p column moving N=1) -> ctxT
           accumulates [h, b] in PSUM. All of it hides inside phase 1.
  phase 2 (tail): Z totals via two tiny PE reductions, 1/Z broadcast via a
           K=1 outer-product matmul, ctxT scale on DVE, G0 closes an
           accumulation whose q-terms ran during phase 1.
  phase 3: 5 serial decoder steps, all-bf16 elementwise:
           gates = G0(PSUM) + x*wxr fused on DVE (scalar_tensor_tensor,
           split f,i,o vs g), gate order permuted so one sigmoid covers
           f,i,o; MLP in bf16; m3 flipped so x_next = out [16, 1] feeds the
           next step's scalar directly from PSUM.
"""

import numpy as np
import ml_dtypes

B, T, H = 128, 2048, 200
NCORES = 8
NB = B // NCORES  # 16 batches per core
NSTEPS = 5
G4 = 4 * H  # 800 gate width
NCH = T // 128  # 16 stride-class chunks (t = 16*k + c -> partition k, chunk c)

_CACHE = {}

# M-packed kproj/tanh layout: batch b's 200 h-rows live at packed rows
# [224*b, 224*b+200) (stride 224 = 7*32 keeps every PE write 32-aligned,
# 16*224 = 3584 = 28 full 128-row tiles); h_out padded 200->224 with zeros.
BSTRIDE = 224
NTILE = NB * BSTRIDE // 128  # 28


def _legal_pieces(off, ln):
    """Split (off, ln) into PE-legal out placements: base 0 takes up to 128
    rows, base 64 up to 64, bases 32/96 up to 32."""
    out = []
    while ln > 0:
        cap = {0: 128, 32: 32, 64: 64, 96: 32}[off % 128]
        take = min(ln, cap)
        out.append((off, take))
        off += take
        ln -= take
    return out


def _batch_tiles(b):
    return sorted({ti for ti, _, _, _ in _segments(b)})


# columns of the per-(batch,tile) zero-masked Va block, in emission order
VA_COLS = {}
_c = 0
for _b in range(16):
    for _ti in range(28):
        if 224 * _b < 128 * (_ti + 1) and 128 * _ti < 224 * _b + 224:
            VA_COLS[(_b, _ti)] = _c
            _c += 1
NV = _c


def _segments(b):
    """Packed-row segments of batch b: (tile, off, hlo, ln) covering
    packed rows [BSTRIDE*b, BSTRIDE*(b+1)), split at 128-row tile edges."""
    out = []
    r0 = BSTRIDE * b
    r = r0
    while r < r0 + BSTRIDE:
        ti, off = divmod(r, 128)
        ln = min(128 - off, r0 + BSTRIDE - r)
        out.append((ti, off, r - r0, ln))
        r += ln
    return out


def _sub_segments(b):
    """Real-h sub-segments of batch b for the scores matmuls:
    (tile, off, hlo, ln) additionally split at h=128 and clipped at h=200."""
    out = []
    for ti, off, hlo, ln in _segments(b):
        ln = min(ln, 200 - hlo)
        if ln <= 0:
            continue
        pieces = []
        if hlo < 128 < hlo + ln:
            cut = 128 - hlo
            pieces = [(off, hlo, cut), (off + cut, 128, ln - cut)]
        else:
            pieces = [(off, hlo, ln)]
        for poff, phlo, pln in pieces:
            # stationary reads obey the same base/size tiling rule as writes
            for qoff, qln in _legal_pieces(poff, pln):
                out.append((ti, qoff, phlo + (qoff - poff), qln))
    return out

BF16 = ml_dtypes.bfloat16
FP8 = ml_dtypes.float8_e4m3fn

# packed attention-weight tensor column layout (bf16, [128, PK_C])
PK_WA0, PK_WA1 = 0, 224          # wa0 [128,224] | wa1 [72,224] (224 = padded)
PK_VA0, PK_VA1 = 448, 449        # va columns
PK_QT0, PK_QT1 = 450, 466        # qt [*,16]
PK_QBR = 482                     # qb as a [1, 224] row (for K=1 bias matmuls)
PK_VAPK = 706                    # zero-masked Va per (batch, tile) [128, NV]
PK_C = 706 + 44

# packed decoder-weight tensor column layout (bf16, [128, DK_C])
DK_WIHC0, DK_WIHC1 = 0, 800      # wihc0 [128,800] | wihc1 [73,800] (row72=bias)
DK_WHH0, DK_WHH1 = 1600, 2400    # whh [128|72, 800]
DK_WXR = 3200                    # wxr [16, 800]
DK_W1T0, DK_W1T1 = 4000, 4100    # w1t [128|73, 100] (row72=b1)
DK_W2T = 4200                    # w2t [101, 50] (row100=b2)
DK_W3T = 4250                    # w3t [51, 1] (row50=b3)
DK_C0 = 4251                     # c0 [16, 200]
DK_C = 4451


def _build_module():
    """Build the Bass module (same NEFF for all 8 cores)."""
    from contextlib import ExitStack

    import concourse.bass as bass
    import concourse.tile as tile
    from concourse import bacc, mybir
    from concourse.masks import make_identity

    dt = mybir.dt
    AF = mybir.ActivationFunctionType
    OP = mybir.AluOpType

    nc = bacc.Bacc(
        "TRN2",
        target_bir_lowering=False,
        debug=False,
        num_devices=NCORES,
    )

    # ---- DRAM tensors (per-core shards; weights replicated) ----
    d_encT = nc.dram_tensor(
        "encTp", [NB, 128, 2 * T], dt.float8e4, kind="ExternalInput"
    ).ap()  # DoubleRow packing: col i*T+t, partition p <-> h_in = i*128+p
    d_encN = nc.dram_tensor("encN", [NB, T, H], dt.bfloat16, kind="ExternalInput").ap()
    d_x0 = nc.dram_tensor("x0s", [NB, 1], dt.float32, kind="ExternalInput").ap()
    d_UaT = nc.dram_tensor("UaTp", [128, 2 * BSTRIDE], dt.float8e4, kind="ExternalInput").ap()
    d_wpk = nc.dram_tensor("wpk", [128, PK_C], dt.bfloat16, kind="ExternalInput").ap()
    d_dpk = nc.dram_tensor("dpk", [128, DK_C], dt.bfloat16, kind="ExternalInput").ap()
    d_ones_b = nc.dram_tensor("ones_b", [1, NB], dt.bfloat16, kind="ExternalInput").ap()
    d_y = nc.dram_tensor("y2", [NB, NSTEPS], dt.float32, kind="ExternalOutput").ap()

    H0, H1 = 128, H - 128  # 128 + 72 partition chunks of the hidden dim

    with tile.TileContext(nc) as tc, ExitStack() as ctx:
        # ---------- persistent pools ----------
        wpool = ctx.enter_context(tc.tile_pool(name="weights", bufs=1))
        spool = ctx.enter_context(tc.tile_pool(name="smalls", bufs=1))

        # warmup: preload the tanh/exp ACT table set while DMAs stream
        wt_a = spool.tile([1, 8], dt.float32)
        nc.vector.memset(wt_a[:], 0.0)
        wt_b = spool.tile([1, 8], dt.float32)
        nc.scalar.activation(wt_b[:], wt_a[:], AF.Tanh)

        # identity for the decoder's h transposes (bf16)
        id_bf = wpool.tile([128, 128], dt.bfloat16)
        make_identity(nc, id_bf[:])

        # ones columns/rows for the tiny PE reductions (sliced on read)
        ones_c_bf = wpool.tile([128, 1], dt.bfloat16)
        nc.vector.memset(ones_c_bf[:], 1.0)
        ones_c_f = wpool.tile([128, 1], dt.float32)
        nc.vector.memset(ones_c_f[:], 1.0)
        ones_sq_f = wpool.tile([NCH, 128], dt.float32)
        nc.vector.memset(ones_sq_f[:], 1.0)

        # packed attention weights: one DMA instead of ten
        wpk = wpool.tile([128, PK_C], dt.bfloat16)
        nc.scalar.dma_start(wpk[:], d_wpk[:, :])
        wa0 = wpk[:, PK_WA0 : PK_WA0 + BSTRIDE]
        wa1 = wpk[0:H1, PK_WA1 : PK_WA1 + BSTRIDE]
        va_pk = wpk[:, PK_VAPK : PK_VAPK + NV]
        qt0 = wpk[:, PK_QT0 : PK_QT0 + NB]
        qt1 = wpk[0:H1, PK_QT1 : PK_QT1 + NB]

        ua_p = wpool.tile([128, 2 * BSTRIDE], dt.float8e4)
        ua3 = ua_p[:].rearrange("p (i m) -> p i m", i=2)

        # packed decoder weights: one DMA (deferred below, behind first encT)
        dpk = wpool.tile([128, DK_C], dt.bfloat16)
        wihc0 = dpk[:, DK_WIHC0 : DK_WIHC0 + G4]
        wihc1 = dpk[0 : H1 + 1, DK_WIHC1 : DK_WIHC1 + G4]
        whh0 = dpk[:, DK_WHH0 : DK_WHH0 + G4]
        whh1 = dpk[0:H1, DK_WHH1 : DK_WHH1 + G4]
        wxr_sb = dpk[0:NB, DK_WXR : DK_WXR + G4]
        w1t0 = dpk[:, DK_W1T0 : DK_W1T0 + 100]
        w1t1 = dpk[0 : H1 + 1, DK_W1T1 : DK_W1T1 + 100]
        w2t = dpk[0:101, DK_W2T : DK_W2T + 50]
        w3t = dpk[0:51, DK_W3T : DK_W3T + 1]
        c0_sb = dpk[0:NB, DK_C0 : DK_C0 + H]

        # ---------- phase 0: packed qproj bias: qp_pk[r, ti] = ----------
        # qproj[h(r,ti), b(r,ti)] + qb[h]; pad rows get tanh(0*scale+qb-row)
        qp_pk_sb = spool.tile([128, NTILE], dt.float32)
        ones1_b = wpool.tile([1, 1], dt.bfloat16)
        nc.vector.memset(ones1_b[:], 1.0)
        with tc.tile_pool(name="qp_psum", bufs=1, space="PSUM") as qp_ps:
            ps = qp_ps.tile([128, NTILE], dt.float32, tag="qp")
            for b in range(NB):
                for ti, off, hlo, ln in _segments(b):
                    for poff, pln in _legal_pieces(off, ln):
                        phlo = hlo + (poff - off)
                        dst = ps[poff : poff + pln, ti : ti + 1]
                        nc.tensor.matmul(
                            dst, wa0[:, phlo : phlo + pln], qt0[:, b : b + 1],
                            start=True, stop=False, tile_position=(0, poff),
                        )
                        nc.tensor.matmul(
                            dst, wa1[:, phlo : phlo + pln], qt1[:, b : b + 1],
                            start=False, stop=False, tile_position=(0, poff),
                        )
                        # + qb (K=1 outer with the qb row)
                        nc.tensor.matmul(
                            dst, wpk[0:1, PK_QBR + phlo : PK_QBR + phlo + pln],
                            ones1_b[:], start=False, stop=True,
                            tile_position=(0, poff),
                        )
            nc.vector.tensor_copy(qp_pk_sb[:], ps[:])

        # G0 PSUM lives from phase 2 through the decoder; allocate its pool
        # ahead of the attention pools so releases stay LIFO
        g0_pool = ctx.enter_context(tc.tile_pool(name="g0_psum", bufs=1, space="PSUM"))
        gp = g0_pool.tile([NB, G4], dt.float32, tag="g0")

        # ---------- phase 1: kproj + tanh + scores + per-wave softmax/ctx ----
        encT_pool = ctx.enter_context(tc.tile_pool(name="encT_pool", bufs=3))
        e_pool = ctx.enter_context(tc.tile_pool(name="e_pool", bufs=4))
        encN_pool = ctx.enter_context(tc.tile_pool(name="encN_pool", bufs=16))
        from contextlib import ExitStack as _ES
        att_ctx = _ES()
        sc_pool = att_ctx.enter_context(tc.tile_pool(name="sc_psum", bufs=1, space="PSUM"))
        ct_pool = att_ctx.enter_context(tc.tile_pool(name="ct_psum", bufs=1, space="PSUM"))
        sc = sc_pool.tile([128, NB * NCH + NB], dt.float32, tag="sc")
        p_sb = spool.tile([128, NB * NCH], dt.bfloat16)
        zc = sc[0:NCH, NB * NCH : NB * NCH + NB]
        # one PSUM tile (PSUM tiles are bank-granular): ct0 | ct1 columns
        ctz = ct_pool.tile([128, 2 * NB], dt.float32, tag="ctz")
        ct0_ps = ctz[:, 0:NB]
        ct1_ps = ctz[0:H1, NB : 2 * NB]

        en_tiles = []
        kp_hist = {}  # batch -> last kproj matmul (encN pacing anchor)
        e_pk = []     # packed e tiles, one per 128-row tile

        def emit_scores(b):
            tiles = _batch_tiles(b)
            for c in range(NCH):
                col = b * NCH + c
                for k, ti in enumerate(tiles):
                    # full-tile contraction; va column is zero outside
                    # batch b's rows, so other batches contribute nothing
                    vcol = VA_COLS[(b, ti)]
                    nc.tensor.matmul(
                        sc[:, col : col + 1],
                        e_pk[ti][:, c : T : NCH],
                        va_pk[:, vcol : vcol + 1],
                        start=(k == 0),
                        stop=(k == len(tiles) - 1),
                    )

        def emit_wave(w):
            # exp + Z colsums + context for batches 4w..4w+3 (scores ready)
            lo = 4 * w * NCH
            nc.scalar.activation(
                p_sb[:, lo : lo + 4 * NCH], sc[:, lo : lo + 4 * NCH], AF.Exp
            )
            for b in range(4 * w, 4 * w + 4):
                nc.tensor.matmul(
                    zc[:, b : b + 1],
                    p_sb[:, b * NCH : (b + 1) * NCH],
                    ones_c_bf[:],
                    start=True,
                    stop=True,
                )
                # complete each accumulation chain before starting the next
                # (two open groups in one PSUM bank trip the zero-region rule)
                for c in range(NCH):
                    nc.tensor.matmul(
                        ct0_ps[:, b : b + 1],
                        en_tiles[b][:, c * H : c * H + H0],
                        p_sb[:, b * NCH + c : b * NCH + c + 1],
                        start=(c == 0),
                        stop=(c == NCH - 1),
                    )
                for c in range(NCH):
                    nc.tensor.matmul(
                        ct1_ps[:, b : b + 1],
                        en_tiles[b][:, c * H + H0 : (c + 1) * H],
                        p_sb[:, b * NCH + c : b * NCH + c + 1],
                        start=(c == 0),
                        stop=(c == NCH - 1),
                    )

        # per-tile segment map and per-batch last tile
        tile_segs = [[] for _ in range(NTILE)]
        for b in range(NB):
            for ti, off, hlo, ln in _segments(b):
                tile_segs[ti].append((b, off, hlo, ln))
        last_tile = [_segments(b)[-1][0] for b in range(NB)]

        import bass_rust as _br
        et_tiles = {}
        scores_done = -1  # highest batch whose scores are emitted

        def load_batch(b):
            et = encT_pool.tile([128, 2 * T], dt.float8e4, tag="et", name=f"et{b}")
            nc.sync.dma_start(et[:], d_encT[b, :])
            et_tiles[b] = et[:].rearrange("p (i t) -> p i t", i=2)
            if b == 0:
                nc.sync.dma_start(ua_p[:], d_UaT[:, :])
            if b == 3:
                # single packed decoder-weight DMA, off the startup path
                nc.sync.dma_start(dpk[:], d_dpk[:, :])

        with tc.tile_pool(name="kp_psum", bufs=2, space="PSUM") as kp_ps:
            load_batch(0)
            for ti in range(NTILE):
                for b, _, _, _ in tile_segs[ti]:
                    if b not in et_tiles:
                        load_batch(b)
                e = e_pool.tile([128, T], dt.bfloat16, tag="e", name=f"e{ti}")
                e_pk.append(e)
                for th in range(2):
                    ps = kp_ps.tile([128, 1024], dt.float32, tag="kp")
                    for b, off, hlo, ln in tile_segs[ti]:
                        for poff, pln in _legal_pieces(off, ln):
                            phlo = hlo + (poff - off)
                            for n in range(2):
                                c0c = th * 1024 + n * 512
                                dst = ps[poff : poff + pln, n * 512 : (n + 1) * 512]
                                if poff == 0 and pln in (32, 64, 128):
                                    # DoubleRow dst only encodes at base 0
                                    # with a 32/64/128-row group
                                    i_kp = nc.tensor.matmul(
                                        dst,
                                        ua3[:, :, phlo : phlo + pln],
                                        et_tiles[b][:, :, c0c : c0c + 512],
                                        start=True,
                                        stop=True,
                                        perf_mode=mybir.MatmulPerfMode.DoubleRow,
                                        tile_position=(0, poff),
                                    )
                                else:
                                    for i in range(2):
                                        i_kp = nc.tensor.matmul(
                                            dst,
                                            ua3[:, i, phlo : phlo + pln],
                                            et_tiles[b][:, i, c0c : c0c + 512],
                                            start=(i == 0),
                                            stop=(i == 1),
                                            tile_position=(0, poff),
                                        )
                                kp_hist[b] = i_kp
                    # e = tanh(kproj/64 + qp_pk[:, ti]) ; write bf16
                    nc.scalar.activation(
                        e[:, th * 1024 : (th + 1) * 1024],
                        ps[:],
                        AF.Tanh,
                        bias=qp_pk_sb[:, ti : ti + 1],
                        scale=1.0 / 64.0,
                    )
                # scores/waves with a one-tile skew; encN paced ~2 batches back
                for b in range(scores_done + 1, NB):
                    if last_tile[b] != ti - 1:
                        break
                    emit_scores(b)
                    scores_done = b
                    en = encN_pool.tile(
                        [128, NCH * H], dt.bfloat16, name=f"en{b}", tag="en"
                    )
                    i_en = nc.gpsimd.dma_start(
                        en[:],
                        d_encN[b].rearrange("(p n) h -> p (n h)", p=128),
                    )
                    _br.add_dep_helper(
                        i_en.ins, kp_hist[max(0, b - 2)].ins, sync=True,
                        reason="encN paced behind kproj two batches back",
                    )
                    en_tiles.append(en)
                    if b % 4 == 3:
                        emit_wave(b // 4)
            for b in range(scores_done + 1, NB):
                emit_scores(b)
                if b >= len(en_tiles):
                    en = encN_pool.tile(
                        [128, NCH * H], dt.bfloat16, name=f"en{b}", tag="en"
                    )
                    nc.gpsimd.dma_start(
                        en[:],
                        d_encN[b].rearrange("(p n) h -> p (n h)", p=128),
                    )
                    en_tiles.append(en)
                if b % 4 == 3:
                    emit_wave(b // 4)

        # ---------- phase 2: Z totals, 1/Z, ctx scale, G0 ----------
        ct0 = spool.tile([H0, NB], dt.bfloat16)
        ct1 = spool.tile([H1 + 1, NB], dt.bfloat16)  # row 72 = ones (bias row)
        nc.scalar.dma_start(ct1[H1 : H1 + 1, :], d_ones_b[:, :])

        with tc.tile_pool(name="z_psum", bufs=1, space="PSUM") as z_ps:
            # q-dependent G0 terms: no ctx dependency, run right away
            for n, nsz in [(0, 512), (512, G4 - 512)]:
                nc.tensor.matmul(
                    gp[:, n : n + nsz], qt0, whh0[:, n : n + nsz],
                    start=True, stop=False,
                )
                nc.tensor.matmul(
                    gp[:, n : n + nsz], qt1, whh1[:, n : n + nsz],
                    start=False, stop=False,
                )
            # Z per batch broadcast down all 128 partitions in one matmul
            # (lhsT = ones [16, 128] -> out[r, b] = sum_ch zc[ch, b]), then
            # reciprocal straight into SBUF
            zc_sb = spool.tile([NCH, NB], dt.float32)
            nc.vector.tensor_copy(zc_sb[:], zc[:])
            zbc = z_ps.tile([128, NB], dt.float32, tag="zbc")
            nc.tensor.matmul(zbc[:], ones_sq_f[:], zc_sb[:], start=True, stop=True)
            rzb_sb = spool.tile([128, NB], dt.float32)
            nc.vector.reciprocal(rzb_sb[:], zbc[:])
            # normalize: ctxT = ctx_raw * (1/Z) broadcast, cast bf16
            nc.vector.tensor_tensor(ct0[:], ct0_ps[:], rzb_sb[:], op=OP.mult)
            nc.vector.tensor_tensor(
                ct1[0:H1, :], ct1_ps[:], rzb_sb[0:H1, :], op=OP.mult
            )
            for n, nsz in [(0, 512), (512, G4 - 512)]:
                nc.tensor.matmul(
                    gp[:, n : n + nsz], ct0[:], wihc0[:, n : n + nsz],
                    start=False, stop=False,
                )
                nc.tensor.matmul(
                    gp[:, n : n + nsz], ct1, wihc1[:, n : n + nsz],
                    start=False, stop=True,
                )
        att_ctx.close()  # release sc/ct PSUM banks before the decoder

        # ---------- phase 3: decoder steps (all bf16, gate order f,i,o|g) ---
        x_sb = spool.tile([NB, 1], dt.float32)
        nc.sync.dma_start(x_sb[:], d_x0[:, :])
        xn_all = spool.tile([NB, NSTEPS], dt.float32)
        ht0 = spool.tile([H0, NB], dt.bfloat16)
        ht1 = spool.tile([H1 + 1, NB], dt.bfloat16)  # row 72 = ones (b1 row)
        nc.scalar.dma_start(ht1[H1 : H1 + 1, :], d_ones_b[:, :])
        o1t = spool.tile([101, NB], dt.bfloat16)  # row 100 = ones (b2 row)
        nc.scalar.dma_start(o1t[100:101, :], d_ones_b[:, :])
        o2t = spool.tile([51, NB], dt.bfloat16)  # row 50 = ones (b3 row)
        nc.scalar.dma_start(o2t[50:51, :], d_ones_b[:, :])

        with (
            tc.tile_pool(name="ls", bufs=2) as ls,
            tc.tile_pool(name="ls_psum", bufs=3, space="PSUM") as lp,
            tc.tile_pool(name="m3_psum", bufs=2, space="PSUM") as mp,
        ):
            xt = x_sb
            for t in range(NSTEPS):
                # gates = g0 + x * wxr, fused on DVE; split f,i,o vs g so
                # the sigmoid can start before the g slice is computed
                gates = ls.tile([NB, G4], dt.bfloat16, tag="gates")
                for glo, ghi in ((0, 2 * H), (3 * H, 4 * H), (2 * H, 3 * H)):
                    nc.vector.scalar_tensor_tensor(
                        gates[:, glo:ghi], wxr_sb[:, glo:ghi], xt[:, 0:1],
                        gp[:, glo:ghi], op0=OP.mult, op1=OP.add,
                    )
                # f,i sigmoid first (feeds t1/t2), then g tanh, then o
                sfio = ls.tile([NB, 3 * H], dt.bfloat16, tag="sfio")
                nc.scalar.activation(sfio[:, 0 : 2 * H], gates[:, 0 : 2 * H], AF.Sigmoid)
                tg = ls.tile([NB, H], dt.bfloat16, tag="tg")
                nc.scalar.activation(tg[:], gates[:, 3 * H : 4 * H], AF.Tanh)
                nc.scalar.activation(
                    sfio[:, 2 * H : 3 * H], gates[:, 2 * H : 3 * H], AF.Sigmoid
                )
                t1 = ls.tile([NB, H], dt.bfloat16, tag="t1")
                nc.vector.tensor_tensor(t1[:], sfio[:, 0:H], c0_sb, op=OP.mult)
                t2 = ls.tile([NB, H], dt.bfloat16, tag="t2")
                nc.vector.tensor_tensor(t2[:], sfio[:, H : 2 * H], tg[:], op=OP.mult)
                cn = ls.tile([NB, H], dt.bfloat16, tag="cn")
                nc.vector.tensor_tensor(cn[:], t1[:], t2[:], op=OP.add)
                tcn = ls.tile([NB, H], dt.bfloat16, tag="tcn")
                nc.scalar.activation(tcn[:], cn[:], AF.Tanh)
                hh = ls.tile([NB, H], dt.bfloat16, tag="hh")
                nc.vector.tensor_tensor(hh[:], sfio[:, 2 * H : 3 * H], tcn[:], op=OP.mult)
                # transpose h -> ht0/ht1 (feature-major for the MLP); relu
                # folded into the PSUM->SBUF copies (DVE max / ACT relu)
                tp0 = lp.tile([128, NB], dt.bfloat16, tag="lsps")
                nc.tensor.transpose(tp0[:], hh[:, 0:H0], id_bf[0:NB, 0:NB])
                nc.vector.tensor_scalar_max(ht0[:], tp0[:], 0.0)
                tp1 = lp.tile([128, NB], dt.bfloat16, tag="lsps")
                nc.tensor.transpose(tp1[0:H1, :], hh[:, H0:H], id_bf[0:NB, 0:NB])
                nc.scalar.activation(ht1[0:H1, :], tp1[0:H1, :], AF.Relu)
                # MLP in feature-major, biases via ones rows
                m1 = lp.tile([100, NB], dt.float32, tag="lsps")
                nc.tensor.matmul(m1[:], w1t0, ht0[:], start=True, stop=False)
                nc.tensor.matmul(m1[:], w1t1, ht1[:], start=False, stop=True)
                nc.vector.tensor_scalar_max(o1t[0:100, :], m1[:], 0.0)
                m2 = lp.tile([50, NB], dt.float32, tag="lsps")
                nc.tensor.matmul(m2[:], w2t, o1t[:], start=True, stop=True)
                nc.vector.tensor_scalar_max(o2t[0:50, :], m2[:], 0.0)
                # m3 flipped: o2 stationary, w3 moving -> out [NB, 1] is
                # directly the next step's x (read from PSUM as STT scalar)
                m3 = mp.tile([NB, 1], dt.float32, tag="m3")
                nc.tensor.matmul(m3[:], o2t[:], w3t, start=True, stop=True)
                nc.vector.tensor_copy(xn_all[:, t : t + 1], m3[:])
                xt = m3
            nc.sync.dma_start(d_y[:, :], xn_all[:])

    # Bacc lowering: register allocation + wait splitting (<=1 wait/inst on HW)
    nc.compile()
    return nc


def _prep_inputs(x, h0, c0, encoder_output, Wa, ba, Ua, bua, Va, bva,
                 W_ih, W_hh, b_ih, b_hh, W1, b1, W2, b2, W3, b3):
    """Host-side layout prep -> list of per-core input maps."""
    f32 = np.float32
    enc = np.ascontiguousarray(encoder_output, dtype=f32)
    q = np.asarray(h0, dtype=f32)[0]          # [B, H]
    c0f = np.asarray(c0, dtype=f32)[0]        # [B, H]
    x0 = np.asarray(x, dtype=f32).reshape(B, 1)

    # gate permutation: torch order (i,f,g,o) -> device order (f,i,o,g) so
    # one sigmoid instr covers f,i,o and tanh covers g
    gperm = np.concatenate(
        [np.arange(H, 2 * H), np.arange(0, H), np.arange(3 * H, 4 * H),
         np.arange(2 * H, 3 * H)]
    )
    W_ihp = np.asarray(W_ih, f32)[gperm]
    W_hhp = np.asarray(W_hh, f32)[gperm]
    bp = (np.asarray(b_ih, f32) + np.asarray(b_hh, f32))[gperm]

    # Ua scaled x64 into fp8 comfortable range; kernel rescales inside tanh.
    # DoubleRow packing: partition p holds h_in = p (i=0) and 128+p (i=1);
    # M padded 200->208 so the pair stride is 16B-aligned.
    uaT = np.asarray(Ua, f32).T * 64.0  # [h_in, h_out]
    uaT_pad = np.zeros((256, BSTRIDE), f32)
    uaT_pad[0:H, 0:H] = uaT
    uaT_p = np.stack([uaT_pad[0:128], uaT_pad[128:256]], axis=1)  # [128, 2, 224]

    # packed attention weights
    wpk = np.zeros((128, PK_C), f32)
    waT = np.asarray(Wa, f32).T  # [h_in, h_out]
    wpk[:, PK_WA0 : PK_WA0 + 200] = waT[0:128]
    wpk[0:72, PK_WA1 : PK_WA1 + 200] = waT[128:200]
    va = np.asarray(Va, f32)[0]
    for (bb, ti), cidx in VA_COLS.items():
        r = np.arange(128)
        h = 128 * ti + r - BSTRIDE * bb
        mask = (h >= 0) & (h < H)
        colv = np.zeros(128, f32)
        colv[mask] = va[h[mask]]
        wpk[:, PK_VAPK + cidx] = colv
    qb = np.asarray(ba, f32) + np.asarray(bua, f32)
    wpk[0, PK_QBR : PK_QBR + H] = qb

    # packed decoder weights
    dpk = np.zeros((128, DK_C), f32)
    w_ihcT = W_ihp[:, 1:].T  # [H, G4]
    dpk[:, DK_WIHC0 : DK_WIHC0 + G4] = w_ihcT[0:128]
    dpk[0:72, DK_WIHC1 : DK_WIHC1 + G4] = w_ihcT[128:200]
    dpk[72, DK_WIHC1 : DK_WIHC1 + G4] = bp
    w_hhT = W_hhp.T
    dpk[:, DK_WHH0 : DK_WHH0 + G4] = w_hhT[0:128]
    dpk[0:72, DK_WHH1 : DK_WHH1 + G4] = w_hhT[128:200]
    dpk[0:NB, DK_WXR : DK_WXR + G4] = np.broadcast_to(
        W_ihp[:, 0].reshape(1, G4), (NB, G4)
    )
    w1T = np.asarray(W1, f32).T
    dpk[:, DK_W1T0 : DK_W1T0 + 100] = w1T[0:128]
    dpk[0:72, DK_W1T1 : DK_W1T1 + 100] = w1T[128:200]
    dpk[72, DK_W1T1 : DK_W1T1 + 100] = np.asarray(b1, f32)
    dpk[0:100, DK_W2T : DK_W2T + 50] = np.asarray(W2, f32).T
    dpk[100, DK_W2T : DK_W2T + 50] = np.asarray(b2, f32)
    dpk[0:50, DK_W3T] = np.asarray(W3, f32)[0]
    dpk[50, DK_W3T] = np.asarray(b3, f32)[0]

    shared = {
        "UaTp": np.ascontiguousarray(uaT_p.reshape(128, 2 * BSTRIDE)).astype(FP8),
        "ones_b": np.ones((1, NB), BF16),
    }

    in_maps = []
    for cix in range(NCORES):
        bs = slice(cix * NB, (cix + 1) * NB)
        enc_c = enc[bs]  # [NB, T, H]
        m = dict(shared)
        encT_c = enc_c.transpose(0, 2, 1)  # [NB, H, T]
        encT_pad = np.concatenate(
            [encT_c, np.zeros((NB, 56, T), f32)], axis=1
        )  # [NB, 256, T]
        encT_p = np.stack([encT_pad[:, 0:128], encT_pad[:, 128:256]], axis=2)
        m["encTp"] = np.ascontiguousarray(encT_p.reshape(NB, 128, 2 * T)).astype(FP8)
        m["encN"] = enc_c.astype(BF16)
        dpk_c = dpk.copy()
        dpk_c[0:NB, DK_C0 : DK_C0 + H] = c0f[bs]
        m["dpk"] = dpk_c.astype(BF16)
        wpk_c = wpk.copy()
        qTc = q[bs].T  # [H, NB]
        wpk_c[:, PK_QT0 : PK_QT0 + NB] = qTc[0:128]
        wpk_c[0:72, PK_QT1 : PK_QT1 + NB] = qTc[128:200]
        m["wpk"] = wpk_c.astype(BF16)
        m["x0s"] = np.ascontiguousarray(x0[bs])
        in_maps.append(m)
    return in_maps


def kernel(**inputs):
    from concourse.bass_utils import run_bass_kernel_spmd

    if "nc" not in _CACHE:
        _CACHE["nc"] = _build_module()
    nc = _CACHE["nc"]

    in_maps = _prep_inputs(**inputs)
    res = run_bass_kernel_spmd(nc, in_maps, core_ids=list(range(NCORES)))
    # y2 per core: [NB, NSTEPS] -> full output [B, NSTEPS]
    out = np.concatenate([r["y2"] for r in res.results], axis=0)
    return np.ascontiguousarray(out.astype(np.float32))


# revision 29
# speedup vs baseline: 1.1335x; 1.1335x over previous
"""Trainium2 Bass kernel for nn_DecoderAttention (Bahdanau attention + LSTM decoder).

Data-parallel over batch: B=128 split across 8 NeuronCores (16 batches/core).
All FLOPs run on device; the host only reshuffles layouts (transpose / dtype
cast / fp8 DoubleRow packing / weight concat with bias rows folded in).

Per-core device pipeline (cost-model-aware layout):
  phase 0: ONE packed DMA for all small attention weights (+one fp32 qb DMA),
           qprojT = Wa @ q^T (+ ba + bua) on PE
  phase 1: per batch b: kprojT = Ua @ enc_b^T as fp8 DoubleRow matmuls
           (K=200 packed 2/partition, one pass, 0.5 cyc/row),
           tanh(kproj/64 + qprojT[:, b]) on ACT -> e tiles [h, t] bf16;
           scores as FLIPPED matmuls: e chunk stationary (K=h, M=128 t's of
           one stride-16 class), Va moving (N=1) -> scores accumulate into
           one [128, 256] PSUM tile, column 16*b+c;
           per WAVE of 4 batches: exp slice [128, 64] -> p (bf16,
           unnormalized), per-batch colsum matmuls -> zc, context via FLIPPED
           matmuls (encN chunk stationary K=t, p column moving N=1) -> ctxT
           accumulates [h, b] in PSUM. All of it hides inside phase 1.
  phase 2 (tail): Z totals via two tiny PE reductions, 1/Z broadcast via a
           K=1 outer-product matmul, ctxT scale on DVE, G0 closes an
           accumulation whose q-terms ran during phase 1.
  phase 3: 5 serial decoder steps, all-bf16 elementwise:
           gates = G0(PSUM) + x*wxr fused on DVE (scalar_tensor_tensor,
           split f,i,o vs g), gate order permuted so one sigmoid covers
           f,i,o; MLP in bf16; m3 flipped so x_next = out [16, 1] feeds the
           next step's scalar directly from PSUM.
"""

import numpy as np
import ml_dtypes

B, T, H = 128, 2048, 200
NCORES = 8
NB = B // NCORES  # 16 batches per core
NSTEPS = 5
G4 = 4 * H  # 800 gate width
NCH = T // 128  # 16 stride-class chunks (t = 16*k + c -> partition k, chunk c)

_CACHE = {}

# M-packed kproj/tanh layout: batch b's 200 h-rows live at packed rows
# [224*b, 224*b+200) (stride 224 = 7*32 keeps every PE write 32-aligned,
# 16*224 = 3584 = 28 full 128-row tiles); h_out padded 200->224 with zeros.
BSTRIDE = 224
NTILE = NB * BSTRIDE // 128  # 28


def _legal_pieces(off, ln):
    """Split (off, ln) into PE-legal out placements: base 0 takes up to 128
    rows, base 64 up to 64, bases 32/96 up to 32."""
    out = []
    while ln > 0:
        cap = {0: 128, 32: 32, 64: 64, 96: 32}[off % 128]
        take = min(ln, cap)
        out.append((off, take))
        off += take
        ln -= take
    return out


def _batch_tiles(b):
    return sorted({ti for ti, _, _, _ in _segments(b)})


# columns of the per-(batch,tile) zero-masked Va block, in emission order
VA_COLS = {}
_c = 0
for _b in range(16):
    for _ti in range(28):
        if 224 * _b < 128 * (_ti + 1) and 128 * _ti < 224 * _b + 224:
            VA_COLS[(_b, _ti)] = _c
            _c += 1
NV = _c


def _segments(b):
    """Packed-row segments of batch b: (tile, off, hlo, ln) covering
    packed rows [BSTRIDE*b, BSTRIDE*(b+1)), split at 128-row tile edges."""
    out = []
    r0 = BSTRIDE * b
    r = r0
    while r < r0 + BSTRIDE:
        ti, off = divmod(r, 128)
        ln = min(128 - off, r0 + BSTRIDE - r)
        out.append((ti, off, r - r0, ln))
        r += ln
    return out


def _sub_segments(b):
    """Real-h sub-segments of batch b for the scores matmuls:
    (tile, off, hlo, ln) additionally split at h=128 and clipped at h=200."""
    out = []
    for ti, off, hlo, ln in _segments(b):
        ln = min(ln, 200 - hlo)
        if ln <= 0:
            continue
        pieces = []
        if hlo < 128 < hlo + ln:
            cut = 128 - hlo
            pieces = [(off, hlo, cut), (off + cut, 128, ln - cut)]
        else:
            pieces = [(off, hlo, ln)]
        for poff, phlo, pln in pieces:
            # stationary reads obey the same base/size tiling rule as writes
            for qoff, qln in _legal_pieces(poff, pln):
                out.append((ti, qoff, phlo + (qoff - poff), qln))
    return out

BF16 = ml_dtypes.bfloat16
FP8 = ml_dtypes.float8_e4m3fn

# packed attention-weight tensor column layout (bf16, [128, PK_C])
PK_WA0, PK_WA1 = 0, 224          # wa0 [128,224] | wa1 [72,224] (224 = padded)
PK_VA0, PK_VA1 = 448, 449        # va columns
PK_QT0, PK_QT1 = 450, 466        # qt [*,16]
PK_QBR = 482                     # qb as a [1, 224] row (for K=1 bias matmuls)
PK_VAPK = 706                    # zero-masked Va per (batch, tile) [128, NV]
PK_C = 706 + 44

# packed decoder-weight tensor column layout (bf16, [128, DK_C])
DK_WIHC0, DK_WIHC1 = 0, 800      # wihc0 [128,800] | wihc1 [73,800] (row72=bias)
DK_WHH0, DK_WHH1 = 1600, 2400    # whh [128|72, 800]
DK_WXR = 3200                    # wxr [16, 800]
DK_W1T0, DK_W1T1 = 4000, 4100    # w1t [128|73, 100] (row72=b1)
DK_W2T = 4200                    # w2t [101, 50] (row100=b2)
DK_W3T = 4250                    # w3t [51, 1] (row50=b3)
DK_C0 = 4251                     # c0 [16, 200]
DK_C = 4451


def _build_module():
    """Build the Bass module (same NEFF for all 8 cores)."""
    from contextlib import ExitStack

    import concourse.bass as bass
    import concourse.tile as tile
    from concourse import bacc, mybir
    from concourse.masks import make_identity

    dt = mybir.dt
    AF = mybir.ActivationFunctionType
    OP = mybir.AluOpType

    nc = bacc.Bacc(
        "TRN2",
        target_bir_lowering=False,
        debug=False,
        num_devices=NCORES,
    )

    # ---- DRAM tensors (per-core shards; weights replicated) ----
    d_encT = nc.dram_tensor(
        "encTp", [NB, 128, 2 * T], dt.float8e4, kind="ExternalInput"
    ).ap()  # DoubleRow packing: col i*T+t, partition p <-> h_in = i*128+p
    d_encN = nc.dram_tensor("encN", [NB, T, H], dt.bfloat16, kind="ExternalInput").ap()
    d_x0 = nc.dram_tensor("x0s", [NB, 1], dt.float32, kind="ExternalInput").ap()
    d_UaT = nc.dram_tensor("UaTp", [128, 2 * 352], dt.float8e4, kind="ExternalInput").ap()
    d_wpk = nc.dram_tensor("wpk", [128, PK_C], dt.bfloat16, kind="ExternalInput").ap()
    d_dpk = nc.dram_tensor("dpk", [128, DK_C], dt.bfloat16, kind="ExternalInput").ap()
    d_ones_b = nc.dram_tensor("ones_b", [1, NB], dt.bfloat16, kind="ExternalInput").ap()
    d_y = nc.dram_tensor("y2", [NB, NSTEPS], dt.float32, kind="ExternalOutput").ap()

    H0, H1 = 128, H - 128  # 128 + 72 partition chunks of the hidden dim

    with tile.TileContext(nc) as tc, ExitStack() as ctx:
        # ---------- persistent pools ----------
        wpool = ctx.enter_context(tc.tile_pool(name="weights", bufs=1))
        spool = ctx.enter_context(tc.tile_pool(name="smalls", bufs=1))

        # warmup: preload the tanh/exp ACT table set while DMAs stream
        wt_a = spool.tile([1, 8], dt.float32)
        nc.vector.memset(wt_a[:], 0.0)
        wt_b = spool.tile([1, 8], dt.float32)
        nc.scalar.activation(wt_b[:], wt_a[:], AF.Tanh)

        # identity for the decoder's h transposes (bf16)
        id_bf = wpool.tile([128, 128], dt.bfloat16)
        make_identity(nc, id_bf[:])

        # ones columns/rows for the tiny PE reductions (sliced on read)
        ones_c_bf = wpool.tile([128, 1], dt.bfloat16)
        nc.vector.memset(ones_c_bf[:], 1.0)
        ones_c_f = wpool.tile([128, 1], dt.float32)
        nc.vector.memset(ones_c_f[:], 1.0)
        ones_sq_f = wpool.tile([NCH, 128], dt.float32)
        nc.vector.memset(ones_sq_f[:], 1.0)

        # packed attention weights: one DMA instead of ten
        wpk = wpool.tile([128, PK_C], dt.bfloat16)
        nc.scalar.dma_start(wpk[:], d_wpk[:, :])
        wa0 = wpk[:, PK_WA0 : PK_WA0 + BSTRIDE]
        wa1 = wpk[0:H1, PK_WA1 : PK_WA1 + BSTRIDE]
        va_pk = wpk[:, PK_VAPK : PK_VAPK + NV]
        qt0 = wpk[:, PK_QT0 : PK_QT0 + NB]
        qt1 = wpk[0:H1, PK_QT1 : PK_QT1 + NB]

        ua_p = wpool.tile([128, 2 * 352], dt.float8e4)
        ua3 = ua_p[:].rearrange("p (i m) -> p i m", i=2)

        # packed decoder weights: one DMA (deferred below, behind first encT)
        dpk = wpool.tile([128, DK_C], dt.bfloat16)
        wihc0 = dpk[:, DK_WIHC0 : DK_WIHC0 + G4]
        wihc1 = dpk[0 : H1 + 1, DK_WIHC1 : DK_WIHC1 + G4]
        whh0 = dpk[:, DK_WHH0 : DK_WHH0 + G4]
        whh1 = dpk[0:H1, DK_WHH1 : DK_WHH1 + G4]
        wxr_sb = dpk[0:NB, DK_WXR : DK_WXR + G4]
        w1t0 = dpk[:, DK_W1T0 : DK_W1T0 + 100]
        w1t1 = dpk[0 : H1 + 1, DK_W1T1 : DK_W1T1 + 100]
        w2t = dpk[0:101, DK_W2T : DK_W2T + 50]
        w3t = dpk[0:51, DK_W3T : DK_W3T + 1]
        c0_sb = dpk[0:NB, DK_C0 : DK_C0 + H]

        # ---------- phase 0: packed qproj bias: qp_pk[r, ti] = ----------
        # qproj[h(r,ti), b(r,ti)] + qb[h]; pad rows get tanh(0*scale+qb-row)
        qp_pk_sb = spool.tile([128, NTILE], dt.float32)
        ones1_b = wpool.tile([1, 1], dt.bfloat16)
        nc.vector.memset(ones1_b[:], 1.0)
        with tc.tile_pool(name="qp_psum", bufs=1, space="PSUM") as qp_ps:
            ps = qp_ps.tile([128, NTILE], dt.float32, tag="qp")
            for b in range(NB):
                for ti, off, hlo, ln in _segments(b):
                    for poff, pln in _legal_pieces(off, ln):
                        phlo = hlo + (poff - off)
                        dst = ps[poff : poff + pln, ti : ti + 1]
                        nc.tensor.matmul(
                            dst, wa0[:, phlo : phlo + pln], qt0[:, b : b + 1],
                            start=True, stop=False, tile_position=(0, poff),
                        )
                        nc.tensor.matmul(
                            dst, wa1[:, phlo : phlo + pln], qt1[:, b : b + 1],
                            start=False, stop=False, tile_position=(0, poff),
                        )
                        # + qb (K=1 outer with the qb row)
                        nc.tensor.matmul(
                            dst, wpk[0:1, PK_QBR + phlo : PK_QBR + phlo + pln],
                            ones1_b[:], start=False, stop=True,
                            tile_position=(0, poff),
                        )
            nc.vector.tensor_copy(qp_pk_sb[:], ps[:])

        # G0 PSUM lives from phase 2 through the decoder; allocate its pool
        # ahead of the attention pools so releases stay LIFO
        g0_pool = ctx.enter_context(tc.tile_pool(name="g0_psum", bufs=1, space="PSUM"))
        gp = g0_pool.tile([NB, G4], dt.float32, tag="g0")

        # ---------- phase 1: kproj + tanh + scores + per-wave softmax/ctx ----
        encT_pool = ctx.enter_context(tc.tile_pool(name="encT_pool", bufs=3))
        e_pool = ctx.enter_context(tc.tile_pool(name="e_pool", bufs=4))
        encN_pool = ctx.enter_context(tc.tile_pool(name="encN_pool", bufs=16))
        from contextlib import ExitStack as _ES
        att_ctx = _ES()
        sc_pool = att_ctx.enter_context(tc.tile_pool(name="sc_psum", bufs=1, space="PSUM"))
        ct_pool = att_ctx.enter_context(tc.tile_pool(name="ct_psum", bufs=1, space="PSUM"))
        sc = sc_pool.tile([128, NB * NCH + NB], dt.float32, tag="sc")
        p_sb = spool.tile([128, NB * NCH], dt.bfloat16)
        zc = sc[0:NCH, NB * NCH : NB * NCH + NB]
        # one PSUM tile (PSUM tiles are bank-granular): ct0 | ct1 columns
        ctz = ct_pool.tile([128, 2 * NB], dt.float32, tag="ctz")
        ct0_ps = ctz[:, 0:NB]
        ct1_ps = ctz[0:H1, NB : 2 * NB]

        en_tiles = []
        kp_hist = {}  # batch -> last kproj matmul (encN pacing anchor)
        e_pk = []     # packed e tiles, one per 128-row tile

        def emit_scores(b):
            tiles = _batch_tiles(b)
            for c in range(NCH):
                col = b * NCH + c
                for k, ti in enumerate(tiles):
                    # full-tile contraction; va column is zero outside
                    # batch b's rows, so other batches contribute nothing
                    vcol = VA_COLS[(b, ti)]
                    nc.tensor.matmul(
                        sc[:, col : col + 1],
                        e_pk[ti][:, c : T : NCH],
                        va_pk[:, vcol : vcol + 1],
                        start=(k == 0),
                        stop=(k == len(tiles) - 1),
                    )

        def emit_wave(w):
            # exp + Z colsums + context for batches 4w..4w+3 (scores ready)
            lo = 4 * w * NCH
            nc.scalar.activation(
                p_sb[:, lo : lo + 4 * NCH], sc[:, lo : lo + 4 * NCH], AF.Exp
            )
            for b in range(4 * w, 4 * w + 4):
                nc.tensor.matmul(
                    zc[:, b : b + 1],
                    p_sb[:, b * NCH : (b + 1) * NCH],
                    ones_c_bf[:],
                    start=True,
                    stop=True,
                )
                # complete each accumulation chain before starting the next
                # (two open groups in one PSUM bank trip the zero-region rule)
                for c in range(NCH):
                    nc.tensor.matmul(
                        ct0_ps[:, b : b + 1],
                        en_tiles[b][:, c * H : c * H + H0],
                        p_sb[:, b * NCH + c : b * NCH + c + 1],
                        start=(c == 0),
                        stop=(c == NCH - 1),
                    )
                for c in range(NCH):
                    nc.tensor.matmul(
                        ct1_ps[:, b : b + 1],
                        en_tiles[b][:, c * H + H0 : (c + 1) * H],
                        p_sb[:, b * NCH + c : b * NCH + c + 1],
                        start=(c == 0),
                        stop=(c == NCH - 1),
                    )

        # per-tile segment map and per-batch last tile
        tile_segs = [[] for _ in range(NTILE)]
        for b in range(NB):
            for ti, off, hlo, ln in _segments(b):
                tile_segs[ti].append((b, off, hlo, ln))
        last_tile = [_segments(b)[-1][0] for b in range(NB)]

        import bass_rust as _br
        et_tiles = {}
        scores_done = -1  # highest batch whose scores are emitted

        def load_batch(b):
            et = encT_pool.tile([128, 2 * T], dt.float8e4, tag="et", name=f"et{b}")
            nc.sync.dma_start(et[:], d_encT[b, :])
            et_tiles[b] = et[:].rearrange("p (i t) -> p i t", i=2)
            if b == 0:
                nc.sync.dma_start(ua_p[:], d_UaT[:, :])
            if b == 3:
                # single packed decoder-weight DMA, off the startup path
                nc.sync.dma_start(dpk[:], d_dpk[:, :])

        with tc.tile_pool(name="kp_psum", bufs=2, space="PSUM") as kp_ps:
            load_batch(0)
            for ti in range(NTILE):
                for b, _, _, _ in tile_segs[ti]:
                    if b not in et_tiles:
                        load_batch(b)
                e = e_pool.tile([128, T], dt.bfloat16, tag="e", name=f"e{ti}")
                e_pk.append(e)
                # plan: wide DoubleRow computes the whole tile from the
                # zero-extended Ua (garbage rows), later pieces overwrite.
                # DR only encodes dst at base 0 with 32/64/128-row groups.
                segs = tile_segs[ti]
                plan = []  # (kind, b, off, ln, ua_lo) kind: dr | wide | ndr
                if len(segs) == 1:
                    b0_, off0_, hlo0_, ln0_ = segs[0]
                    plan.append(("dr", b0_, 0, 128, 96 + hlo0_))
                else:
                    (bx, ox, hx, lx), (by, oy, hy, ly) = segs[0], segs[1]
                    if lx in (32, 64):
                        # wide(Y) first, DR(X) overwrites rows 0..lx
                        plan.append(("wide", by, 0, 128, 96 + hy - oy))
                        plan.append(("dr", bx, 0, lx, 96 + hx))
                    else:
                        # wide(X) first, non-DR(Y) overwrites the top rows
                        plan.append(("wide", bx, 0, 128, 96 + hx))
                        plan.append(("ndr", by, oy, ly, 96 + hy))
                for th in range(2):
                    ps = kp_ps.tile([128, 1024], dt.float32, tag="kp")
                    for kind, b, poff, pln, ua_lo in plan:
                        for n in range(2):
                            c0c = th * 1024 + n * 512
                            dst = ps[poff : poff + pln, n * 512 : (n + 1) * 512]
                            if kind in ("dr", "wide"):
                                i_kp = nc.tensor.matmul(
                                    dst,
                                    ua3[:, :, ua_lo : ua_lo + pln],
                                    et_tiles[b][:, :, c0c : c0c + 512],
                                    start=True,
                                    stop=True,
                                    perf_mode=mybir.MatmulPerfMode.DoubleRow,
                                    tile_position=(0, 0),
                                )
                            else:
                                for i in range(2):
                                    i_kp = nc.tensor.matmul(
                                        dst,
                                        ua3[:, i, ua_lo : ua_lo + pln],
                                        et_tiles[b][:, i, c0c : c0c + 512],
                                        start=(i == 0),
                                        stop=(i == 1),
                                        tile_position=(0, poff),
                                    )
                            kp_hist[b] = i_kp
                    # e = tanh(kproj/64 + qp_pk[:, ti]) ; write bf16
                    nc.scalar.activation(
                        e[:, th * 1024 : (th + 1) * 1024],
                        ps[:],
                        AF.Tanh,
                        bias=qp_pk_sb[:, ti : ti + 1],
                        scale=1.0 / 64.0,
                    )
                # scores/waves with a one-tile skew; encN paced ~2 batches back
                for b in range(scores_done + 1, NB):
                    if last_tile[b] != ti - 1:
                        break
                    emit_scores(b)
                    scores_done = b
                    en = encN_pool.tile(
                        [128, NCH * H], dt.bfloat16, name=f"en{b}", tag="en"
                    )
                    i_en = nc.gpsimd.dma_start(
                        en[:],
                        d_encN[b].rearrange("(p n) h -> p (n h)", p=128),
                    )
                    _br.add_dep_helper(
                        i_en.ins, kp_hist[max(0, b - 2)].ins, sync=True,
                        reason="encN paced behind kproj two batches back",
                    )
                    en_tiles.append(en)
                    if b % 4 == 3:
                        emit_wave(b // 4)
            for b in range(scores_done + 1, NB):
                emit_scores(b)
                if b >= len(en_tiles):
                    en = encN_pool.tile(
                        [128, NCH * H], dt.bfloat16, name=f"en{b}", tag="en"
                    )
                    nc.gpsimd.dma_start(
                        en[:],
                        d_encN[b].rearrange("(p n) h -> p (n h)", p=128),
                    )
                    en_tiles.append(en)
                if b % 4 == 3:
                    emit_wave(b // 4)

        # ---------- phase 2: Z totals, 1/Z, ctx scale, G0 ----------
        ct0 = spool.tile([H0, NB], dt.bfloat16)
        ct1 = spool.tile([H1 + 1, NB], dt.bfloat16)  # row 72 = ones (bias row)
        nc.scalar.dma_start(ct1[H1 : H1 + 1, :], d_ones_b[:, :])

        with tc.tile_pool(name="z_psum", bufs=1, space="PSUM") as z_ps:
            # q-dependent G0 terms: no ctx dependency, run right away
            for n, nsz in [(0, 512), (512, G4 - 512)]:
                nc.tensor.matmul(
                    gp[:, n : n + nsz], qt0, whh0[:, n : n + nsz],
                    start=True, stop=False,
                )
                nc.tensor.matmul(
                    gp[:, n : n + nsz], qt1, whh1[:, n : n + nsz],
                    start=False, stop=False,
                )
            # Z per batch broadcast down all 128 partitions in one matmul
            # (lhsT = ones [16, 128] -> out[r, b] = sum_ch zc[ch, b]), then
            # reciprocal straight into SBUF
            zc_sb = spool.tile([NCH, NB], dt.float32)
            nc.vector.tensor_copy(zc_sb[:], zc[:])
            zbc = z_ps.tile([128, NB], dt.float32, tag="zbc")
            nc.tensor.matmul(zbc[:], ones_sq_f[:], zc_sb[:], start=True, stop=True)
            rzb_sb = spool.tile([128, NB], dt.float32)
            nc.vector.reciprocal(rzb_sb[:], zbc[:])
            # normalize: ctxT = ctx_raw * (1/Z) broadcast, cast bf16
            nc.vector.tensor_tensor(ct0[:], ct0_ps[:], rzb_sb[:], op=OP.mult)
            nc.vector.tensor_tensor(
                ct1[0:H1, :], ct1_ps[:], rzb_sb[0:H1, :], op=OP.mult
            )
            for n, nsz in [(0, 512), (512, G4 - 512)]:
                nc.tensor.matmul(
                    gp[:, n : n + nsz], ct0[:], wihc0[:, n : n + nsz],
                    start=False, stop=False,
                )
                nc.tensor.matmul(
                    gp[:, n : n + nsz], ct1, wihc1[:, n : n + nsz],
                    start=False, stop=True,
                )
        att_ctx.close()  # release sc/ct PSUM banks before the decoder

        # ---------- phase 3: decoder steps (all bf16, gate order f,i,o|g) ---
        x_sb = spool.tile([NB, 1], dt.float32)
        nc.sync.dma_start(x_sb[:], d_x0[:, :])
        xn_all = spool.tile([NB, NSTEPS], dt.float32)
        ht0 = spool.tile([H0, NB], dt.bfloat16)
        ht1 = spool.tile([H1 + 1, NB], dt.bfloat16)  # row 72 = ones (b1 row)
        nc.scalar.dma_start(ht1[H1 : H1 + 1, :], d_ones_b[:, :])
        o1t = spool.tile([101, NB], dt.bfloat16)  # row 100 = ones (b2 row)
        nc.scalar.dma_start(o1t[100:101, :], d_ones_b[:, :])
        o2t = spool.tile([51, NB], dt.bfloat16)  # row 50 = ones (b3 row)
        nc.scalar.dma_start(o2t[50:51, :], d_ones_b[:, :])

        with (
            tc.tile_pool(name="ls", bufs=2) as ls,
            tc.tile_pool(name="ls_psum", bufs=3, space="PSUM") as lp,
            tc.tile_pool(name="m3_psum", bufs=2, space="PSUM") as mp,
        ):
            xt = x_sb
            for t in range(NSTEPS):
                # gates = g0 + x * wxr, fused on DVE; split f,i,o vs g so
                # the sigmoid can start before the g slice is computed
                gates = ls.tile([NB, G4], dt.bfloat16, tag="gates")
                for glo, ghi in ((0, 2 * H), (3 * H, 4 * H), (2 * H, 3 * H)):
                    nc.vector.scalar_tensor_tensor(
                        gates[:, glo:ghi], wxr_sb[:, glo:ghi], xt[:, 0:1],
                        gp[:, glo:ghi], op0=OP.mult, op1=OP.add,
                    )
                # f,i sigmoid first (feeds t1/t2), then g tanh, then o
                sfio = ls.tile([NB, 3 * H], dt.bfloat16, tag="sfio")
                nc.scalar.activation(sfio[:, 0 : 2 * H], gates[:, 0 : 2 * H], AF.Sigmoid)
                tg = ls.tile([NB, H], dt.bfloat16, tag="tg")
                nc.scalar.activation(tg[:], gates[:, 3 * H : 4 * H], AF.Tanh)
                nc.scalar.activation(
                    sfio[:, 2 * H : 3 * H], gates[:, 2 * H : 3 * H], AF.Sigmoid
                )
                t1 = ls.tile([NB, H], dt.bfloat16, tag="t1")
                nc.vector.tensor_tensor(t1[:], sfio[:, 0:H], c0_sb, op=OP.mult)
                t2 = ls.tile([NB, H], dt.bfloat16, tag="t2")
                nc.vector.tensor_tensor(t2[:], sfio[:, H : 2 * H], tg[:], op=OP.mult)
                cn = ls.tile([NB, H], dt.bfloat16, tag="cn")
                nc.vector.tensor_tensor(cn[:], t1[:], t2[:], op=OP.add)
                tcn = ls.tile([NB, H], dt.bfloat16, tag="tcn")
                nc.scalar.activation(tcn[:], cn[:], AF.Tanh)
                hh = ls.tile([NB, H], dt.bfloat16, tag="hh")
                nc.vector.tensor_tensor(hh[:], sfio[:, 2 * H : 3 * H], tcn[:], op=OP.mult)
                # transpose h -> ht0/ht1 (feature-major for the MLP); relu
                # folded into the PSUM->SBUF copies (DVE max / ACT relu)
                tp0 = lp.tile([128, NB], dt.bfloat16, tag="lsps")
                nc.tensor.transpose(tp0[:], hh[:, 0:H0], id_bf[0:NB, 0:NB])
                nc.vector.tensor_scalar_max(ht0[:], tp0[:], 0.0)
                tp1 = lp.tile([128, NB], dt.bfloat16, tag="lsps")
                nc.tensor.transpose(tp1[0:H1, :], hh[:, H0:H], id_bf[0:NB, 0:NB])
                nc.scalar.activation(ht1[0:H1, :], tp1[0:H1, :], AF.Relu)
                # MLP in feature-major, biases via ones rows
                m1 = lp.tile([100, NB], dt.float32, tag="lsps")
                nc.tensor.matmul(m1[:], w1t0, ht0[:], start=True, stop=False)
                nc.tensor.matmul(m1[:], w1t1, ht1[:], start=False, stop=True)
                nc.vector.tensor_scalar_max(o1t[0:100, :], m1[:], 0.0)
                m2 = lp.tile([50, NB], dt.float32, tag="lsps")
                nc.tensor.matmul(m2[:], w2t, o1t[:], start=True, stop=True)
                nc.vector.tensor_scalar_max(o2t[0:50, :], m2[:], 0.0)
                # m3 flipped: o2 stationary, w3 moving -> out [NB, 1] is
                # directly the next step's x (read from PSUM as STT scalar)
                m3 = mp.tile([NB, 1], dt.float32, tag="m3")
                nc.tensor.matmul(m3[:], o2t[:], w3t, start=True, stop=True)
                nc.vector.tensor_copy(xn_all[:, t : t + 1], m3[:])
                xt = m3
            nc.sync.dma_start(d_y[:, :], xn_all[:])

    # Bacc lowering: register allocation + wait splitting (<=1 wait/inst on HW)
    nc.compile()
    return nc


def _prep_inputs(x, h0, c0, encoder_output, Wa, ba, Ua, bua, Va, bva,
                 W_ih, W_hh, b_ih, b_hh, W1, b1, W2, b2, W3, b3):
    """Host-side layout prep -> list of per-core input maps."""
    f32 = np.float32
    enc = np.ascontiguousarray(encoder_output, dtype=f32)
    q = np.asarray(h0, dtype=f32)[0]          # [B, H]
    c0f = np.asarray(c0, dtype=f32)[0]        # [B, H]
    x0 = np.asarray(x, dtype=f32).reshape(B, 1)

    # gate permutation: torch order (i,f,g,o) -> device order (f,i,o,g) so
    # one sigmoid instr covers f,i,o and tanh covers g
    gperm = np.concatenate(
        [np.arange(H, 2 * H), np.arange(0, H), np.arange(3 * H, 4 * H),
         np.arange(2 * H, 3 * H)]
    )
    W_ihp = np.asarray(W_ih, f32)[gperm]
    W_hhp = np.asarray(W_hh, f32)[gperm]
    bp = (np.asarray(b_ih, f32) + np.asarray(b_hh, f32))[gperm]

    # Ua scaled x64 into fp8 comfortable range; kernel rescales inside tanh.
    # DoubleRow packing: partition p holds h_in = p (i=0) and 128+p (i=1);
    # M padded 200->208 so the pair stride is 16B-aligned.
    uaT = np.asarray(Ua, f32).T * 64.0  # [h_in, h_out]
    # h_out axis zero-extended left by 96 and right to 352 so "wide" DoubleRow
    # matmuls can compute whole 128-row tiles with garbage rows outside a
    # segment (overwritten by later pieces)
    uaT_pad = np.zeros((256, 352), f32)
    uaT_pad[0:H, 96 : 96 + H] = uaT
    uaT_p = np.stack([uaT_pad[0:128], uaT_pad[128:256]], axis=1)  # [128, 2, 352]

    # packed attention weights
    wpk = np.zeros((128, PK_C), f32)
    waT = np.asarray(Wa, f32).T  # [h_in, h_out]
    wpk[:, PK_WA0 : PK_WA0 + 200] = waT[0:128]
    wpk[0:72, PK_WA1 : PK_WA1 + 200] = waT[128:200]
    va = np.asarray(Va, f32)[0]
    for (bb, ti), cidx in VA_COLS.items():
        r = np.arange(128)
        h = 128 * ti + r - BSTRIDE * bb
        mask = (h >= 0) & (h < H)
        colv = np.zeros(128, f32)
        colv[mask] = va[h[mask]]
        wpk[:, PK_VAPK + cidx] = colv
    qb = np.asarray(ba, f32) + np.asarray(bua, f32)
    wpk[0, PK_QBR : PK_QBR + H] = qb

    # packed decoder weights
    dpk = np.zeros((128, DK_C), f32)
    w_ihcT = W_ihp[:, 1:].T  # [H, G4]
    dpk[:, DK_WIHC0 : DK_WIHC0 + G4] = w_ihcT[0:128]
    dpk[0:72, DK_WIHC1 : DK_WIHC1 + G4] = w_ihcT[128:200]
    dpk[72, DK_WIHC1 : DK_WIHC1 + G4] = bp
    w_hhT = W_hhp.T
    dpk[:, DK_WHH0 : DK_WHH0 + G4] = w_hhT[0:128]
    dpk[0:72, DK_WHH1 : DK_WHH1 + G4] = w_hhT[128:200]
    dpk[0:NB, DK_WXR : DK_WXR + G4] = np.broadcast_to(
        W_ihp[:, 0].reshape(1, G4), (NB, G4)
    )
    w1T = np.asarray(W1, f32).T
    dpk[:, DK_W1T0 : DK_W1T0 + 100] = w1T[0:128]
    dpk[0:72, DK_W1T1 : DK_W1T1 + 100] = w1T[128:200]
    dpk[72, DK_W1T1 : DK_W1T1 + 100] = np.asarray(b1, f32)
    dpk[0:100, DK_W2T : DK_W2T + 50] = np.asarray(W2, f32).T
    dpk[100, DK_W2T : DK_W2T + 50] = np.asarray(b2, f32)
    dpk[0:50, DK_W3T] = np.asarray(W3, f32)[0]
    dpk[50, DK_W3T] = np.asarray(b3, f32)[0]

    shared = {
        "UaTp": np.ascontiguousarray(uaT_p.reshape(128, 2 * 352)).astype(FP8),
        "ones_b": np.ones((1, NB), BF16),
    }

    in_maps = []
    for cix in range(NCORES):
        bs = slice(cix * NB, (cix + 1) * NB)
        enc_c = enc[bs]  # [NB, T, H]
        m = dict(shared)
        encT_c = enc_c.transpose(0, 2, 1)  # [NB, H, T]
        encT_pad = np.concatenate(
            [encT_c, np.zeros((NB, 56, T), f32)], axis=1
        )  # [NB, 256, T]
        encT_p = np.stack([encT_pad[:, 0:128], encT_pad[:, 128:256]], axis=2)
        m["encTp"] = np.ascontiguousarray(encT_p.reshape(NB, 128, 2 * T)).astype(FP8)
        m["encN"] = enc_c.astype(BF16)
        dpk_c = dpk.copy()
        dpk_c[0:NB, DK_C0 : DK_C0 + H] = c0f[bs]
        m["dpk"] = dpk_c.astype(BF16)
        wpk_c = wpk.copy()
        qTc = q[bs].T  # [H, NB]
        wpk_c[:, PK_QT0 : PK_QT0 + NB] = qTc[0:128]
        wpk_c[0:72, PK_QT1 : PK_QT1 + NB] = qTc[128:200]
        m["wpk"] = wpk_c.astype(BF16)
        m["x0s"] = np.ascontiguousarray(x0[bs])
        in_maps.append(m)
    return in_maps


def kernel(**inputs):
    from concourse.bass_utils import run_bass_kernel_spmd

    if "nc" not in _CACHE:
        _CACHE["nc"] = _build_module()
    nc = _CACHE["nc"]

    in_maps = _prep_inputs(**inputs)
    res = run_bass_kernel_spmd(nc, in_maps, core_ids=list(range(NCORES)))
    # y2 per core: [NB, NSTEPS] -> full output [B, NSTEPS]
    out = np.concatenate([r["y2"] for r in res.results], axis=0)
    return np.ascontiguousarray(out.astype(np.float32))


# revision 31
# speedup vs baseline: 1.1431x; 1.0085x over previous
"""Trainium2 Bass kernel for nn_DecoderAttention (Bahdanau attention + LSTM decoder).

Data-parallel over batch: B=128 split across 8 NeuronCores (16 batches/core).
All FLOPs run on device; the host only reshuffles layouts (transpose / dtype
cast / fp8 DoubleRow packing / weight concat with bias rows folded in).

Per-core device pipeline (cost-model-aware layout):
  phase 0: ONE packed DMA for all small attention weights (+one fp32 qb DMA),
           qprojT = Wa @ q^T (+ ba + bua) on PE
  phase 1: per batch b: kprojT = Ua @ enc_b^T as fp8 DoubleRow matmuls
           (K=200 packed 2/partition, one pass, 0.5 cyc/row),
           tanh(kproj/64 + qprojT[:, b]) on ACT -> e tiles [h, t] bf16;
           scores as FLIPPED matmuls: e chunk stationary (K=h, M=128 t's of
           one stride-16 class), Va moving (N=1) -> scores accumulate into
           one [128, 256] PSUM tile, column 16*b+c;
           per WAVE of 4 batches: exp slice [128, 64] -> p (bf16,
           unnormalized), per-batch colsum matmuls -> zc, context via FLIPPED
           matmuls (encN chunk stationary K=t, p column moving N=1) -> ctxT
           accumulates [h, b] in PSUM. All of it hides inside phase 1.
  phase 2 (tail): Z totals via two tiny PE reductions, 1/Z broadcast via a
           K=1 outer-product matmul, ctxT scale on DVE, G0 closes an
           accumulation whose q-terms ran during phase 1.
  phase 3: 5 serial decoder steps, all-bf16 elementwise:
           gates = G0(PSUM) + x*wxr fused on DVE (scalar_tensor_tensor,
           split f,i,o vs g), gate order permuted so one sigmoid covers
           f,i,o; MLP in bf16; m3 flipped so x_next = out [16, 1] feeds the
           next step's scalar directly from PSUM.
"""

import numpy as np
import ml_dtypes

B, T, H = 128, 2048, 200
NCORES = 8
NB = B // NCORES  # 16 batches per core
NSTEPS = 5
G4 = 4 * H  # 800 gate width
NCH = T // 128  # 16 stride-class chunks (t = 16*k + c -> partition k, chunk c)

_CACHE = {}

# M-packed kproj/tanh layout: batch b's 200 h-rows live at packed rows
# [224*b, 224*b+200) (stride 224 = 7*32 keeps every PE write 32-aligned,
# 16*224 = 3584 = 28 full 128-row tiles); h_out padded 200->224 with zeros.
BSTRIDE = 224
NTILE = NB * BSTRIDE // 128  # 28


def _legal_pieces(off, ln):
    """Split (off, ln) into PE-legal out placements: base 0 takes up to 128
    rows, base 64 up to 64, bases 32/96 up to 32."""
    out = []
    while ln > 0:
        cap = {0: 128, 32: 32, 64: 64, 96: 32}[off % 128]
        take = min(ln, cap)
        out.append((off, take))
        off += take
        ln -= take
    return out


def _batch_tiles(b):
    return sorted({ti for ti, _, _, _ in _segments(b)})


# columns of the per-(batch,tile) zero-masked Va block, in emission order
VA_COLS = {}
_c = 0
for _b in range(16):
    for _ti in range(28):
        if 224 * _b < 128 * (_ti + 1) and 128 * _ti < 224 * _b + 224:
            VA_COLS[(_b, _ti)] = _c
            _c += 1
NV = _c


def _segments(b):
    """Packed-row segments of batch b: (tile, off, hlo, ln) covering
    packed rows [BSTRIDE*b, BSTRIDE*(b+1)), split at 128-row tile edges."""
    out = []
    r0 = BSTRIDE * b
    r = r0
    while r < r0 + BSTRIDE:
        ti, off = divmod(r, 128)
        ln = min(128 - off, r0 + BSTRIDE - r)
        out.append((ti, off, r - r0, ln))
        r += ln
    return out


def _sub_segments(b):
    """Real-h sub-segments of batch b for the scores matmuls:
    (tile, off, hlo, ln) additionally split at h=128 and clipped at h=200."""
    out = []
    for ti, off, hlo, ln in _segments(b):
        ln = min(ln, 200 - hlo)
        if ln <= 0:
            continue
        pieces = []
        if hlo < 128 < hlo + ln:
            cut = 128 - hlo
            pieces = [(off, hlo, cut), (off + cut, 128, ln - cut)]
        else:
            pieces = [(off, hlo, ln)]
        for poff, phlo, pln in pieces:
            # stationary reads obey the same base/size tiling rule as writes
            for qoff, qln in _legal_pieces(poff, pln):
                out.append((ti, qoff, phlo + (qoff - poff), qln))
    return out

BF16 = ml_dtypes.bfloat16
FP8 = ml_dtypes.float8_e4m3fn

# packed attention-weight tensor column layout (bf16, [128, PK_C])
PK_WA0, PK_WA1 = 0, 224          # wa0 [128,224] | wa1 [72,224] (224 = padded)
PK_VA0, PK_VA1 = 448, 449        # va columns
PK_QT0, PK_QT1 = 450, 466        # qt [*,16]
PK_QBR = 482                     # qb as a [1, 224] row (for K=1 bias matmuls)
PK_VAPK = 706                    # zero-masked Va per (batch, tile) [128, NV]
PK_C = 706 + 44

# packed decoder-weight tensor column layout (bf16, [128, DK_C])
DK_WIHC0, DK_WIHC1 = 0, 800      # wihc0 [128,800] | wihc1 [73,800] (row72=bias)
DK_WHH0, DK_WHH1 = 1600, 2400    # whh [128|72, 800]
DK_WXR = 3200                    # wxr [16, 800]
DK_W1T0, DK_W1T1 = 4000, 4100    # w1t [128|73, 100] (row72=b1)
DK_W2T = 4200                    # w2t [101, 50] (row100=b2)
DK_W3T = 4250                    # w3t [51, 1] (row50=b3)
DK_C0 = 4251                     # c0 [16, 200]
DK_C = 4451


def _build_module():
    """Build the Bass module (same NEFF for all 8 cores)."""
    from contextlib import ExitStack

    import concourse.bass as bass
    import concourse.tile as tile
    from concourse import bacc, mybir
    from concourse.masks import make_identity

    dt = mybir.dt
    AF = mybir.ActivationFunctionType
    OP = mybir.AluOpType

    nc = bacc.Bacc(
        "TRN2",
        target_bir_lowering=False,
        debug=False,
        num_devices=NCORES,
    )

    # ---- DRAM tensors (per-core shards; weights replicated) ----
    d_encT = nc.dram_tensor(
        "encTp", [NB, 128, 2 * T], dt.float8e4, kind="ExternalInput"
    ).ap()  # DoubleRow packing: col i*T+t, partition p <-> h_in = i*128+p
    d_encN = nc.dram_tensor("encN", [NB, T, H], dt.bfloat16, kind="ExternalInput").ap()
    d_x0 = nc.dram_tensor("x0s", [NB, 1], dt.float32, kind="ExternalInput").ap()
    d_UaT = nc.dram_tensor("UaTp", [128, 2 * 352], dt.float8e4, kind="ExternalInput").ap()
    d_wpk = nc.dram_tensor("wpk", [128, PK_C], dt.bfloat16, kind="ExternalInput").ap()
    d_dpk = nc.dram_tensor("dpk", [128, DK_C], dt.bfloat16, kind="ExternalInput").ap()
    d_ones_b = nc.dram_tensor("ones_b", [1, NB], dt.bfloat16, kind="ExternalInput").ap()
    d_y = nc.dram_tensor("y2", [NB, NSTEPS], dt.float32, kind="ExternalOutput").ap()

    H0, H1 = 128, H - 128  # 128 + 72 partition chunks of the hidden dim

    with tile.TileContext(nc) as tc, ExitStack() as ctx:
        # ---------- persistent pools ----------
        wpool = ctx.enter_context(tc.tile_pool(name="weights", bufs=1))
        spool = ctx.enter_context(tc.tile_pool(name="smalls", bufs=1))

        # warmup: preload the tanh/exp ACT table set while DMAs stream
        wt_a = spool.tile([1, 8], dt.float32)
        nc.vector.memset(wt_a[:], 0.0)
        wt_b = spool.tile([1, 8], dt.float32)
        nc.scalar.activation(wt_b[:], wt_a[:], AF.Tanh)

        # identity for the decoder's h transposes (bf16)
        id_bf = wpool.tile([128, 128], dt.bfloat16)
        make_identity(nc, id_bf[:])

        # ones columns/rows for the tiny PE reductions (sliced on read)
        ones_c_bf = wpool.tile([128, 1], dt.bfloat16)
        nc.vector.memset(ones_c_bf[:], 1.0)
        ones_c_f = wpool.tile([128, 1], dt.float32)
        nc.vector.memset(ones_c_f[:], 1.0)
        ones_sq_f = wpool.tile([NCH, 128], dt.float32)
        nc.vector.memset(ones_sq_f[:], 1.0)

        # packed attention weights: one DMA (triggered after et0/ua below so
        # the kproj-critical transfers go first on the serialized DMA engine)
        wpk = wpool.tile([128, PK_C], dt.bfloat16)
        wa0 = wpk[:, PK_WA0 : PK_WA0 + BSTRIDE]
        wa1 = wpk[0:H1, PK_WA1 : PK_WA1 + BSTRIDE]
        va_pk = wpk[:, PK_VAPK : PK_VAPK + NV]
        qt0 = wpk[:, PK_QT0 : PK_QT0 + NB]
        qt1 = wpk[0:H1, PK_QT1 : PK_QT1 + NB]

        ua_p = wpool.tile([128, 2 * 352], dt.float8e4)
        ua3 = ua_p[:].rearrange("p (i m) -> p i m", i=2)

        # packed decoder weights: one DMA (deferred below, behind first encT)
        dpk = wpool.tile([128, DK_C], dt.bfloat16)
        wihc0 = dpk[:, DK_WIHC0 : DK_WIHC0 + G4]
        wihc1 = dpk[0 : H1 + 1, DK_WIHC1 : DK_WIHC1 + G4]
        whh0 = dpk[:, DK_WHH0 : DK_WHH0 + G4]
        whh1 = dpk[0:H1, DK_WHH1 : DK_WHH1 + G4]
        wxr_sb = dpk[0:NB, DK_WXR : DK_WXR + G4]
        w1t0 = dpk[:, DK_W1T0 : DK_W1T0 + 100]
        w1t1 = dpk[0 : H1 + 1, DK_W1T1 : DK_W1T1 + 100]
        w2t = dpk[0:101, DK_W2T : DK_W2T + 50]
        w3t = dpk[0:51, DK_W3T : DK_W3T + 1]
        c0_sb = dpk[0:NB, DK_C0 : DK_C0 + H]

        # phase-1 encT pool + loader, defined early: the first encT/Ua DMA
        # triggers go BEFORE wpk on the serialized DMA engine, then wpk,
        # then the phase-0 matmuls that read wpk
        encT_pool = ctx.enter_context(tc.tile_pool(name="encT_pool", bufs=3))
        et_tiles = {}

        def load_batch(b):
            et = encT_pool.tile([128, 2 * T], dt.float8e4, tag="et", name=f"et{b}")
            nc.sync.dma_start(et[:], d_encT[b, :])
            et_tiles[b] = et[:].rearrange("p (i t) -> p i t", i=2)
            if b == 0:
                nc.sync.dma_start(ua_p[:], d_UaT[:, :])
            if b == 3:
                # single packed decoder-weight DMA, off the startup path
                nc.sync.dma_start(dpk[:], d_dpk[:, :])

        load_batch(0)
        nc.scalar.dma_start(wpk[:], d_wpk[:, :])

        # ---------- phase 0: packed qproj bias: qp_pk[r, ti] = ----------
        # qproj[h(r,ti), b(r,ti)] + qb[h]; pad rows get tanh(0*scale+qb-row)
        qp_pk_sb = spool.tile([128, NTILE], dt.float32)
        ones1_b = wpool.tile([1, 1], dt.bfloat16)
        nc.vector.memset(ones1_b[:], 1.0)
        with tc.tile_pool(name="qp_psum", bufs=1, space="PSUM") as qp_ps:
            ps = qp_ps.tile([128, NTILE], dt.float32, tag="qp")
            for b in range(NB):
                for ti, off, hlo, ln in _segments(b):
                    for poff, pln in _legal_pieces(off, ln):
                        phlo = hlo + (poff - off)
                        dst = ps[poff : poff + pln, ti : ti + 1]
                        nc.tensor.matmul(
                            dst, wa0[:, phlo : phlo + pln], qt0[:, b : b + 1],
                            start=True, stop=False, tile_position=(0, poff),
                        )
                        nc.tensor.matmul(
                            dst, wa1[:, phlo : phlo + pln], qt1[:, b : b + 1],
                            start=False, stop=False, tile_position=(0, poff),
                        )
                        # + qb (K=1 outer with the qb row)
                        nc.tensor.matmul(
                            dst, wpk[0:1, PK_QBR + phlo : PK_QBR + phlo + pln],
                            ones1_b[:], start=False, stop=True,
                            tile_position=(0, poff),
                        )
            nc.vector.tensor_copy(qp_pk_sb[:], ps[:])

        # ---------- phase 1: kproj + tanh + scores + per-wave softmax/ctx ----
        e_pool = ctx.enter_context(tc.tile_pool(name="e_pool", bufs=4))
        encN_pool = ctx.enter_context(tc.tile_pool(name="encN_pool", bufs=16))
        from contextlib import ExitStack as _ES
        att_ctx = _ES()
        sc_pool = att_ctx.enter_context(tc.tile_pool(name="sc_psum", bufs=1, space="PSUM"))
        ct_pool = att_ctx.enter_context(tc.tile_pool(name="ct_psum", bufs=1, space="PSUM"))
        sc = sc_pool.tile([128, NB * NCH + NB], dt.float32, tag="sc")
        p_sb = spool.tile([128, NB * NCH], dt.bfloat16)
        zc = sc[0:NCH, NB * NCH : NB * NCH + NB]
        # one PSUM tile (PSUM tiles are bank-granular): ct0 | ct1 columns
        ctz = ct_pool.tile([128, 2 * NB], dt.float32, tag="ctz")
        ct0_ps = ctz[:, 0:NB]
        ct1_ps = ctz[0:H1, NB : 2 * NB]

        en_tiles = []
        kp_hist = {}  # batch -> last kproj matmul (encN pacing anchor)
        e_pk = []     # packed e tiles, one per 128-row tile

        def emit_scores(b):
            tiles = _batch_tiles(b)
            for c in range(NCH):
                col = b * NCH + c
                for k, ti in enumerate(tiles):
                    # full-tile contraction; va column is zero outside
                    # batch b's rows, so other batches contribute nothing
                    vcol = VA_COLS[(b, ti)]
                    nc.tensor.matmul(
                        sc[:, col : col + 1],
                        e_pk[ti][:, c : T : NCH],
                        va_pk[:, vcol : vcol + 1],
                        start=(k == 0),
                        stop=(k == len(tiles) - 1),
                    )

        def emit_wave(w):
            # exp + Z colsums + context for batches 4w..4w+3 (scores ready)
            lo = 4 * w * NCH
            nc.scalar.activation(
                p_sb[:, lo : lo + 4 * NCH], sc[:, lo : lo + 4 * NCH], AF.Exp
            )
            for b in range(4 * w, 4 * w + 4):
                nc.tensor.matmul(
                    zc[:, b : b + 1],
                    p_sb[:, b * NCH : (b + 1) * NCH],
                    ones_c_bf[:],
                    start=True,
                    stop=True,
                )
                # complete each accumulation chain before starting the next
                # (two open groups in one PSUM bank trip the zero-region rule)
                for c in range(NCH):
                    nc.tensor.matmul(
                        ct0_ps[:, b : b + 1],
                        en_tiles[b][:, c * H : c * H + H0],
                        p_sb[:, b * NCH + c : b * NCH + c + 1],
                        start=(c == 0),
                        stop=(c == NCH - 1),
                    )
                for c in range(NCH):
                    nc.tensor.matmul(
                        ct1_ps[:, b : b + 1],
                        en_tiles[b][:, c * H + H0 : (c + 1) * H],
                        p_sb[:, b * NCH + c : b * NCH + c + 1],
                        start=(c == 0),
                        stop=(c == NCH - 1),
                    )

        # per-tile segment map and per-batch last tile
        tile_segs = [[] for _ in range(NTILE)]
        for b in range(NB):
            for ti, off, hlo, ln in _segments(b):
                tile_segs[ti].append((b, off, hlo, ln))
        last_tile = [_segments(b)[-1][0] for b in range(NB)]

        import bass_rust as _br
        scores_done = -1  # highest batch whose scores are emitted

        with tc.tile_pool(name="kp_psum", bufs=3, space="PSUM") as kp_ps:
            for ti in range(NTILE):
                for b, _, _, _ in tile_segs[ti]:
                    if b not in et_tiles:
                        load_batch(b)
                e = e_pool.tile([128, T], dt.bfloat16, tag="e", name=f"e{ti}")
                e_pk.append(e)
                # plan: wide DoubleRow computes the whole tile from the
                # zero-extended Ua (garbage rows), later pieces overwrite.
                # DR only encodes dst at base 0 with 32/64/128-row groups.
                segs = tile_segs[ti]
                plan = []  # (kind, b, off, ln, ua_lo) kind: dr | wide | ndr
                if len(segs) == 1:
                    b0_, off0_, hlo0_, ln0_ = segs[0]
                    plan.append(("dr", b0_, 0, 128, 96 + hlo0_))
                else:
                    (bx, ox, hx, lx), (by, oy, hy, ly) = segs[0], segs[1]
                    if lx in (32, 64):
                        # wide(Y) first, DR(X) overwrites rows 0..lx
                        plan.append(("wide", by, 0, 128, 96 + hy - oy))
                        plan.append(("dr", bx, 0, lx, 96 + hx))
                    else:
                        # wide(X) first, non-DR(Y) overwrites the top rows
                        plan.append(("wide", bx, 0, 128, 96 + hx))
                        plan.append(("ndr", by, oy, ly, 96 + hy))
                for th in range(2):
                    ps = kp_ps.tile([128, 1024], dt.float32, tag="kp")
                    for kind, b, poff, pln, ua_lo in plan:
                        for n in range(2):
                            c0c = th * 1024 + n * 512
                            dst = ps[poff : poff + pln, n * 512 : (n + 1) * 512]
                            if kind in ("dr", "wide"):
                                i_kp = nc.tensor.matmul(
                                    dst,
                                    ua3[:, :, ua_lo : ua_lo + pln],
                                    et_tiles[b][:, :, c0c : c0c + 512],
                                    start=True,
                                    stop=True,
                                    perf_mode=mybir.MatmulPerfMode.DoubleRow,
                                    tile_position=(0, 0),
                                )
                            else:
                                for i in range(2):
                                    i_kp = nc.tensor.matmul(
                                        dst,
                                        ua3[:, i, ua_lo : ua_lo + pln],
                                        et_tiles[b][:, i, c0c : c0c + 512],
                                        start=(i == 0),
                                        stop=(i == 1),
                                        tile_position=(0, poff),
                                    )
                            kp_hist[b] = i_kp
                    # e = tanh(kproj/64 + qp_pk[:, ti]) ; write bf16
                    nc.scalar.activation(
                        e[:, th * 1024 : (th + 1) * 1024],
                        ps[:],
                        AF.Tanh,
                        bias=qp_pk_sb[:, ti : ti + 1],
                        scale=1.0 / 64.0,
                    )
                # scores/waves with a one-tile skew; encN paced ~2 batches back
                for b in range(scores_done + 1, NB):
                    if last_tile[b] != ti - 1:
                        break
                    emit_scores(b)
                    scores_done = b
                    en = encN_pool.tile(
                        [128, NCH * H], dt.bfloat16, name=f"en{b}", tag="en"
                    )
                    i_en = nc.gpsimd.dma_start(
                        en[:],
                        d_encN[b].rearrange("(p n) h -> p (n h)", p=128),
                    )
                    _br.add_dep_helper(
                        i_en.ins, kp_hist[max(0, b - 2)].ins, sync=True,
                        reason="encN paced behind kproj two batches back",
                    )
                    en_tiles.append(en)
                    if b % 4 == 3:
                        emit_wave(b // 4)
            for b in range(scores_done + 1, NB):
                emit_scores(b)
                if b >= len(en_tiles):
                    en = encN_pool.tile(
                        [128, NCH * H], dt.bfloat16, name=f"en{b}", tag="en"
                    )
                    nc.gpsimd.dma_start(
                        en[:],
                        d_encN[b].rearrange("(p n) h -> p (n h)", p=128),
                    )
                    en_tiles.append(en)
                if b % 4 == 3:
                    emit_wave(b // 4)

        # ---------- phase 2: Z totals, 1/Z, ctx scale, G0 ----------
        ct0 = spool.tile([H0, NB], dt.bfloat16)
        ct1 = spool.tile([H1 + 1, NB], dt.bfloat16)  # row 72 = ones (bias row)
        nc.scalar.dma_start(ct1[H1 : H1 + 1, :], d_ones_b[:, :])

        with tc.tile_pool(name="z_psum", bufs=1, space="PSUM") as z_ps:
            # Z per batch broadcast down all 128 partitions in one matmul
            # (lhsT = ones [16, 128] -> out[r, b] = sum_ch zc[ch, b]), then
            # reciprocal straight into SBUF
            zc_sb = spool.tile([NCH, NB], dt.float32)
            nc.vector.tensor_copy(zc_sb[:], zc[:])
            zbc = z_ps.tile([128, NB], dt.float32, tag="zbc")
            nc.tensor.matmul(zbc[:], ones_sq_f[:], zc_sb[:], start=True, stop=True)
            rzb_sb = spool.tile([128, NB], dt.float32)
            nc.vector.reciprocal(rzb_sb[:], zbc[:])
            # normalize: ctxT = ctx_raw * (1/Z) broadcast, cast bf16
            nc.vector.tensor_tensor(ct0[:], ct0_ps[:], rzb_sb[:], op=OP.mult)
            nc.vector.tensor_tensor(
                ct1[0:H1, :], ct1_ps[:], rzb_sb[0:H1, :], op=OP.mult
            )
        att_ctx.close()  # release sc/ct/kp PSUM banks
        g0_pool = ctx.enter_context(tc.tile_pool(name="g0_psum", bufs=1, space="PSUM"))
        gp = g0_pool.tile([NB, G4], dt.float32, tag="g0")
        for n, nsz in [(0, 512), (512, G4 - 512)]:
            nc.tensor.matmul(
                gp[:, n : n + nsz], ct0[:], wihc0[:, n : n + nsz],
                start=True, stop=False,
            )
            nc.tensor.matmul(
                gp[:, n : n + nsz], ct1, wihc1[:, n : n + nsz],
                start=False, stop=False,
            )
            nc.tensor.matmul(
                gp[:, n : n + nsz], qt0, whh0[:, n : n + nsz],
                start=False, stop=False,
            )
            nc.tensor.matmul(
                gp[:, n : n + nsz], qt1, whh1[:, n : n + nsz],
                start=False, stop=True,
            )

        # ---------- phase 3: decoder steps (all bf16, gate order f,i,o|g) ---
        x_sb = spool.tile([NB, 1], dt.float32)
        nc.sync.dma_start(x_sb[:], d_x0[:, :])
        xn_all = spool.tile([NB, NSTEPS], dt.float32)
        ht0 = spool.tile([H0, NB], dt.bfloat16)
        ht1 = spool.tile([H1 + 1, NB], dt.bfloat16)  # row 72 = ones (b1 row)
        nc.scalar.dma_start(ht1[H1 : H1 + 1, :], d_ones_b[:, :])
        o1t = spool.tile([101, NB], dt.bfloat16)  # row 100 = ones (b2 row)
        nc.scalar.dma_start(o1t[100:101, :], d_ones_b[:, :])
        o2t = spool.tile([51, NB], dt.bfloat16)  # row 50 = ones (b3 row)
        nc.scalar.dma_start(o2t[50:51, :], d_ones_b[:, :])

        with (
            tc.tile_pool(name="ls", bufs=2) as ls,
            tc.tile_pool(name="ls_psum", bufs=3, space="PSUM") as lp,
            tc.tile_pool(name="m3_psum", bufs=2, space="PSUM") as mp,
        ):
            xt = x_sb
            for t in range(NSTEPS):
                # gates = g0 + x * wxr, fused on DVE; split f,i,o vs g so
                # the sigmoid can start before the g slice is computed
                gates = ls.tile([NB, G4], dt.bfloat16, tag="gates")
                for glo, ghi in ((0, 2 * H), (3 * H, 4 * H), (2 * H, 3 * H)):
                    nc.vector.scalar_tensor_tensor(
                        gates[:, glo:ghi], wxr_sb[:, glo:ghi], xt[:, 0:1],
                        gp[:, glo:ghi], op0=OP.mult, op1=OP.add,
                    )
                # f,i sigmoid first (feeds t1/t2), then g tanh, then o
                sfio = ls.tile([NB, 3 * H], dt.bfloat16, tag="sfio")
                nc.scalar.activation(sfio[:, 0 : 2 * H], gates[:, 0 : 2 * H], AF.Sigmoid)
                tg = ls.tile([NB, H], dt.bfloat16, tag="tg")
                nc.scalar.activation(tg[:], gates[:, 3 * H : 4 * H], AF.Tanh)
                nc.scalar.activation(
                    sfio[:, 2 * H : 3 * H], gates[:, 2 * H : 3 * H], AF.Sigmoid
                )
                t1 = ls.tile([NB, H], dt.bfloat16, tag="t1")
                nc.vector.tensor_tensor(t1[:], sfio[:, 0:H], c0_sb, op=OP.mult)
                t2 = ls.tile([NB, H], dt.bfloat16, tag="t2")
                nc.vector.tensor_tensor(t2[:], sfio[:, H : 2 * H], tg[:], op=OP.mult)
                cn = ls.tile([NB, H], dt.bfloat16, tag="cn")
                nc.vector.tensor_tensor(cn[:], t1[:], t2[:], op=OP.add)
                tcn = ls.tile([NB, H], dt.bfloat16, tag="tcn")
                nc.scalar.activation(tcn[:], cn[:], AF.Tanh)
                hh = ls.tile([NB, H], dt.bfloat16, tag="hh")
                nc.vector.tensor_tensor(hh[:], sfio[:, 2 * H : 3 * H], tcn[:], op=OP.mult)
                # transpose h -> ht0/ht1 (feature-major for the MLP); relu
                # folded into the PSUM->SBUF copies (DVE max / ACT relu)
                tp0 = lp.tile([128, NB], dt.bfloat16, tag="lsps")
                nc.tensor.transpose(tp0[:], hh[:, 0:H0], id_bf[0:NB, 0:NB])
                nc.vector.tensor_scalar_max(ht0[:], tp0[:], 0.0)
                tp1 = lp.tile([128, NB], dt.bfloat16, tag="lsps")
                nc.tensor.transpose(tp1[0:H1, :], hh[:, H0:H], id_bf[0:NB, 0:NB])
                nc.scalar.activation(ht1[0:H1, :], tp1[0:H1, :], AF.Relu)
                # MLP in feature-major, biases via ones rows
                m1 = lp.tile([100, NB], dt.float32, tag="lsps")
                nc.tensor.matmul(m1[:], w1t0, ht0[:], start=True, stop=False)
                nc.tensor.matmul(m1[:], w1t1, ht1[:], start=False, stop=True)
                nc.vector.tensor_scalar_max(o1t[0:100, :], m1[:], 0.0)
                m2 = lp.tile([50, NB], dt.float32, tag="lsps")
                nc.tensor.matmul(m2[:], w2t, o1t[:], start=True, stop=True)
                nc.vector.tensor_scalar_max(o2t[0:50, :], m2[:], 0.0)
                # m3 flipped: o2 stationary, w3 moving -> out [NB, 1] is
                # directly the next step's x (read from PSUM as STT scalar)
                m3 = mp.tile([NB, 1], dt.float32, tag="m3")
                nc.tensor.matmul(m3[:], o2t[:], w3t, start=True, stop=True)
                nc.vector.tensor_copy(xn_all[:, t : t + 1], m3[:])
                xt = m3
            nc.sync.dma_start(d_y[:, :], xn_all[:])

    # Bacc lowering: register allocation + wait splitting (<=1 wait/inst on HW)
    nc.compile()
    return nc


def _prep_inputs(x, h0, c0, encoder_output, Wa, ba, Ua, bua, Va, bva,
                 W_ih, W_hh, b_ih, b_hh, W1, b1, W2, b2, W3, b3):
    """Host-side layout prep -> list of per-core input maps."""
    f32 = np.float32
    enc = np.ascontiguousarray(encoder_output, dtype=f32)
    q = np.asarray(h0, dtype=f32)[0]          # [B, H]
    c0f = np.asarray(c0, dtype=f32)[0]        # [B, H]
    x0 = np.asarray(x, dtype=f32).reshape(B, 1)

    # gate permutation: torch order (i,f,g,o) -> device order (f,i,o,g) so
    # one sigmoid instr covers f,i,o and tanh covers g
    gperm = np.concatenate(
        [np.arange(H, 2 * H), np.arange(0, H), np.arange(3 * H, 4 * H),
         np.arange(2 * H, 3 * H)]
    )
    W_ihp = np.asarray(W_ih, f32)[gperm]
    W_hhp = np.asarray(W_hh, f32)[gperm]
    bp = (np.asarray(b_ih, f32) + np.asarray(b_hh, f32))[gperm]

    # Ua scaled x64 into fp8 comfortable range; kernel rescales inside tanh.
    # DoubleRow packing: partition p holds h_in = p (i=0) and 128+p (i=1);
    # M padded 200->208 so the pair stride is 16B-aligned.
    uaT = np.asarray(Ua, f32).T * 64.0  # [h_in, h_out]
    # h_out axis zero-extended left by 96 and right to 352 so "wide" DoubleRow
    # matmuls can compute whole 128-row tiles with garbage rows outside a
    # segment (overwritten by later pieces)
    uaT_pad = np.zeros((256, 352), f32)
    uaT_pad[0:H, 96 : 96 + H] = uaT
    uaT_p = np.stack([uaT_pad[0:128], uaT_pad[128:256]], axis=1)  # [128, 2, 352]

    # packed attention weights
    wpk = np.zeros((128, PK_C), f32)
    waT = np.asarray(Wa, f32).T  # [h_in, h_out]
    wpk[:, PK_WA0 : PK_WA0 + 200] = waT[0:128]
    wpk[0:72, PK_WA1 : PK_WA1 + 200] = waT[128:200]
    va = np.asarray(Va, f32)[0]
    for (bb, ti), cidx in VA_COLS.items():
        r = np.arange(128)
        h = 128 * ti + r - BSTRIDE * bb
        mask = (h >= 0) & (h < H)
        colv = np.zeros(128, f32)
        colv[mask] = va[h[mask]]
        wpk[:, PK_VAPK + cidx] = colv
    qb = np.asarray(ba, f32) + np.asarray(bua, f32)
    wpk[0, PK_QBR : PK_QBR + H] = qb

    # packed decoder weights
    dpk = np.zeros((128, DK_C), f32)
    w_ihcT = W_ihp[:, 1:].T  # [H, G4]
    dpk[:, DK_WIHC0 : DK_WIHC0 + G4] = w_ihcT[0:128]
    dpk[0:72, DK_WIHC1 : DK_WIHC1 + G4] = w_ihcT[128:200]
    dpk[72, DK_WIHC1 : DK_WIHC1 + G4] = bp
    w_hhT = W_hhp.T
    dpk[:, DK_WHH0 : DK_WHH0 + G4] = w_hhT[0:128]
    dpk[0:72, DK_WHH1 : DK_WHH1 + G4] = w_hhT[128:200]
    dpk[0:NB, DK_WXR : DK_WXR + G4] = np.broadcast_to(
        W_ihp[:, 0].reshape(1, G4), (NB, G4)
    )
    w1T = np.asarray(W1, f32).T
    dpk[:, DK_W1T0 : DK_W1T0 + 100] = w1T[0:128]
    dpk[0:72, DK_W1T1 : DK_W1T1 + 100] = w1T[128:200]
    dpk[72, DK_W1T1 : DK_W1T1 + 100] = np.asarray(b1, f32)
    dpk[0:100, DK_W2T : DK_W2T + 50] = np.asarray(W2, f32).T
    dpk[100, DK_W2T : DK_W2T + 50] = np.asarray(b2, f32)
    dpk[0:50, DK_W3T] = np.asarray(W3, f32)[0]
    dpk[50, DK_W3T] = np.asarray(b3, f32)[0]

    shared = {
        "UaTp": np.ascontiguousarray(uaT_p.reshape(128, 2 * 352)).astype(FP8),
        "ones_b": np.ones((1, NB), BF16),
    }

    in_maps = []
    for cix in range(NCORES):
        bs = slice(cix * NB, (cix + 1) * NB)
        enc_c = enc[bs]  # [NB, T, H]
        m = dict(shared)
        encT_c = enc_c.transpose(0, 2, 1)  # [NB, H, T]
        encT_pad = np.concatenate(
            [encT_c, np.zeros((NB, 56, T), f32)], axis=1
        )  # [NB, 256, T]
        encT_p = np.stack([encT_pad[:, 0:128], encT_pad[:, 128:256]], axis=2)
        m["encTp"] = np.ascontiguousarray(encT_p.reshape(NB, 128, 2 * T)).astype(FP8)
        m["encN"] = enc_c.astype(BF16)
        dpk_c = dpk.copy()
        dpk_c[0:NB, DK_C0 : DK_C0 + H] = c0f[bs]
        m["dpk"] = dpk_c.astype(BF16)
        wpk_c = wpk.copy()
        qTc = q[bs].T  # [H, NB]
        wpk_c[:, PK_QT0 : PK_QT0 + NB] = qTc[0:128]
        wpk_c[0:72, PK_QT1 : PK_QT1 + NB] = qTc[128:200]
        m["wpk"] = wpk_c.astype(BF16)
        m["x0s"] = np.ascontiguousarray(x0[bs])
        in_maps.append(m)
    return in_maps


def kernel(**inputs):
    from concourse.bass_utils import run_bass_kernel_spmd

    if "nc" not in _CACHE:
        _CACHE["nc"] = _build_module()
    nc = _CACHE["nc"]

    in_maps = _prep_inputs(**inputs)
    res = run_bass_kernel_spmd(nc, in_maps, core_ids=list(range(NCORES)))
    # y2 per core: [NB, NSTEPS] -> full output [B, NSTEPS]
    out = np.concatenate([r["y2"] for r in res.results], axis=0)
    return np.ascontiguousarray(out.astype(np.float32))


# revision 32
# speedup vs baseline: 1.1594x; 1.0142x over previous
"""Trainium2 Bass kernel for nn_DecoderAttention (Bahdanau attention + LSTM decoder).

Data-parallel over batch: B=128 split across 8 NeuronCores (16 batches/core).
All FLOPs run on device; the host only reshuffles layouts (transpose / dtype
cast / fp8 DoubleRow packing / weight concat with bias rows folded in).

Per-core device pipeline (cost-model-aware layout):
  phase 0: ONE packed DMA for all small attention weights (+one fp32 qb DMA),
           qprojT = Wa @ q^T (+ ba + bua) on PE
  phase 1: per batch b: kprojT = Ua @ enc_b^T as fp8 DoubleRow matmuls
           (K=200 packed 2/partition, one pass, 0.5 cyc/row),
           tanh(kproj/64 + qprojT[:, b]) on ACT -> e tiles [h, t] bf16;
           scores as FLIPPED matmuls: e chunk stationary (K=h, M=128 t's of
           one stride-16 class), Va moving (N=1) -> scores accumulate into
           one [128, 256] PSUM tile, column 16*b+c;
           per WAVE of 4 batches: exp slice [128, 64] -> p (bf16,
           unnormalized), per-batch colsum matmuls -> zc, context via FLIPPED
           matmuls (encN chunk stationary K=t, p column moving N=1) -> ctxT
           accumulates [h, b] in PSUM. All of it hides inside phase 1.
  phase 2 (tail): Z totals via two tiny PE reductions, 1/Z broadcast via a
           K=1 outer-product matmul, ctxT scale on DVE, G0 closes an
           accumulation whose q-terms ran during phase 1.
  phase 3: 5 serial decoder steps, all-bf16 elementwise:
           gates = G0(PSUM) + x*wxr fused on DVE (scalar_tensor_tensor,
           split f,i,o vs g), gate order permuted so one sigmoid covers
           f,i,o; MLP in bf16; m3 flipped so x_next = out [16, 1] feeds the
           next step's scalar directly from PSUM.
"""

import numpy as np
import ml_dtypes

B, T, H = 128, 2048, 200
NCORES = 8
NB = B // NCORES  # 16 batches per core
NSTEPS = 5
G4 = 4 * H  # 800 gate width
NCH = T // 128  # 16 stride-class chunks (t = 16*k + c -> partition k, chunk c)

_CACHE = {}

# M-packed kproj/tanh layout: batch b's 200 h-rows live at packed rows
# [224*b, 224*b+200) (stride 224 = 7*32 keeps every PE write 32-aligned,
# 16*224 = 3584 = 28 full 128-row tiles); h_out padded 200->224 with zeros.
BSTRIDE = 224
NTILE = NB * BSTRIDE // 128  # 28


def _legal_pieces(off, ln):
    """Split (off, ln) into PE-legal out placements: base 0 takes up to 128
    rows, base 64 up to 64, bases 32/96 up to 32."""
    out = []
    while ln > 0:
        cap = {0: 128, 32: 32, 64: 64, 96: 32}[off % 128]
        take = min(ln, cap)
        out.append((off, take))
        off += take
        ln -= take
    return out


def _batch_tiles(b):
    return sorted({ti for ti, _, _, _ in _segments(b)})


# columns of the per-(batch,tile) zero-masked Va block, in emission order
VA_COLS = {}
_c = 0
for _b in range(16):
    for _ti in range(28):
        if 224 * _b < 128 * (_ti + 1) and 128 * _ti < 224 * _b + 224:
            VA_COLS[(_b, _ti)] = _c
            _c += 1
NV = _c


def _segments(b):
    """Packed-row segments of batch b: (tile, off, hlo, ln) covering
    packed rows [BSTRIDE*b, BSTRIDE*(b+1)), split at 128-row tile edges."""
    out = []
    r0 = BSTRIDE * b
    r = r0
    while r < r0 + BSTRIDE:
        ti, off = divmod(r, 128)
        ln = min(128 - off, r0 + BSTRIDE - r)
        out.append((ti, off, r - r0, ln))
        r += ln
    return out


def _sub_segments(b):
    """Real-h sub-segments of batch b for the scores matmuls:
    (tile, off, hlo, ln) additionally split at h=128 and clipped at h=200."""
    out = []
    for ti, off, hlo, ln in _segments(b):
        ln = min(ln, 200 - hlo)
        if ln <= 0:
            continue
        pieces = []
        if hlo < 128 < hlo + ln:
            cut = 128 - hlo
            pieces = [(off, hlo, cut), (off + cut, 128, ln - cut)]
        else:
            pieces = [(off, hlo, ln)]
        for poff, phlo, pln in pieces:
            # stationary reads obey the same base/size tiling rule as writes
            for qoff, qln in _legal_pieces(poff, pln):
                out.append((ti, qoff, phlo + (qoff - poff), qln))
    return out

BF16 = ml_dtypes.bfloat16
FP8 = ml_dtypes.float8_e4m3fn

# packed attention-weight tensor column layout (bf16, [128, PK_C])
PK_WA0, PK_WA1 = 0, 224          # wa0 [128,224] | wa1 [72,224] (224 = padded)
PK_VA0, PK_VA1 = 448, 449        # va columns
PK_QT0, PK_QT1 = 450, 466        # qt [*,16]
PK_QBR = 482                     # qb as a [1, 224] row (for K=1 bias matmuls)
PK_VAPK = 706                    # zero-masked Va per (batch, tile) [128, NV]
PK_C = 706 + 44

# packed decoder-weight tensor column layout (bf16, [128, DK_C])
DK_WIHC0, DK_WIHC1 = 0, 800      # wihc0 [128,800] | wihc1 [73,800] (row72=bias)
DK_WHH0, DK_WHH1 = 1600, 2400    # whh [128|72, 800]
DK_WXR = 3200                    # wxr [16, 800]
DK_W1T0, DK_W1T1 = 4000, 4100    # w1t [128|73, 100] (row72=b1)
DK_W2T = 4200                    # w2t [101, 50] (row100=b2)
DK_W3T = 4250                    # w3t [51, 1] (row50=b3)
DK_C0 = 4251                     # c0 [16, 200]
DK_C = 4451


def _build_module():
    """Build the Bass module (same NEFF for all 8 cores)."""
    from contextlib import ExitStack

    import concourse.bass as bass
    import concourse.tile as tile
    from concourse import bacc, mybir
    from concourse.masks import make_identity

    dt = mybir.dt
    AF = mybir.ActivationFunctionType
    OP = mybir.AluOpType

    nc = bacc.Bacc(
        "TRN2",
        target_bir_lowering=False,
        debug=False,
        num_devices=NCORES,
    )

    # ---- DRAM tensors (per-core shards; weights replicated) ----
    d_encT = nc.dram_tensor(
        "encTp", [NB, 128, 2 * T], dt.float8e4, kind="ExternalInput"
    ).ap()  # DoubleRow packing: col i*T+t, partition p <-> h_in = i*128+p
    d_encN = nc.dram_tensor("encN", [NB, T, H], dt.bfloat16, kind="ExternalInput").ap()
    d_x0 = nc.dram_tensor("x0s", [NB, 1], dt.float32, kind="ExternalInput").ap()
    d_UaT = nc.dram_tensor("UaTp", [128, 2 * 352], dt.float8e4, kind="ExternalInput").ap()
    d_wpk = nc.dram_tensor("wpk", [128, PK_C], dt.bfloat16, kind="ExternalInput").ap()
    d_dpk = nc.dram_tensor("dpk", [128, DK_C], dt.bfloat16, kind="ExternalInput").ap()
    d_ones_b = nc.dram_tensor("ones_b", [1, NB], dt.bfloat16, kind="ExternalInput").ap()
    d_y = nc.dram_tensor("y2", [NB, NSTEPS], dt.float32, kind="ExternalOutput").ap()

    H0, H1 = 128, H - 128  # 128 + 72 partition chunks of the hidden dim

    with tile.TileContext(nc) as tc, ExitStack() as ctx:
        # ---------- persistent pools ----------
        wpool = ctx.enter_context(tc.tile_pool(name="weights", bufs=1))
        spool = ctx.enter_context(tc.tile_pool(name="smalls", bufs=1))

        # warmup: preload the tanh/exp ACT table set while DMAs stream
        wt_a = spool.tile([1, 8], dt.float32)
        nc.vector.memset(wt_a[:], 0.0)
        wt_b = spool.tile([1, 8], dt.float32)
        nc.scalar.activation(wt_b[:], wt_a[:], AF.Tanh)

        # identity for the decoder's h transposes (bf16)
        id_bf = wpool.tile([128, 128], dt.bfloat16)
        make_identity(nc, id_bf[:])

        # ones columns/rows for the tiny PE reductions (sliced on read)
        ones_c_bf = wpool.tile([128, 1], dt.bfloat16)
        nc.vector.memset(ones_c_bf[:], 1.0)
        ones_c_f = wpool.tile([128, 1], dt.float32)
        nc.vector.memset(ones_c_f[:], 1.0)
        ones_sq_f = wpool.tile([NCH, 128], dt.float32)
        nc.vector.memset(ones_sq_f[:], 1.0)

        # packed attention weights: one DMA (triggered after et0/ua below so
        # the kproj-critical transfers go first on the serialized DMA engine)
        wpk = wpool.tile([128, PK_C], dt.bfloat16)
        wa0 = wpk[:, PK_WA0 : PK_WA0 + BSTRIDE]
        wa1 = wpk[0:H1, PK_WA1 : PK_WA1 + BSTRIDE]
        va_pk = wpk[:, PK_VAPK : PK_VAPK + NV]
        qt0 = wpk[:, PK_QT0 : PK_QT0 + NB]
        qt1 = wpk[0:H1, PK_QT1 : PK_QT1 + NB]

        ua_p = wpool.tile([128, 2 * 352], dt.float8e4)
        ua3 = ua_p[:].rearrange("p (i m) -> p i m", i=2)

        # packed decoder weights: one DMA (deferred below, behind first encT)
        dpk = wpool.tile([128, DK_C], dt.bfloat16)
        wihc0 = dpk[:, DK_WIHC0 : DK_WIHC0 + G4]
        wihc1 = dpk[0 : H1 + 1, DK_WIHC1 : DK_WIHC1 + G4]
        whh0 = dpk[:, DK_WHH0 : DK_WHH0 + G4]
        whh1 = dpk[0:H1, DK_WHH1 : DK_WHH1 + G4]
        wxr_sb = dpk[0:NB, DK_WXR : DK_WXR + G4]
        w1t0 = dpk[:, DK_W1T0 : DK_W1T0 + 100]
        w1t1 = dpk[0 : H1 + 1, DK_W1T1 : DK_W1T1 + 100]
        w2t = dpk[0:101, DK_W2T : DK_W2T + 50]
        w3t = dpk[0:51, DK_W3T : DK_W3T + 1]
        c0_sb = dpk[0:NB, DK_C0 : DK_C0 + H]

        # phase-1 encT pool + loader, defined early: the first encT/Ua DMA
        # triggers go BEFORE wpk on the serialized DMA engine, then wpk,
        # then the phase-0 matmuls that read wpk
        encT_pool = ctx.enter_context(tc.tile_pool(name="encT_pool", bufs=4))
        et_tiles = {}

        def load_batch(b):
            et = encT_pool.tile([128, 2 * T], dt.float8e4, tag="et", name=f"et{b}")
            nc.sync.dma_start(et[:], d_encT[b, :])
            et_tiles[b] = et[:].rearrange("p (i t) -> p i t", i=2)
            if b == 0:
                nc.sync.dma_start(ua_p[:], d_UaT[:, :])
            if b == 3:
                # single packed decoder-weight DMA, off the startup path
                nc.sync.dma_start(dpk[:], d_dpk[:, :])

        load_batch(0)
        nc.scalar.dma_start(wpk[:], d_wpk[:, :])

        # ---------- phase 0: packed qproj bias: qp_pk[r, ti] = ----------
        # qproj[h(r,ti), b(r,ti)] + qb[h]; pad rows get tanh(0*scale+qb-row)
        qp_pk_sb = spool.tile([128, NTILE], dt.float32)
        ones1_b = wpool.tile([1, 1], dt.bfloat16)
        nc.vector.memset(ones1_b[:], 1.0)
        with tc.tile_pool(name="qp_psum", bufs=1, space="PSUM") as qp_ps:
            ps = qp_ps.tile([128, NTILE], dt.float32, tag="qp")
            for b in range(NB):
                for ti, off, hlo, ln in _segments(b):
                    for poff, pln in _legal_pieces(off, ln):
                        phlo = hlo + (poff - off)
                        dst = ps[poff : poff + pln, ti : ti + 1]
                        nc.tensor.matmul(
                            dst, wa0[:, phlo : phlo + pln], qt0[:, b : b + 1],
                            start=True, stop=False, tile_position=(0, poff),
                        )
                        nc.tensor.matmul(
                            dst, wa1[:, phlo : phlo + pln], qt1[:, b : b + 1],
                            start=False, stop=False, tile_position=(0, poff),
                        )
                        # + qb (K=1 outer with the qb row)
                        nc.tensor.matmul(
                            dst, wpk[0:1, PK_QBR + phlo : PK_QBR + phlo + pln],
                            ones1_b[:], start=False, stop=True,
                            tile_position=(0, poff),
                        )
            nc.vector.tensor_copy(qp_pk_sb[:], ps[:])

        # ---------- phase 1: kproj + tanh + scores + per-wave softmax/ctx ----
        e_pool = ctx.enter_context(tc.tile_pool(name="e_pool", bufs=6))
        encN_pool = ctx.enter_context(tc.tile_pool(name="encN_pool", bufs=16))
        from contextlib import ExitStack as _ES
        att_ctx = _ES()
        sc_pool = att_ctx.enter_context(tc.tile_pool(name="sc_psum", bufs=1, space="PSUM"))
        ct_pool = att_ctx.enter_context(tc.tile_pool(name="ct_psum", bufs=1, space="PSUM"))
        sc = sc_pool.tile([128, NB * NCH + NB], dt.float32, tag="sc")
        p_sb = spool.tile([128, NB * NCH], dt.bfloat16)
        zc = sc[0:NCH, NB * NCH : NB * NCH + NB]
        # one PSUM tile (PSUM tiles are bank-granular): ct0 | ct1 columns
        ctz = ct_pool.tile([128, 2 * NB], dt.float32, tag="ctz")
        ct0_ps = ctz[:, 0:NB]
        ct1_ps = ctz[0:H1, NB : 2 * NB]

        en_tiles = []
        kp_hist = {}  # batch -> last kproj matmul (encN pacing anchor)
        e_pk = []     # packed e tiles, one per 128-row tile

        def emit_scores(b):
            tiles = _batch_tiles(b)
            for c in range(NCH):
                col = b * NCH + c
                for k, ti in enumerate(tiles):
                    # full-tile contraction; va column is zero outside
                    # batch b's rows, so other batches contribute nothing
                    vcol = VA_COLS[(b, ti)]
                    nc.tensor.matmul(
                        sc[:, col : col + 1],
                        e_pk[ti][:, c : T : NCH],
                        va_pk[:, vcol : vcol + 1],
                        start=(k == 0),
                        stop=(k == len(tiles) - 1),
                    )

        def emit_wave(w):
            # exp + Z colsums + context for batches 4w..4w+3 (scores ready)
            lo = 4 * w * NCH
            nc.scalar.activation(
                p_sb[:, lo : lo + 4 * NCH], sc[:, lo : lo + 4 * NCH], AF.Exp
            )
            for b in range(4 * w, 4 * w + 4):
                nc.tensor.matmul(
                    zc[:, b : b + 1],
                    p_sb[:, b * NCH : (b + 1) * NCH],
                    ones_c_bf[:],
                    start=True,
                    stop=True,
                )
                # complete each accumulation chain before starting the next
                # (two open groups in one PSUM bank trip the zero-region rule)
                for c in range(NCH):
                    nc.tensor.matmul(
                        ct0_ps[:, b : b + 1],
                        en_tiles[b][:, c * H : c * H + H0],
                        p_sb[:, b * NCH + c : b * NCH + c + 1],
                        start=(c == 0),
                        stop=(c == NCH - 1),
                    )
                for c in range(NCH):
                    nc.tensor.matmul(
                        ct1_ps[:, b : b + 1],
                        en_tiles[b][:, c * H + H0 : (c + 1) * H],
                        p_sb[:, b * NCH + c : b * NCH + c + 1],
                        start=(c == 0),
                        stop=(c == NCH - 1),
                    )

        # per-tile segment map and per-batch last tile
        tile_segs = [[] for _ in range(NTILE)]
        for b in range(NB):
            for ti, off, hlo, ln in _segments(b):
                tile_segs[ti].append((b, off, hlo, ln))
        last_tile = [_segments(b)[-1][0] for b in range(NB)]

        import bass_rust as _br
        scores_done = -1  # highest batch whose scores are emitted

        with tc.tile_pool(name="kp_psum", bufs=3, space="PSUM") as kp_ps:
            for ti in range(NTILE):
                for b, _, _, _ in tile_segs[ti]:
                    if b not in et_tiles:
                        load_batch(b)
                e = e_pool.tile([128, T], dt.bfloat16, tag="e", name=f"e{ti}")
                e_pk.append(e)
                # plan: wide DoubleRow computes the whole tile from the
                # zero-extended Ua (garbage rows), later pieces overwrite.
                # DR only encodes dst at base 0 with 32/64/128-row groups.
                segs = tile_segs[ti]
                plan = []  # (kind, b, off, ln, ua_lo) kind: dr | wide | ndr
                if len(segs) == 1:
                    b0_, off0_, hlo0_, ln0_ = segs[0]
                    plan.append(("dr", b0_, 0, 128, 96 + hlo0_))
                else:
                    (bx, ox, hx, lx), (by, oy, hy, ly) = segs[0], segs[1]
                    if lx in (32, 64):
                        # wide(Y) first, DR(X) overwrites rows 0..lx
                        plan.append(("wide", by, 0, 128, 96 + hy - oy))
                        plan.append(("dr", bx, 0, lx, 96 + hx))
                    else:
                        # wide(X) first, non-DR(Y) overwrites the top rows
                        plan.append(("wide", bx, 0, 128, 96 + hx))
                        plan.append(("ndr", by, oy, ly, 96 + hy))
                for th in range(2):
                    ps = kp_ps.tile([128, 1024], dt.float32, tag="kp")
                    for kind, b, poff, pln, ua_lo in plan:
                        for n in range(2):
                            c0c = th * 1024 + n * 512
                            dst = ps[poff : poff + pln, n * 512 : (n + 1) * 512]
                            if kind in ("dr", "wide"):
                                i_kp = nc.tensor.matmul(
                                    dst,
                                    ua3[:, :, ua_lo : ua_lo + pln],
                                    et_tiles[b][:, :, c0c : c0c + 512],
                                    start=True,
                                    stop=True,
                                    perf_mode=mybir.MatmulPerfMode.DoubleRow,
                                    tile_position=(0, 0),
                                )
                            else:
                                for i in range(2):
                                    i_kp = nc.tensor.matmul(
                                        dst,
                                        ua3[:, i, ua_lo : ua_lo + pln],
                                        et_tiles[b][:, i, c0c : c0c + 512],
                                        start=(i == 0),
                                        stop=(i == 1),
                                        tile_position=(0, poff),
                                    )
                            kp_hist[b] = i_kp
                    # e = tanh(kproj/64 + qp_pk[:, ti]) ; write bf16
                    nc.scalar.activation(
                        e[:, th * 1024 : (th + 1) * 1024],
                        ps[:],
                        AF.Tanh,
                        bias=qp_pk_sb[:, ti : ti + 1],
                        scale=1.0 / 64.0,
                    )
                # scores/waves with a one-tile skew; encN paced ~2 batches back
                for b in range(scores_done + 1, NB):
                    if last_tile[b] > ti - 2:
                        break
                    emit_scores(b)
                    scores_done = b
                    en = encN_pool.tile(
                        [128, NCH * H], dt.bfloat16, name=f"en{b}", tag="en"
                    )
                    i_en = nc.gpsimd.dma_start(
                        en[:],
                        d_encN[b].rearrange("(p n) h -> p (n h)", p=128),
                    )
                    _br.add_dep_helper(
                        i_en.ins, kp_hist[max(0, b - 2)].ins, sync=True,
                        reason="encN paced behind kproj two batches back",
                    )
                    en_tiles.append(en)
                    if b % 4 == 3:
                        emit_wave(b // 4)
            for b in range(scores_done + 1, NB):
                emit_scores(b)
                if b >= len(en_tiles):
                    en = encN_pool.tile(
                        [128, NCH * H], dt.bfloat16, name=f"en{b}", tag="en"
                    )
                    nc.gpsimd.dma_start(
                        en[:],
                        d_encN[b].rearrange("(p n) h -> p (n h)", p=128),
                    )
                    en_tiles.append(en)
                if b % 4 == 3:
                    emit_wave(b // 4)

        # ---------- phase 2: Z totals, 1/Z, ctx scale, G0 ----------
        ct0 = spool.tile([H0, NB], dt.bfloat16)
        ct1 = spool.tile([H1 + 1, NB], dt.bfloat16)  # row 72 = ones (bias row)
        nc.scalar.dma_start(ct1[H1 : H1 + 1, :], d_ones_b[:, :])

        with tc.tile_pool(name="z_psum", bufs=1, space="PSUM") as z_ps:
            # Z per batch broadcast down all 128 partitions in one matmul
            # (lhsT = ones [16, 128] -> out[r, b] = sum_ch zc[ch, b]), then
            # reciprocal straight into SBUF
            zc_sb = spool.tile([NCH, NB], dt.float32)
            nc.vector.tensor_copy(zc_sb[:], zc[:])
            zbc = z_ps.tile([128, NB], dt.float32, tag="zbc")
            nc.tensor.matmul(zbc[:], ones_sq_f[:], zc_sb[:], start=True, stop=True)
            rzb_sb = spool.tile([128, NB], dt.float32)
            nc.vector.reciprocal(rzb_sb[:], zbc[:])
            # normalize: ctxT = ctx_raw * (1/Z) broadcast, cast bf16
            nc.vector.tensor_tensor(ct0[:], ct0_ps[:], rzb_sb[:], op=OP.mult)
            nc.vector.tensor_tensor(
                ct1[0:H1, :], ct1_ps[:], rzb_sb[0:H1, :], op=OP.mult
            )
        att_ctx.close()  # release sc/ct/kp PSUM banks
        g0_pool = ctx.enter_context(tc.tile_pool(name="g0_psum", bufs=1, space="PSUM"))
        gp = g0_pool.tile([NB, G4], dt.float32, tag="g0")
        for n, nsz in [(0, 512), (512, G4 - 512)]:
            nc.tensor.matmul(
                gp[:, n : n + nsz], qt0, whh0[:, n : n + nsz],
                start=True, stop=False,
            )
            nc.tensor.matmul(
                gp[:, n : n + nsz], qt1, whh1[:, n : n + nsz],
                start=False, stop=False,
            )
            nc.tensor.matmul(
                gp[:, n : n + nsz], ct0[:], wihc0[:, n : n + nsz],
                start=False, stop=False,
            )
            nc.tensor.matmul(
                gp[:, n : n + nsz], ct1, wihc1[:, n : n + nsz],
                start=False, stop=True,
            )

        # ---------- phase 3: decoder steps (all bf16, gate order f,i,o|g) ---
        x_sb = spool.tile([NB, 1], dt.float32)
        nc.sync.dma_start(x_sb[:], d_x0[:, :])
        xn_all = spool.tile([NB, NSTEPS], dt.float32)
        ht0 = spool.tile([H0, NB], dt.bfloat16)
        ht1 = spool.tile([H1 + 1, NB], dt.bfloat16)  # row 72 = ones (b1 row)
        nc.scalar.dma_start(ht1[H1 : H1 + 1, :], d_ones_b[:, :])
        o1t = spool.tile([101, NB], dt.bfloat16)  # row 100 = ones (b2 row)
        nc.scalar.dma_start(o1t[100:101, :], d_ones_b[:, :])
        o2t = spool.tile([51, NB], dt.bfloat16)  # row 50 = ones (b3 row)
        nc.scalar.dma_start(o2t[50:51, :], d_ones_b[:, :])

        with (
            tc.tile_pool(name="ls", bufs=2) as ls,
            tc.tile_pool(name="ls_psum", bufs=3, space="PSUM") as lp,
            tc.tile_pool(name="m3_psum", bufs=2, space="PSUM") as mp,
        ):
            xt = x_sb
            for t in range(NSTEPS):
                # gates = g0 + x * wxr, fused on DVE; split f,i,o vs g so
                # the sigmoid can start before the g slice is computed
                gates = ls.tile([NB, G4], dt.bfloat16, tag="gates")
                for glo, ghi in ((0, 2 * H), (3 * H, 4 * H), (2 * H, 3 * H)):
                    nc.vector.scalar_tensor_tensor(
                        gates[:, glo:ghi], wxr_sb[:, glo:ghi], xt[:, 0:1],
                        gp[:, glo:ghi], op0=OP.mult, op1=OP.add,
                    )
                # f,i sigmoid first (feeds t1/t2), then g tanh, then o
                sfio = ls.tile([NB, 3 * H], dt.bfloat16, tag="sfio")
                nc.scalar.activation(sfio[:, 0 : 2 * H], gates[:, 0 : 2 * H], AF.Sigmoid)
                tg = ls.tile([NB, H], dt.bfloat16, tag="tg")
                nc.scalar.activation(tg[:], gates[:, 3 * H : 4 * H], AF.Tanh)
                nc.scalar.activation(
                    sfio[:, 2 * H : 3 * H], gates[:, 2 * H : 3 * H], AF.Sigmoid
                )
                t1 = ls.tile([NB, H], dt.bfloat16, tag="t1")
                nc.vector.tensor_tensor(t1[:], sfio[:, 0:H], c0_sb, op=OP.mult)
                t2 = ls.tile([NB, H], dt.bfloat16, tag="t2")
                nc.vector.tensor_tensor(t2[:], sfio[:, H : 2 * H], tg[:], op=OP.mult)
                cn = ls.tile([NB, H], dt.bfloat16, tag="cn")
                nc.vector.tensor_tensor(cn[:], t1[:], t2[:], op=OP.add)
                tcn = ls.tile([NB, H], dt.bfloat16, tag="tcn")
                nc.scalar.activation(tcn[:], cn[:], AF.Tanh)
                hh = ls.tile([NB, H], dt.bfloat16, tag="hh")
                nc.vector.tensor_tensor(hh[:], sfio[:, 2 * H : 3 * H], tcn[:], op=OP.mult)
                # transpose h -> ht0/ht1 (feature-major for the MLP); relu
                # folded into the PSUM->SBUF copies (DVE max / ACT relu)
                tp0 = lp.tile([128, NB], dt.bfloat16, tag="lsps")
                nc.tensor.transpose(tp0[:], hh[:, 0:H0], id_bf[0:NB, 0:NB])
                nc.vector.tensor_scalar_max(ht0[:], tp0[:], 0.0)
                tp1 = lp.tile([128, NB], dt.bfloat16, tag="lsps")
                nc.tensor.transpose(tp1[0:H1, :], hh[:, H0:H], id_bf[0:NB, 0:NB])
                nc.scalar.activation(ht1[0:H1, :], tp1[0:H1, :], AF.Relu)
                # MLP in feature-major, biases via ones rows
                m1 = lp.tile([100, NB], dt.float32, tag="lsps")
                nc.tensor.matmul(m1[:], w1t0, ht0[:], start=True, stop=False)
                nc.tensor.matmul(m1[:], w1t1, ht1[:], start=False, stop=True)
                nc.vector.tensor_scalar_max(o1t[0:100, :], m1[:], 0.0)
                m2 = lp.tile([50, NB], dt.float32, tag="lsps")
                nc.tensor.matmul(m2[:], w2t, o1t[:], start=True, stop=True)
                nc.vector.tensor_scalar_max(o2t[0:50, :], m2[:], 0.0)
                # m3 flipped: o2 stationary, w3 moving -> out [NB, 1] is
                # directly the next step's x (read from PSUM as STT scalar)
                m3 = mp.tile([NB, 1], dt.float32, tag="m3")
                nc.tensor.matmul(m3[:], o2t[:], w3t, start=True, stop=True)
                nc.vector.tensor_copy(xn_all[:, t : t + 1], m3[:])
                xt = m3
            nc.sync.dma_start(d_y[:, :], xn_all[:])

    # Bacc lowering: register allocation + wait splitting (<=1 wait/inst on HW)
    nc.compile()
    return nc


def _prep_inputs(x, h0, c0, encoder_output, Wa, ba, Ua, bua, Va, bva,
                 W_ih, W_hh, b_ih, b_hh, W1, b1, W2, b2, W3, b3):
    """Host-side layout prep -> list of per-core input maps."""
    f32 = np.float32
    enc = np.ascontiguousarray(encoder_output, dtype=f32)
    q = np.asarray(h0, dtype=f32)[0]          # [B, H]
    c0f = np.asarray(c0, dtype=f32)[0]        # [B, H]
    x0 = np.asarray(x, dtype=f32).reshape(B, 1)

    # gate permutation: torch order (i,f,g,o) -> device order (f,i,o,g) so
    # one sigmoid instr covers f,i,o and tanh covers g
    gperm = np.concatenate(
        [np.arange(H, 2 * H), np.arange(0, H), np.arange(3 * H, 4 * H),
         np.arange(2 * H, 3 * H)]
    )
    W_ihp = np.asarray(W_ih, f32)[gperm]
    W_hhp = np.asarray(W_hh, f32)[gperm]
    bp = (np.asarray(b_ih, f32) + np.asarray(b_hh, f32))[gperm]

    # Ua scaled x64 into fp8 comfortable range; kernel rescales inside tanh.
    # DoubleRow packing: partition p holds h_in = p (i=0) and 128+p (i=1);
    # M padded 200->208 so the pair stride is 16B-aligned.
    uaT = np.asarray(Ua, f32).T * 64.0  # [h_in, h_out]
    # h_out axis zero-extended left by 96 and right to 352 so "wide" DoubleRow
    # matmuls can compute whole 128-row tiles with garbage rows outside a
    # segment (overwritten by later pieces)
    uaT_pad = np.zeros((256, 352), f32)
    uaT_pad[0:H, 96 : 96 + H] = uaT
    uaT_p = np.stack([uaT_pad[0:128], uaT_pad[128:256]], axis=1)  # [128, 2, 352]

    # packed attention weights
    wpk = np.zeros((128, PK_C), f32)
    waT = np.asarray(Wa, f32).T  # [h_in, h_out]
    wpk[:, PK_WA0 : PK_WA0 + 200] = waT[0:128]
    wpk[0:72, PK_WA1 : PK_WA1 + 200] = waT[128:200]
    va = np.asarray(Va, f32)[0]
    for (bb, ti), cidx in VA_COLS.items():
        r = np.arange(128)
        h = 128 * ti + r - BSTRIDE * bb
        mask = (h >= 0) & (h < H)
        colv = np.zeros(128, f32)
        colv[mask] = va[h[mask]]
        wpk[:, PK_VAPK + cidx] = colv
    qb = np.asarray(ba, f32) + np.asarray(bua, f32)
    wpk[0, PK_QBR : PK_QBR + H] = qb

    # packed decoder weights
    dpk = np.zeros((128, DK_C), f32)
    w_ihcT = W_ihp[:, 1:].T  # [H, G4]
    dpk[:, DK_WIHC0 : DK_WIHC0 + G4] = w_ihcT[0:128]
    dpk[0:72, DK_WIHC1 : DK_WIHC1 + G4] = w_ihcT[128:200]
    dpk[72, DK_WIHC1 : DK_WIHC1 + G4] = bp
    w_hhT = W_hhp.T
    dpk[:, DK_WHH0 : DK_WHH0 + G4] = w_hhT[0:128]
    dpk[0:72, DK_WHH1 : DK_WHH1 + G4] = w_hhT[128:200]
    dpk[0:NB, DK_WXR : DK_WXR + G4] = np.broadcast_to(
        W_ihp[:, 0].reshape(1, G4), (NB, G4)
    )
    w1T = np.asarray(W1, f32).T
    dpk[:, DK_W1T0 : DK_W1T0 + 100] = w1T[0:128]
    dpk[0:72, DK_W1T1 : DK_W1T1 + 100] = w1T[128:200]
    dpk[72, DK_W1T1 : DK_W1T1 + 100] = np.asarray(b1, f32)
    dpk[0:100, DK_W2T : DK_W2T + 50] = np.asarray(W2, f32).T
    dpk[100, DK_W2T : DK_W2T + 50] = np.asarray(b2, f32)
    dpk[0:50, DK_W3T] = np.asarray(W3, f32)[0]
    dpk[50, DK_W3T] = np.asarray(b3, f32)[0]

    shared = {
        "UaTp": np.ascontiguousarray(uaT_p.reshape(128, 2 * 352)).astype(FP8),
        "ones_b": np.ones((1, NB), BF16),
    }

    in_maps = []
    for cix in range(NCORES):
        bs = slice(cix * NB, (cix + 1) * NB)
        enc_c = enc[bs]  # [NB, T, H]
        m = dict(shared)
        encT_c = enc_c.transpose(0, 2, 1)  # [NB, H, T]
        encT_pad = np.concatenate(
            [encT_c, np.zeros((NB, 56, T), f32)], axis=1
        )  # [NB, 256, T]
        encT_p = np.stack([encT_pad[:, 0:128], encT_pad[:, 128:256]], axis=2)
        m["encTp"] = np.ascontiguousarray(encT_p.reshape(NB, 128, 2 * T)).astype(FP8)
        m["encN"] = enc_c.astype(BF16)
        dpk_c = dpk.copy()
        dpk_c[0:NB, DK_C0 : DK_C0 + H] = c0f[bs]
        m["dpk"] = dpk_c.astype(BF16)
        wpk_c = wpk.copy()
        qTc = q[bs].T  # [H, NB]
        wpk_c[:, PK_QT0 : PK_QT0 + NB] = qTc[0:128]
        wpk_c[0:72, PK_QT1 : PK_QT1 + NB] = qTc[128:200]
        m["wpk"] = wpk_c.astype(BF16)
        m["x0s"] = np.ascontiguousarray(x0[bs])
        in_maps.append(m)
    return in_maps


def kernel(**inputs):
    from concourse.bass_utils import run_bass_kernel_spmd

    if "nc" not in _CACHE:
        _CACHE["nc"] = _build_module()
    nc = _CACHE["nc"]

    in_maps = _prep_inputs(**inputs)
    res = run_bass_kernel_spmd(nc, in_maps, core_ids=list(range(NCORES)))
    # y2 per core: [NB, NSTEPS] -> full output [B, NSTEPS]
    out = np.concatenate([r["y2"] for r in res.results], axis=0)
    return np.ascontiguousarray(out.astype(np.float32))


# revision 33
# speedup vs baseline: 1.1599x; 1.0005x over previous
"""Trainium2 Bass kernel for nn_DecoderAttention (Bahdanau attention + LSTM decoder).

Data-parallel over batch: B=128 split across 8 NeuronCores (16 batches/core).
All FLOPs run on device; the host only reshuffles layouts (transpose / dtype
cast / fp8 DoubleRow packing / weight concat with bias rows folded in).

Per-core device pipeline (cost-model-aware layout):
  phase 0: ONE packed DMA for all small attention weights (+one fp32 qb DMA),
           qprojT = Wa @ q^T (+ ba + bua) on PE
  phase 1: per batch b: kprojT = Ua @ enc_b^T as fp8 DoubleRow matmuls
           (K=200 packed 2/partition, one pass, 0.5 cyc/row),
           tanh(kproj/64 + qprojT[:, b]) on ACT -> e tiles [h, t] bf16;
           scores as FLIPPED matmuls: e chunk stationary (K=h, M=128 t's of
           one stride-16 class), Va moving (N=1) -> scores accumulate into
           one [128, 256] PSUM tile, column 16*b+c;
           per WAVE of 4 batches: exp slice [128, 64] -> p (bf16,
           unnormalized), per-batch colsum matmuls -> zc, context via FLIPPED
           matmuls (encN chunk stationary K=t, p column moving N=1) -> ctxT
           accumulates [h, b] in PSUM. All of it hides inside phase 1.
  phase 2 (tail): Z totals via two tiny PE reductions, 1/Z broadcast via a
           K=1 outer-product matmul, ctxT scale on DVE, G0 closes an
           accumulation whose q-terms ran during phase 1.
  phase 3: 5 serial decoder steps, all-bf16 elementwise:
           gates = G0(PSUM) + x*wxr fused on DVE (scalar_tensor_tensor,
           split f,i,o vs g), gate order permuted so one sigmoid covers
           f,i,o; MLP in bf16; m3 flipped so x_next = out [16, 1] feeds the
           next step's scalar directly from PSUM.
"""

import numpy as np
import ml_dtypes

B, T, H = 128, 2048, 200
NCORES = 8
NB = B // NCORES  # 16 batches per core
NSTEPS = 5
G4 = 4 * H  # 800 gate width
NCH = T // 128  # 16 stride-class chunks (t = 16*k + c -> partition k, chunk c)

_CACHE = {}

# M-packed kproj/tanh layout: batch b's 200 h-rows live at packed rows
# [224*b, 224*b+200) (stride 224 = 7*32 keeps every PE write 32-aligned,
# 16*224 = 3584 = 28 full 128-row tiles); h_out padded 200->224 with zeros.
BSTRIDE = 224
NTILE = NB * BSTRIDE // 128  # 28


def _legal_pieces(off, ln):
    """Split (off, ln) into PE-legal out placements: base 0 takes up to 128
    rows, base 64 up to 64, bases 32/96 up to 32."""
    out = []
    while ln > 0:
        cap = {0: 128, 32: 32, 64: 64, 96: 32}[off % 128]
        take = min(ln, cap)
        out.append((off, take))
        off += take
        ln -= take
    return out


def _batch_tiles(b):
    return sorted({ti for ti, _, _, _ in _segments(b)})


# columns of the per-(batch,tile) zero-masked Va block, in emission order
VA_COLS = {}
_c = 0
for _b in range(16):
    for _ti in range(28):
        if 224 * _b < 128 * (_ti + 1) and 128 * _ti < 224 * _b + 224:
            VA_COLS[(_b, _ti)] = _c
            _c += 1
NV = _c


def _segments(b):
    """Packed-row segments of batch b: (tile, off, hlo, ln) covering
    packed rows [BSTRIDE*b, BSTRIDE*(b+1)), split at 128-row tile edges."""
    out = []
    r0 = BSTRIDE * b
    r = r0
    while r < r0 + BSTRIDE:
        ti, off = divmod(r, 128)
        ln = min(128 - off, r0 + BSTRIDE - r)
        out.append((ti, off, r - r0, ln))
        r += ln
    return out


def _sub_segments(b):
    """Real-h sub-segments of batch b for the scores matmuls:
    (tile, off, hlo, ln) additionally split at h=128 and clipped at h=200."""
    out = []
    for ti, off, hlo, ln in _segments(b):
        ln = min(ln, 200 - hlo)
        if ln <= 0:
            continue
        pieces = []
        if hlo < 128 < hlo + ln:
            cut = 128 - hlo
            pieces = [(off, hlo, cut), (off + cut, 128, ln - cut)]
        else:
            pieces = [(off, hlo, ln)]
        for poff, phlo, pln in pieces:
            # stationary reads obey the same base/size tiling rule as writes
            for qoff, qln in _legal_pieces(poff, pln):
                out.append((ti, qoff, phlo + (qoff - poff), qln))
    return out

BF16 = ml_dtypes.bfloat16
FP8 = ml_dtypes.float8_e4m3fn

# packed attention-weight tensor column layout (bf16, [128, PK_C])
PK_WA0, PK_WA1 = 0, 224          # wa0 [128,224] | wa1 [72,224] (224 = padded)
PK_VA0, PK_VA1 = 448, 449        # va columns
PK_QT0, PK_QT1 = 450, 466        # qt [*,16]
PK_QBR = 482                     # qb as a [1, 224] row (for K=1 bias matmuls)
PK_VAPK = 706                    # zero-masked Va per (batch, tile) [128, NV]
PK_C = 706 + 44

# packed decoder-weight tensor column layout (bf16, [128, DK_C])
DK_WIHC0, DK_WIHC1 = 0, 800      # wihc0 [128,800] | wihc1 [73,800] (row72=bias)
DK_WHH0, DK_WHH1 = 1600, 2400    # whh [128|72, 800]
DK_WXR = 3200                    # wxr [16, 800]
DK_W1T0, DK_W1T1 = 4000, 4100    # w1t [128|73, 100] (row72=b1)
DK_W2T = 4200                    # w2t [101, 50] (row100=b2)
DK_W3T = 4250                    # w3t [51, 1] (row50=b3)
DK_C0 = 4251                     # c0 [16, 200]
DK_C = 4451


def _build_module():
    """Build the Bass module (same NEFF for all 8 cores)."""
    from contextlib import ExitStack

    import concourse.bass as bass
    import concourse.tile as tile
    from concourse import bacc, mybir
    from concourse.masks import make_identity

    dt = mybir.dt
    AF = mybir.ActivationFunctionType
    OP = mybir.AluOpType

    nc = bacc.Bacc(
        "TRN2",
        target_bir_lowering=False,
        debug=False,
        num_devices=NCORES,
    )

    # ---- DRAM tensors (per-core shards; weights replicated) ----
    d_encT = nc.dram_tensor(
        "encTp", [NB, 128, 2 * T], dt.float8e4, kind="ExternalInput"
    ).ap()  # DoubleRow packing: col i*T+t, partition p <-> h_in = i*128+p
    d_encN = nc.dram_tensor("encN", [NB, T, H], dt.bfloat16, kind="ExternalInput").ap()
    d_x0 = nc.dram_tensor("x0s", [NB, 1], dt.float32, kind="ExternalInput").ap()
    d_UaT = nc.dram_tensor("UaTp", [128, 2 * 352], dt.float8e4, kind="ExternalInput").ap()
    d_wpk = nc.dram_tensor("wpk", [128, PK_C], dt.bfloat16, kind="ExternalInput").ap()
    d_dpk = nc.dram_tensor("dpk", [128, DK_C], dt.bfloat16, kind="ExternalInput").ap()
    d_ones_b = nc.dram_tensor("ones_b", [1, NB], dt.bfloat16, kind="ExternalInput").ap()
    d_y = nc.dram_tensor("y2", [NB, NSTEPS], dt.float32, kind="ExternalOutput").ap()

    H0, H1 = 128, H - 128  # 128 + 72 partition chunks of the hidden dim

    with tile.TileContext(nc) as tc, ExitStack() as ctx:
        # ---------- persistent pools ----------
        wpool = ctx.enter_context(tc.tile_pool(name="weights", bufs=1))
        spool = ctx.enter_context(tc.tile_pool(name="smalls", bufs=1))

        # warmup: preload the tanh/exp ACT table set while DMAs stream
        wt_a = spool.tile([1, 8], dt.float32)
        nc.vector.memset(wt_a[:], 0.0)
        wt_b = spool.tile([1, 8], dt.float32)
        nc.scalar.activation(wt_b[:], wt_a[:], AF.Tanh)

        # identity for the decoder's h transposes (bf16)
        id_bf = wpool.tile([128, 128], dt.bfloat16)
        make_identity(nc, id_bf[:])

        # ones columns/rows for the tiny PE reductions (sliced on read)
        ones_c_bf = wpool.tile([128, 1], dt.bfloat16)
        nc.vector.memset(ones_c_bf[:], 1.0)
        ones_c_f = wpool.tile([128, 1], dt.float32)
        nc.vector.memset(ones_c_f[:], 1.0)
        ones_sq_f = wpool.tile([NCH, 128], dt.float32)
        nc.vector.memset(ones_sq_f[:], 1.0)

        # packed attention weights: one DMA (triggered after et0/ua below so
        # the kproj-critical transfers go first on the serialized DMA engine)
        wpk = wpool.tile([128, PK_C], dt.bfloat16)
        wa0 = wpk[:, PK_WA0 : PK_WA0 + BSTRIDE]
        wa1 = wpk[0:H1, PK_WA1 : PK_WA1 + BSTRIDE]
        va_pk = wpk[:, PK_VAPK : PK_VAPK + NV]
        qt0 = wpk[:, PK_QT0 : PK_QT0 + NB]
        qt1 = wpk[0:H1, PK_QT1 : PK_QT1 + NB]

        ua_p = wpool.tile([128, 2 * 352], dt.float8e4)
        ua3 = ua_p[:].rearrange("p (i m) -> p i m", i=2)

        # packed decoder weights: one DMA (deferred below, behind first encT)
        dpk = wpool.tile([128, DK_C], dt.bfloat16)
        wihc0 = dpk[:, DK_WIHC0 : DK_WIHC0 + G4]
        wihc1 = dpk[0 : H1 + 1, DK_WIHC1 : DK_WIHC1 + G4]
        whh0 = dpk[:, DK_WHH0 : DK_WHH0 + G4]
        whh1 = dpk[0:H1, DK_WHH1 : DK_WHH1 + G4]
        wxr_sb = dpk[0:NB, DK_WXR : DK_WXR + G4]
        w1t0 = dpk[:, DK_W1T0 : DK_W1T0 + 100]
        w1t1 = dpk[0 : H1 + 1, DK_W1T1 : DK_W1T1 + 100]
        w2t = dpk[0:101, DK_W2T : DK_W2T + 50]
        w3t = dpk[0:51, DK_W3T : DK_W3T + 1]
        c0_sb = dpk[0:NB, DK_C0 : DK_C0 + H]

        # phase-1 encT pool + loader, defined early: the first encT/Ua DMA
        # triggers go BEFORE wpk on the serialized DMA engine, then wpk,
        # then the phase-0 matmuls that read wpk
        encT_pool = ctx.enter_context(tc.tile_pool(name="encT_pool", bufs=4))
        et_tiles = {}

        def load_batch(b):
            et = encT_pool.tile([128, 2 * T], dt.float8e4, tag="et", name=f"et{b}")
            nc.sync.dma_start(et[:], d_encT[b, :])
            et_tiles[b] = et[:].rearrange("p (i t) -> p i t", i=2)
            if b == 0:
                nc.sync.dma_start(ua_p[:], d_UaT[:, :])
            if b == 3:
                # packed decoder weights, split in two so neither transfer
                # stalls the encT pipeline on the serialized DMA engine
                nc.sync.dma_start(dpk[:, 0:2200], d_dpk[:, 0:2200])
            if b == 7:
                nc.sync.dma_start(dpk[:, 2200:DK_C], d_dpk[:, 2200:DK_C])

        load_batch(0)
        nc.scalar.dma_start(wpk[:], d_wpk[:, :])

        # ---------- phase 0: packed qproj bias: qp_pk[r, ti] = ----------
        # qproj[h(r,ti), b(r,ti)] + qb[h]; pad rows get tanh(0*scale+qb-row)
        qp_pk_sb = spool.tile([128, NTILE], dt.float32)
        ones1_b = wpool.tile([1, 1], dt.bfloat16)
        nc.vector.memset(ones1_b[:], 1.0)
        with tc.tile_pool(name="qp_psum", bufs=1, space="PSUM") as qp_ps:
            ps = qp_ps.tile([128, NTILE], dt.float32, tag="qp")
            for b in range(NB):
                for ti, off, hlo, ln in _segments(b):
                    for poff, pln in _legal_pieces(off, ln):
                        phlo = hlo + (poff - off)
                        dst = ps[poff : poff + pln, ti : ti + 1]
                        nc.tensor.matmul(
                            dst, wa0[:, phlo : phlo + pln], qt0[:, b : b + 1],
                            start=True, stop=False, tile_position=(0, poff),
                        )
                        nc.tensor.matmul(
                            dst, wa1[:, phlo : phlo + pln], qt1[:, b : b + 1],
                            start=False, stop=False, tile_position=(0, poff),
                        )
                        # + qb (K=1 outer with the qb row)
                        nc.tensor.matmul(
                            dst, wpk[0:1, PK_QBR + phlo : PK_QBR + phlo + pln],
                            ones1_b[:], start=False, stop=True,
                            tile_position=(0, poff),
                        )
            nc.vector.tensor_copy(qp_pk_sb[:], ps[:])

        # ---------- phase 1: kproj + tanh + scores + per-wave softmax/ctx ----
        e_pool = ctx.enter_context(tc.tile_pool(name="e_pool", bufs=6))
        encN_pool = ctx.enter_context(tc.tile_pool(name="encN_pool", bufs=16))
        from contextlib import ExitStack as _ES
        att_ctx = _ES()
        sc_pool = att_ctx.enter_context(tc.tile_pool(name="sc_psum", bufs=1, space="PSUM"))
        ct_pool = att_ctx.enter_context(tc.tile_pool(name="ct_psum", bufs=1, space="PSUM"))
        sc = sc_pool.tile([128, NB * NCH + NB], dt.float32, tag="sc")
        p_sb = spool.tile([128, NB * NCH], dt.bfloat16)
        zc = sc[0:NCH, NB * NCH : NB * NCH + NB]
        # one PSUM tile (PSUM tiles are bank-granular): ct0 | ct1 columns
        ctz = ct_pool.tile([128, 2 * NB], dt.float32, tag="ctz")
        ct0_ps = ctz[:, 0:NB]
        ct1_ps = ctz[0:H1, NB : 2 * NB]

        en_tiles = []
        kp_hist = {}  # batch -> last kproj matmul (encN pacing anchor)
        e_pk = []     # packed e tiles, one per 128-row tile

        def emit_scores(b):
            tiles = _batch_tiles(b)
            for c in range(NCH):
                col = b * NCH + c
                for k, ti in enumerate(tiles):
                    # full-tile contraction; va column is zero outside
                    # batch b's rows, so other batches contribute nothing
                    vcol = VA_COLS[(b, ti)]
                    nc.tensor.matmul(
                        sc[:, col : col + 1],
                        e_pk[ti][:, c : T : NCH],
                        va_pk[:, vcol : vcol + 1],
                        start=(k == 0),
                        stop=(k == len(tiles) - 1),
                    )

        def emit_wave(w):
            # exp + Z colsums + context for batches 4w..4w+3 (scores ready)
            lo = 4 * w * NCH
            nc.scalar.activation(
                p_sb[:, lo : lo + 4 * NCH], sc[:, lo : lo + 4 * NCH], AF.Exp
            )
            for b in range(4 * w, 4 * w + 4):
                nc.tensor.matmul(
                    zc[:, b : b + 1],
                    p_sb[:, b * NCH : (b + 1) * NCH],
                    ones_c_bf[:],
                    start=True,
                    stop=True,
                )
                # complete each accumulation chain before starting the next
                # (two open groups in one PSUM bank trip the zero-region rule)
                for c in range(NCH):
                    nc.tensor.matmul(
                        ct0_ps[:, b : b + 1],
                        en_tiles[b][:, c * H : c * H + H0],
                        p_sb[:, b * NCH + c : b * NCH + c + 1],
                        start=(c == 0),
                        stop=(c == NCH - 1),
                    )
                for c in range(NCH):
                    nc.tensor.matmul(
                        ct1_ps[:, b : b + 1],
                        en_tiles[b][:, c * H + H0 : (c + 1) * H],
                        p_sb[:, b * NCH + c : b * NCH + c + 1],
                        start=(c == 0),
                        stop=(c == NCH - 1),
                    )

        # per-tile segment map and per-batch last tile
        tile_segs = [[] for _ in range(NTILE)]
        for b in range(NB):
            for ti, off, hlo, ln in _segments(b):
                tile_segs[ti].append((b, off, hlo, ln))
        last_tile = [_segments(b)[-1][0] for b in range(NB)]

        import bass_rust as _br
        scores_done = -1  # highest batch whose scores are emitted

        with tc.tile_pool(name="kp_psum", bufs=3, space="PSUM") as kp_ps:
            for ti in range(NTILE):
                for b, _, _, _ in tile_segs[ti]:
                    if b not in et_tiles:
                        load_batch(b)
                e = e_pool.tile([128, T], dt.bfloat16, tag="e", name=f"e{ti}")
                e_pk.append(e)
                # plan: wide DoubleRow computes the whole tile from the
                # zero-extended Ua (garbage rows), later pieces overwrite.
                # DR only encodes dst at base 0 with 32/64/128-row groups.
                segs = tile_segs[ti]
                plan = []  # (kind, b, off, ln, ua_lo) kind: dr | wide | ndr
                if len(segs) == 1:
                    b0_, off0_, hlo0_, ln0_ = segs[0]
                    plan.append(("dr", b0_, 0, 128, 96 + hlo0_))
                else:
                    (bx, ox, hx, lx), (by, oy, hy, ly) = segs[0], segs[1]
                    if lx in (32, 64):
                        # wide(Y) first, DR(X) overwrites rows 0..lx
                        plan.append(("wide", by, 0, 128, 96 + hy - oy))
                        plan.append(("dr", bx, 0, lx, 96 + hx))
                    else:
                        # wide(X) first, non-DR(Y) overwrites the top rows
                        plan.append(("wide", bx, 0, 128, 96 + hx))
                        plan.append(("ndr", by, oy, ly, 96 + hy))
                for th in range(2):
                    ps = kp_ps.tile([128, 1024], dt.float32, tag="kp")
                    for kind, b, poff, pln, ua_lo in plan:
                        for n in range(2):
                            c0c = th * 1024 + n * 512
                            dst = ps[poff : poff + pln, n * 512 : (n + 1) * 512]
                            if kind in ("dr", "wide"):
                                i_kp = nc.tensor.matmul(
                                    dst,
                                    ua3[:, :, ua_lo : ua_lo + pln],
                                    et_tiles[b][:, :, c0c : c0c + 512],
                                    start=True,
                                    stop=True,
                                    perf_mode=mybir.MatmulPerfMode.DoubleRow,
                                    tile_position=(0, 0),
                                )
                            else:
                                for i in range(2):
                                    i_kp = nc.tensor.matmul(
                                        dst,
                                        ua3[:, i, ua_lo : ua_lo + pln],
                                        et_tiles[b][:, i, c0c : c0c + 512],
                                        start=(i == 0),
                                        stop=(i == 1),
                                        tile_position=(0, poff),
                                    )
                            kp_hist[b] = i_kp
                    # e = tanh(kproj/64 + qp_pk[:, ti]) ; write bf16
                    nc.scalar.activation(
                        e[:, th * 1024 : (th + 1) * 1024],
                        ps[:],
                        AF.Tanh,
                        bias=qp_pk_sb[:, ti : ti + 1],
                        scale=1.0 / 64.0,
                    )
                # scores/waves with a one-tile skew; encN paced ~2 batches back
                for b in range(scores_done + 1, NB):
                    if last_tile[b] > ti - 2:
                        break
                    emit_scores(b)
                    scores_done = b
                    en = encN_pool.tile(
                        [128, NCH * H], dt.bfloat16, name=f"en{b}", tag="en"
                    )
                    i_en = nc.gpsimd.dma_start(
                        en[:],
                        d_encN[b].rearrange("(p n) h -> p (n h)", p=128),
                    )
                    _br.add_dep_helper(
                        i_en.ins, kp_hist[max(0, b - 2)].ins, sync=True,
                        reason="encN paced behind kproj two batches back",
                    )
                    en_tiles.append(en)
                    if b % 4 == 3:
                        emit_wave(b // 4)
            for b in range(scores_done + 1, NB):
                emit_scores(b)
                if b >= len(en_tiles):
                    en = encN_pool.tile(
                        [128, NCH * H], dt.bfloat16, name=f"en{b}", tag="en"
                    )
                    nc.gpsimd.dma_start(
                        en[:],
                        d_encN[b].rearrange("(p n) h -> p (n h)", p=128),
                    )
                    en_tiles.append(en)
                if b % 4 == 3:
                    emit_wave(b // 4)

        # ---------- phase 2: Z totals, 1/Z, ctx scale, G0 ----------
        ct0 = spool.tile([H0, NB], dt.bfloat16)
        ct1 = spool.tile([H1 + 1, NB], dt.bfloat16)  # row 72 = ones (bias row)
        nc.scalar.dma_start(ct1[H1 : H1 + 1, :], d_ones_b[:, :])

        with tc.tile_pool(name="z_psum", bufs=1, space="PSUM") as z_ps:
            # Z per batch broadcast down all 128 partitions in one matmul
            # (lhsT = ones [16, 128] -> out[r, b] = sum_ch zc[ch, b]), then
            # reciprocal straight into SBUF
            zc_sb = spool.tile([NCH, NB], dt.float32)
            nc.vector.tensor_copy(zc_sb[:], zc[:])
            zbc = z_ps.tile([128, NB], dt.float32, tag="zbc")
            nc.tensor.matmul(zbc[:], ones_sq_f[:], zc_sb[:], start=True, stop=True)
            rzb_sb = spool.tile([128, NB], dt.float32)
            nc.vector.reciprocal(rzb_sb[:], zbc[:])
            # normalize: ctxT = ctx_raw * (1/Z) broadcast, cast bf16
            nc.vector.tensor_tensor(ct0[:], ct0_ps[:], rzb_sb[:], op=OP.mult)
            nc.vector.tensor_tensor(
                ct1[0:H1, :], ct1_ps[:], rzb_sb[0:H1, :], op=OP.mult
            )
        att_ctx.close()  # release sc/ct/kp PSUM banks
        g0_pool = ctx.enter_context(tc.tile_pool(name="g0_psum", bufs=1, space="PSUM"))
        gp = g0_pool.tile([NB, G4], dt.float32, tag="g0")
        for n, nsz in [(0, 512), (512, G4 - 512)]:
            nc.tensor.matmul(
                gp[:, n : n + nsz], qt0, whh0[:, n : n + nsz],
                start=True, stop=False,
            )
            nc.tensor.matmul(
                gp[:, n : n + nsz], qt1, whh1[:, n : n + nsz],
                start=False, stop=False,
            )
            nc.tensor.matmul(
                gp[:, n : n + nsz], ct0[:], wihc0[:, n : n + nsz],
                start=False, stop=False,
            )
            nc.tensor.matmul(
                gp[:, n : n + nsz], ct1, wihc1[:, n : n + nsz],
                start=False, stop=True,
            )

        # ---------- phase 3: decoder steps (all bf16, gate order f,i,o|g) ---
        x_sb = spool.tile([NB, 1], dt.float32)
        nc.sync.dma_start(x_sb[:], d_x0[:, :])
        xn_all = spool.tile([NB, NSTEPS], dt.float32)
        ht0 = spool.tile([H0, NB], dt.bfloat16)
        ht1 = spool.tile([H1 + 1, NB], dt.bfloat16)  # row 72 = ones (b1 row)
        nc.scalar.dma_start(ht1[H1 : H1 + 1, :], d_ones_b[:, :])
        o1t = spool.tile([101, NB], dt.bfloat16)  # row 100 = ones (b2 row)
        nc.scalar.dma_start(o1t[100:101, :], d_ones_b[:, :])
        o2t = spool.tile([51, NB], dt.bfloat16)  # row 50 = ones (b3 row)
        nc.scalar.dma_start(o2t[50:51, :], d_ones_b[:, :])

        with (
            tc.tile_pool(name="ls", bufs=2) as ls,
            tc.tile_pool(name="ls_psum", bufs=3, space="PSUM") as lp,
            tc.tile_pool(name="m3_psum", bufs=2, space="PSUM") as mp,
        ):
            xt = x_sb
            for t in range(NSTEPS):
                # gates = g0 + x * wxr, fused on DVE; split f,i,o vs g so
                # the sigmoid can start before the g slice is computed
                gates = ls.tile([NB, G4], dt.bfloat16, tag="gates")
                for glo, ghi in ((0, 2 * H), (3 * H, 4 * H), (2 * H, 3 * H)):
                    nc.vector.scalar_tensor_tensor(
                        gates[:, glo:ghi], wxr_sb[:, glo:ghi], xt[:, 0:1],
                        gp[:, glo:ghi], op0=OP.mult, op1=OP.add,
                    )
                # f,i sigmoid first (feeds t1/t2), then g tanh, then o
                sfio = ls.tile([NB, 3 * H], dt.bfloat16, tag="sfio")
                nc.scalar.activation(sfio[:, 0 : 2 * H], gates[:, 0 : 2 * H], AF.Sigmoid)
                tg = ls.tile([NB, H], dt.bfloat16, tag="tg")
                nc.scalar.activation(tg[:], gates[:, 3 * H : 4 * H], AF.Tanh)
                nc.scalar.activation(
                    sfio[:, 2 * H : 3 * H], gates[:, 2 * H : 3 * H], AF.Sigmoid
                )
                t1 = ls.tile([NB, H], dt.bfloat16, tag="t1")
                nc.vector.tensor_tensor(t1[:], sfio[:, 0:H], c0_sb, op=OP.mult)
                t2 = ls.tile([NB, H], dt.bfloat16, tag="t2")
                nc.vector.tensor_tensor(t2[:], sfio[:, H : 2 * H], tg[:], op=OP.mult)
                cn = ls.tile([NB, H], dt.bfloat16, tag="cn")
                nc.vector.tensor_tensor(cn[:], t1[:], t2[:], op=OP.add)
                tcn = ls.tile([NB, H], dt.bfloat16, tag="tcn")
                nc.scalar.activation(tcn[:], cn[:], AF.Tanh)
                hh = ls.tile([NB, H], dt.bfloat16, tag="hh")
                nc.vector.tensor_tensor(hh[:], sfio[:, 2 * H : 3 * H], tcn[:], op=OP.mult)
                # transpose h -> ht0/ht1 (feature-major for the MLP); relu
                # folded into the PSUM->SBUF copies (DVE max / ACT relu)
                tp0 = lp.tile([128, NB], dt.bfloat16, tag="lsps")
                nc.tensor.transpose(tp0[:], hh[:, 0:H0], id_bf[0:NB, 0:NB])
                nc.vector.tensor_scalar_max(ht0[:], tp0[:], 0.0)
                tp1 = lp.tile([128, NB], dt.bfloat16, tag="lsps")
                nc.tensor.transpose(tp1[0:H1, :], hh[:, H0:H], id_bf[0:NB, 0:NB])
                nc.scalar.activation(ht1[0:H1, :], tp1[0:H1, :], AF.Relu)
                # MLP in feature-major, biases via ones rows
                m1 = lp.tile([100, NB], dt.float32, tag="lsps")
                nc.tensor.matmul(m1[:], w1t0, ht0[:], start=True, stop=False)
                nc.tensor.matmul(m1[:], w1t1, ht1[:], start=False, stop=True)
                nc.vector.tensor_scalar_max(o1t[0:100, :], m1[:], 0.0)
                m2 = lp.tile([50, NB], dt.float32, tag="lsps")
                nc.tensor.matmul(m2[:], w2t, o1t[:], start=True, stop=True)
                nc.vector.tensor_scalar_max(o2t[0:50, :], m2[:], 0.0)
                # m3 flipped: o2 stationary, w3 moving -> out [NB, 1] is
                # directly the next step's x (read from PSUM as STT scalar)
                m3 = mp.tile([NB, 1], dt.float32, tag="m3")
                nc.tensor.matmul(m3[:], o2t[:], w3t, start=True, stop=True)
                nc.vector.tensor_copy(xn_all[:, t : t + 1], m3[:])
                xt = m3
            nc.sync.dma_start(d_y[:, :], xn_all[:])

    # Bacc lowering: register allocation + wait splitting (<=1 wait/inst on HW)
    nc.compile()
    return nc


def _prep_inputs(x, h0, c0, encoder_output, Wa, ba, Ua, bua, Va, bva,
                 W_ih, W_hh, b_ih, b_hh, W1, b1, W2, b2, W3, b3):
    """Host-side layout prep -> list of per-core input maps."""
    f32 = np.float32
    enc = np.ascontiguousarray(encoder_output, dtype=f32)
    q = np.asarray(h0, dtype=f32)[0]          # [B, H]
    c0f = np.asarray(c0, dtype=f32)[0]        # [B, H]
    x0 = np.asarray(x, dtype=f32).reshape(B, 1)

    # gate permutation: torch order (i,f,g,o) -> device order (f,i,o,g) so
    # one sigmoid instr covers f,i,o and tanh covers g
    gperm = np.concatenate(
        [np.arange(H, 2 * H), np.arange(0, H), np.arange(3 * H, 4 * H),
         np.arange(2 * H, 3 * H)]
    )
    W_ihp = np.asarray(W_ih, f32)[gperm]
    W_hhp = np.asarray(W_hh, f32)[gperm]
    bp = (np.asarray(b_ih, f32) + np.asarray(b_hh, f32))[gperm]

    # Ua scaled x64 into fp8 comfortable range; kernel rescales inside tanh.
    # DoubleRow packing: partition p holds h_in = p (i=0) and 128+p (i=1);
    # M padded 200->208 so the pair stride is 16B-aligned.
    uaT = np.asarray(Ua, f32).T * 64.0  # [h_in, h_out]
    # h_out axis zero-extended left by 96 and right to 352 so "wide" DoubleRow
    # matmuls can compute whole 128-row tiles with garbage rows outside a
    # segment (overwritten by later pieces)
    uaT_pad = np.zeros((256, 352), f32)
    uaT_pad[0:H, 96 : 96 + H] = uaT
    uaT_p = np.stack([uaT_pad[0:128], uaT_pad[128:256]], axis=1)  # [128, 2, 352]

    # packed attention weights
    wpk = np.zeros((128, PK_C), f32)
    waT = np.asarray(Wa, f32).T  # [h_in, h_out]
    wpk[:, PK_WA0 : PK_WA0 + 200] = waT[0:128]
    wpk[0:72, PK_WA1 : PK_WA1 + 200] = waT[128:200]
    va = np.asarray(Va, f32)[0]
    for (bb, ti), cidx in VA_COLS.items():
        r = np.arange(128)
        h = 128 * ti + r - BSTRIDE * bb
        mask = (h >= 0) & (h < H)
        colv = np.zeros(128, f32)
        colv[mask] = va[h[mask]]
        wpk[:, PK_VAPK + cidx] = colv
    qb = np.asarray(ba, f32) + np.asarray(bua, f32)
    wpk[0, PK_QBR : PK_QBR + H] = qb

    # packed decoder weights
    dpk = np.zeros((128, DK_C), f32)
    w_ihcT = W_ihp[:, 1:].T  # [H, G4]
    dpk[:, DK_WIHC0 : DK_WIHC0 + G4] = w_ihcT[0:128]
    dpk[0:72, DK_WIHC1 : DK_WIHC1 + G4] = w_ihcT[128:200]
    dpk[72, DK_WIHC1 : DK_WIHC1 + G4] = bp
    w_hhT = W_hhp.T
    dpk[:, DK_WHH0 : DK_WHH0 + G4] = w_hhT[0:128]
    dpk[0:72, DK_WHH1 : DK_WHH1 + G4] = w_hhT[128:200]
    dpk[0:NB, DK_WXR : DK_WXR + G4] = np.broadcast_to(
        W_ihp[:, 0].reshape(1, G4), (NB, G4)
    )
    w1T = np.asarray(W1, f32).T
    dpk[:, DK_W1T0 : DK_W1T0 + 100] = w1T[0:128]
    dpk[0:72, DK_W1T1 : DK_W1T1 + 100] = w1T[128:200]
    dpk[72, DK_W1T1 : DK_W1T1 + 100] = np.asarray(b1, f32)
    dpk[0:100, DK_W2T : DK_W2T + 50] = np.asarray(W2, f32).T
    dpk[100, DK_W2T : DK_W2T + 50] = np.asarray(b2, f32)
    dpk[0:50, DK_W3T] = np.asarray(W3, f32)[0]
    dpk[50, DK_W3T] = np.asarray(b3, f32)[0]

    shared = {
        "UaTp": np.ascontiguousarray(uaT_p.reshape(128, 2 * 352)).astype(FP8),
        "ones_b": np.ones((1, NB), BF16),
    }

    in_maps = []
    for cix in range(NCORES):
        bs = slice(cix * NB, (cix + 1) * NB)
        enc_c = enc[bs]  # [NB, T, H]
        m = dict(shared)
        encT_c = enc_c.transpose(0, 2, 1)  # [NB, H, T]
        encT_pad = np.concatenate(
            [encT_c, np.zeros((NB, 56, T), f32)], axis=1
        )  # [NB, 256, T]
        encT_p = np.stack([encT_pad[:, 0:128], encT_pad[:, 128:256]], axis=2)
        m["encTp"] = np.ascontiguousarray(encT_p.reshape(NB, 128, 2 * T)).astype(FP8)
        m["encN"] = enc_c.astype(BF16)
        dpk_c = dpk.copy()
        dpk_c[0:NB, DK_C0 : DK_C0 + H] = c0f[bs]
        m["dpk"] = dpk_c.astype(BF16)
        wpk_c = wpk.copy()
        qTc = q[bs].T  # [H, NB]
        wpk_c[:, PK_QT0 : PK_QT0 + NB] = qTc[0:128]
        wpk_c[0:72, PK_QT1 : PK_QT1 + NB] = qTc[128:200]
        m["wpk"] = wpk_c.astype(BF16)
        m["x0s"] = np.ascontiguousarray(x0[bs])
        in_maps.append(m)
    return in_maps


def kernel(**inputs):
    from concourse.bass_utils import run_bass_kernel_spmd

    if "nc" not in _CACHE:
        _CACHE["nc"] = _build_module()
    nc = _CACHE["nc"]

    in_maps = _prep_inputs(**inputs)
    res = run_bass_kernel_spmd(nc, in_maps, core_ids=list(range(NCORES)))
    # y2 per core: [NB, NSTEPS] -> full output [B, NSTEPS]
    out = np.concatenate([r["y2"] for r in res.results], axis=0)
    return np.ascontiguousarray(out.astype(np.float32))


# revision 35
# speedup vs baseline: 1.1705x; 1.0091x over previous
"""Trainium2 Bass kernel for nn_DecoderAttention (Bahdanau attention + LSTM decoder).

Data-parallel over batch: B=128 split across 8 NeuronCores (16 batches/core).
All FLOPs run on device; the host only reshuffles layouts (transpose / dtype
cast / fp8 DoubleRow packing / weight concat with bias rows folded in).

Per-core device pipeline (cost-model-aware layout):
  phase 0: ONE packed DMA for all small attention weights (+one fp32 qb DMA),
           qprojT = Wa @ q^T (+ ba + bua) on PE
  phase 1: per batch b: kprojT = Ua @ enc_b^T as fp8 DoubleRow matmuls
           (K=200 packed 2/partition, one pass, 0.5 cyc/row),
           tanh(kproj/64 + qprojT[:, b]) on ACT -> e tiles [h, t] bf16;
           scores as FLIPPED matmuls: e chunk stationary (K=h, M=128 t's of
           one stride-16 class), Va moving (N=1) -> scores accumulate into
           one [128, 256] PSUM tile, column 16*b+c;
           per WAVE of 4 batches: exp slice [128, 64] -> p (bf16,
           unnormalized), per-batch colsum matmuls -> zc, context via FLIPPED
           matmuls (encN chunk stationary K=t, p column moving N=1) -> ctxT
           accumulates [h, b] in PSUM. All of it hides inside phase 1.
  phase 2 (tail): Z totals via two tiny PE reductions, 1/Z broadcast via a
           K=1 outer-product matmul, ctxT scale on DVE, G0 closes an
           accumulation whose q-terms ran during phase 1.
  phase 3: 5 serial decoder steps, all-bf16 elementwise:
           gates = G0(PSUM) + x*wxr fused on DVE (scalar_tensor_tensor,
           split f,i,o vs g), gate order permuted so one sigmoid covers
           f,i,o; MLP in bf16; m3 flipped so x_next = out [16, 1] feeds the
           next step's scalar directly from PSUM.
"""

import numpy as np
import ml_dtypes

B, T, H = 128, 2048, 200
NCORES = 8
NB = B // NCORES  # 16 batches per core
NSTEPS = 5
G4 = 4 * H  # 800 gate width
NCH = T // 128  # 16 stride-class chunks (t = 16*k + c -> partition k, chunk c)

_CACHE = {}

# M-packed kproj/tanh layout: batch b's 200 h-rows live at packed rows
# [224*b, 224*b+200) (stride 224 = 7*32 keeps every PE write 32-aligned,
# 16*224 = 3584 = 28 full 128-row tiles); h_out padded 200->224 with zeros.
BSTRIDE = 224
NTILE = NB * BSTRIDE // 128  # 28


def _legal_pieces(off, ln):
    """Split (off, ln) into PE-legal out placements: base 0 takes up to 128
    rows, base 64 up to 64, bases 32/96 up to 32."""
    out = []
    while ln > 0:
        cap = {0: 128, 32: 32, 64: 64, 96: 32}[off % 128]
        take = min(ln, cap)
        out.append((off, take))
        off += take
        ln -= take
    return out


def _batch_tiles(b):
    return sorted({ti for ti, _, _, _ in _segments(b)})


# columns of the per-(batch,tile) zero-masked Va block, in emission order
VA_COLS = {}
_c = 0
for _b in range(16):
    for _ti in range(28):
        if 224 * _b < 128 * (_ti + 1) and 128 * _ti < 224 * _b + 224:
            VA_COLS[(_b, _ti)] = _c
            _c += 1
NV = _c


def _segments(b):
    """Packed-row segments of batch b: (tile, off, hlo, ln) covering
    packed rows [BSTRIDE*b, BSTRIDE*(b+1)), split at 128-row tile edges."""
    out = []
    r0 = BSTRIDE * b
    r = r0
    while r < r0 + BSTRIDE:
        ti, off = divmod(r, 128)
        ln = min(128 - off, r0 + BSTRIDE - r)
        out.append((ti, off, r - r0, ln))
        r += ln
    return out


def _sub_segments(b):
    """Real-h sub-segments of batch b for the scores matmuls:
    (tile, off, hlo, ln) additionally split at h=128 and clipped at h=200."""
    out = []
    for ti, off, hlo, ln in _segments(b):
        ln = min(ln, 200 - hlo)
        if ln <= 0:
            continue
        pieces = []
        if hlo < 128 < hlo + ln:
            cut = 128 - hlo
            pieces = [(off, hlo, cut), (off + cut, 128, ln - cut)]
        else:
            pieces = [(off, hlo, ln)]
        for poff, phlo, pln in pieces:
            # stationary reads obey the same base/size tiling rule as writes
            for qoff, qln in _legal_pieces(poff, pln):
                out.append((ti, qoff, phlo + (qoff - poff), qln))
    return out

BF16 = ml_dtypes.bfloat16
FP8 = ml_dtypes.float8_e4m3fn

# packed attention-weight tensor column layout (bf16, [128, PK_C])
PK_WA0, PK_WA1 = 0, 224          # wa0 [128,224] | wa1 [72,224] (224 = padded)
PK_VA0, PK_VA1 = 448, 449        # va columns
PK_QT0, PK_QT1 = 450, 466        # qt [*,16]
PK_QBR = 482                     # qb as a [1, 224] row (for K=1 bias matmuls)
PK_VAPK = 706                    # zero-masked Va per (batch, tile) [128, NV]
PK_C = 706 + 44

# packed decoder-weight tensor column layout (bf16, [128, DK_C])
DK_WIHC0, DK_WIHC1 = 0, 800      # wihc0 [128,800] | wihc1 [73,800] (row72=bias)
DK_WHH0, DK_WHH1 = 1600, 2400    # whh [128|72, 800]
DK_WXR = 3200                    # wxr [16, 800]
DK_W1T0, DK_W1T1 = 4000, 4100    # w1t [128|73, 100] (row72=b1)
DK_W2T = 4200                    # w2t [101, 50] (row100=b2)
DK_W3T = 4250                    # w3t [51, 1] (row50=b3)
DK_C0 = 4251                     # c0 [16, 200]
DK_C = 4451


def _build_module():
    """Build the Bass module (same NEFF for all 8 cores)."""
    from contextlib import ExitStack

    import concourse.bass as bass
    import concourse.tile as tile
    from concourse import bacc, mybir
    from concourse.masks import make_identity

    dt = mybir.dt
    AF = mybir.ActivationFunctionType
    OP = mybir.AluOpType

    nc = bacc.Bacc(
        "TRN2",
        target_bir_lowering=False,
        debug=False,
        num_devices=NCORES,
    )

    # ---- DRAM tensors (per-core shards; weights replicated) ----
    d_encT = nc.dram_tensor(
        "encTp", [NB, 128, 2 * T], dt.float8e4, kind="ExternalInput"
    ).ap()  # DoubleRow packing: col i*T+t, partition p <-> h_in = i*128+p
    d_encN = nc.dram_tensor("encN", [NB, T, H], dt.bfloat16, kind="ExternalInput").ap()
    d_x0 = nc.dram_tensor("x0s", [NB, 1], dt.float32, kind="ExternalInput").ap()
    d_UaT = nc.dram_tensor("UaTp", [128, 2 * 352], dt.float8e4, kind="ExternalInput").ap()
    d_wpk = nc.dram_tensor("wpk", [128, PK_C], dt.bfloat16, kind="ExternalInput").ap()
    d_dpk = nc.dram_tensor("dpk", [128, DK_C], dt.bfloat16, kind="ExternalInput").ap()
    d_ones_b = nc.dram_tensor("ones_b", [1, NB], dt.bfloat16, kind="ExternalInput").ap()
    d_y = nc.dram_tensor("y2", [NB, NSTEPS], dt.float32, kind="ExternalOutput").ap()

    H0, H1 = 128, H - 128  # 128 + 72 partition chunks of the hidden dim

    with tile.TileContext(nc) as tc, ExitStack() as ctx:
        # ---------- persistent pools ----------
        wpool = ctx.enter_context(tc.tile_pool(name="weights", bufs=1))
        spool = ctx.enter_context(tc.tile_pool(name="smalls", bufs=1))

        # warmup: preload the tanh/exp ACT table set while DMAs stream
        wt_a = spool.tile([1, 8], dt.float32)
        nc.vector.memset(wt_a[:], 0.0)
        wt_b = spool.tile([1, 8], dt.float32)
        nc.scalar.activation(wt_b[:], wt_a[:], AF.Tanh)

        # identity for the decoder's h transposes (bf16)
        id_bf = wpool.tile([128, 128], dt.bfloat16)
        make_identity(nc, id_bf[:])

        # ones columns/rows for the tiny PE reductions (sliced on read)
        ones_c_bf = wpool.tile([128, 1], dt.bfloat16)
        nc.vector.memset(ones_c_bf[:], 1.0)
        ones_c_f = wpool.tile([128, 1], dt.float32)
        nc.vector.memset(ones_c_f[:], 1.0)
        ones_sq_f = wpool.tile([NCH, 128], dt.float32)
        nc.vector.memset(ones_sq_f[:], 1.0)

        # packed attention weights: one DMA (triggered after et0/ua below so
        # the kproj-critical transfers go first on the serialized DMA engine)
        wpk = wpool.tile([128, PK_C], dt.bfloat16)
        wa0 = wpk[:, PK_WA0 : PK_WA0 + BSTRIDE]
        wa1 = wpk[0:H1, PK_WA1 : PK_WA1 + BSTRIDE]
        va_pk = wpk[:, PK_VAPK : PK_VAPK + NV]
        qt0 = wpk[:, PK_QT0 : PK_QT0 + NB]
        qt1 = wpk[0:H1, PK_QT1 : PK_QT1 + NB]

        ua_p = wpool.tile([128, 2 * 352], dt.float8e4)
        ua3 = ua_p[:].rearrange("p (i m) -> p i m", i=2)

        # packed decoder weights: one DMA (deferred below, behind first encT)
        dpk = wpool.tile([128, DK_C], dt.bfloat16)
        wihc0 = dpk[:, DK_WIHC0 : DK_WIHC0 + G4]
        wihc1 = dpk[0 : H1 + 1, DK_WIHC1 : DK_WIHC1 + G4]
        whh0 = dpk[:, DK_WHH0 : DK_WHH0 + G4]
        whh1 = dpk[0:H1, DK_WHH1 : DK_WHH1 + G4]
        wxr_sb = dpk[0:NB, DK_WXR : DK_WXR + G4]
        w1t0 = dpk[:, DK_W1T0 : DK_W1T0 + 100]
        w1t1 = dpk[0 : H1 + 1, DK_W1T1 : DK_W1T1 + 100]
        w2t = dpk[0:101, DK_W2T : DK_W2T + 50]
        w3t = dpk[0:51, DK_W3T : DK_W3T + 1]
        c0_sb = dpk[0:NB, DK_C0 : DK_C0 + H]

        # phase-1 encT pool + loader, defined early: the first encT/Ua DMA
        # triggers go BEFORE wpk on the serialized DMA engine, then wpk,
        # then the phase-0 matmuls that read wpk
        encT_pool = ctx.enter_context(tc.tile_pool(name="encT_pool", bufs=4))
        et_tiles = {}

        def load_batch(b):
            et = encT_pool.tile([128, 2 * T], dt.float8e4, tag="et", name=f"et{b}")
            nc.sync.dma_start(et[:], d_encT[b, :])
            et_tiles[b] = et[:].rearrange("p (i t) -> p i t", i=2)
            if b == 0:
                nc.sync.dma_start(ua_p[:], d_UaT[:, :])
            if b == 3:
                # packed decoder weights, split in two so neither transfer
                # stalls the encT pipeline on the serialized DMA engine
                nc.sync.dma_start(dpk[:, 0:2200], d_dpk[:, 0:2200])
            if b == 7:
                nc.sync.dma_start(dpk[:, 2200:DK_C], d_dpk[:, 2200:DK_C])

        load_batch(0)
        nc.scalar.dma_start(wpk[:], d_wpk[:, :])

        # ---------- phase 0: packed qproj bias: qp_pk[r, ti] = ----------
        # qproj[h(r,ti), b(r,ti)] + qb[h]; pad rows get tanh(0*scale+qb-row)
        qp_pk_sb = spool.tile([128, NTILE], dt.float32)
        ones1_b = wpool.tile([1, 1], dt.bfloat16)
        nc.vector.memset(ones1_b[:], 1.0)
        with tc.tile_pool(name="qp_psum", bufs=1, space="PSUM") as qp_ps:
            # burn the PE p-state ramp while the first DMAs are in flight
            warm = qp_ps.tile([128, 128], dt.float32, tag="warm")
            for _ in range(3):
                nc.tensor.matmul(warm[:], id_bf[:], id_bf[:], start=True, stop=True)
            ps = qp_ps.tile([128, NTILE], dt.float32, tag="qp")
            for b in range(NB):
                for ti, off, hlo, ln in _segments(b):
                    for poff, pln in _legal_pieces(off, ln):
                        phlo = hlo + (poff - off)
                        dst = ps[poff : poff + pln, ti : ti + 1]
                        nc.tensor.matmul(
                            dst, wa0[:, phlo : phlo + pln], qt0[:, b : b + 1],
                            start=True, stop=False, tile_position=(0, poff),
                        )
                        nc.tensor.matmul(
                            dst, wa1[:, phlo : phlo + pln], qt1[:, b : b + 1],
                            start=False, stop=False, tile_position=(0, poff),
                        )
                        # + qb (K=1 outer with the qb row)
                        nc.tensor.matmul(
                            dst, wpk[0:1, PK_QBR + phlo : PK_QBR + phlo + pln],
                            ones1_b[:], start=False, stop=True,
                            tile_position=(0, poff),
                        )
            nc.vector.tensor_copy(qp_pk_sb[:], ps[:])

        # ---------- phase 1: kproj + tanh + scores + per-wave softmax/ctx ----
        e_pool = ctx.enter_context(tc.tile_pool(name="e_pool", bufs=6))
        encN_pool = ctx.enter_context(tc.tile_pool(name="encN_pool", bufs=16))
        from contextlib import ExitStack as _ES
        att_ctx = _ES()
        sc_pool = att_ctx.enter_context(tc.tile_pool(name="sc_psum", bufs=1, space="PSUM"))
        ct_pool = att_ctx.enter_context(tc.tile_pool(name="ct_psum", bufs=1, space="PSUM"))
        sc = sc_pool.tile([128, NB * NCH + NB], dt.float32, tag="sc")
        p_sb = spool.tile([128, NB * NCH], dt.bfloat16)
        zc = sc[0:NCH, NB * NCH : NB * NCH + NB]
        # one PSUM tile (PSUM tiles are bank-granular): ct0 | ct1 columns
        ctz = ct_pool.tile([128, 2 * NB], dt.float32, tag="ctz")
        ct0_ps = ctz[:, 0:NB]
        ct1_ps = ctz[0:H1, NB : 2 * NB]

        en_tiles = []
        kp_hist = {}  # batch -> last kproj matmul (encN pacing anchor)
        e_pk = []     # packed e tiles, one per 128-row tile

        def emit_scores(b):
            tiles = _batch_tiles(b)
            for c in range(NCH):
                col = b * NCH + c
                for k, ti in enumerate(tiles):
                    # full-tile contraction; va column is zero outside
                    # batch b's rows, so other batches contribute nothing
                    vcol = VA_COLS[(b, ti)]
                    nc.tensor.matmul(
                        sc[:, col : col + 1],
                        e_pk[ti][:, c : T : NCH],
                        va_pk[:, vcol : vcol + 1],
                        start=(k == 0),
                        stop=(k == len(tiles) - 1),
                    )

        def emit_wave(w):
            # exp + Z colsums + context for batches 4w..4w+3 (scores ready)
            lo = 4 * w * NCH
            nc.scalar.activation(
                p_sb[:, lo : lo + 4 * NCH], sc[:, lo : lo + 4 * NCH], AF.Exp
            )
            for b in range(4 * w, 4 * w + 4):
                nc.tensor.matmul(
                    zc[:, b : b + 1],
                    p_sb[:, b * NCH : (b + 1) * NCH],
                    ones_c_bf[:],
                    start=True,
                    stop=True,
                )
                # complete each accumulation chain before starting the next
                # (two open groups in one PSUM bank trip the zero-region rule)
                for c in range(NCH):
                    nc.tensor.matmul(
                        ct0_ps[:, b : b + 1],
                        en_tiles[b][:, c * H : c * H + H0],
                        p_sb[:, b * NCH + c : b * NCH + c + 1],
                        start=(c == 0),
                        stop=(c == NCH - 1),
                    )
                for c in range(NCH):
                    nc.tensor.matmul(
                        ct1_ps[:, b : b + 1],
                        en_tiles[b][:, c * H + H0 : (c + 1) * H],
                        p_sb[:, b * NCH + c : b * NCH + c + 1],
                        start=(c == 0),
                        stop=(c == NCH - 1),
                    )

        # per-tile segment map and per-batch last tile
        tile_segs = [[] for _ in range(NTILE)]
        for b in range(NB):
            for ti, off, hlo, ln in _segments(b):
                tile_segs[ti].append((b, off, hlo, ln))
        last_tile = [_segments(b)[-1][0] for b in range(NB)]

        import bass_rust as _br
        scores_done = -1  # highest batch whose scores are emitted

        with tc.tile_pool(name="kp_psum", bufs=3, space="PSUM") as kp_ps:
            for ti in range(NTILE):
                for b, _, _, _ in tile_segs[ti]:
                    if b not in et_tiles:
                        load_batch(b)
                e = e_pool.tile([128, T], dt.bfloat16, tag="e", name=f"e{ti}")
                e_pk.append(e)
                # plan: wide DoubleRow computes the whole tile from the
                # zero-extended Ua (garbage rows), later pieces overwrite.
                # DR only encodes dst at base 0 with 32/64/128-row groups.
                segs = tile_segs[ti]
                plan = []  # (kind, b, off, ln, ua_lo) kind: dr | wide | ndr
                if len(segs) == 1:
                    b0_, off0_, hlo0_, ln0_ = segs[0]
                    plan.append(("dr", b0_, 0, 128, 96 + hlo0_))
                else:
                    (bx, ox, hx, lx), (by, oy, hy, ly) = segs[0], segs[1]
                    if lx in (32, 64):
                        # wide(Y) first, DR(X) overwrites rows 0..lx
                        plan.append(("wide", by, 0, 128, 96 + hy - oy))
                        plan.append(("dr", bx, 0, lx, 96 + hx))
                    else:
                        # wide(X) first, non-DR(Y) overwrites the top rows
                        plan.append(("wide", bx, 0, 128, 96 + hx))
                        plan.append(("ndr", by, oy, ly, 96 + hy))
                for th in range(2):
                    ps = kp_ps.tile([128, 1024], dt.float32, tag="kp")
                    for kind, b, poff, pln, ua_lo in plan:
                        for n in range(2):
                            c0c = th * 1024 + n * 512
                            dst = ps[poff : poff + pln, n * 512 : (n + 1) * 512]
                            if kind in ("dr", "wide"):
                                i_kp = nc.tensor.matmul(
                                    dst,
                                    ua3[:, :, ua_lo : ua_lo + pln],
                                    et_tiles[b][:, :, c0c : c0c + 512],
                                    start=True,
                                    stop=True,
                                    perf_mode=mybir.MatmulPerfMode.DoubleRow,
                                    tile_position=(0, 0),
                                )
                            else:
                                for i in range(2):
                                    i_kp = nc.tensor.matmul(
                                        dst,
                                        ua3[:, i, ua_lo : ua_lo + pln],
                                        et_tiles[b][:, i, c0c : c0c + 512],
                                        start=(i == 0),
                                        stop=(i == 1),
                                        tile_position=(0, poff),
                                    )
                            kp_hist[b] = i_kp
                    # e = tanh(kproj/64 + qp_pk[:, ti]) ; write bf16
                    nc.scalar.activation(
                        e[:, th * 1024 : (th + 1) * 1024],
                        ps[:],
                        AF.Tanh,
                        bias=qp_pk_sb[:, ti : ti + 1],
                        scale=1.0 / 64.0,
                    )
                # scores/waves with a one-tile skew; encN paced ~2 batches back
                for b in range(scores_done + 1, NB):
                    if last_tile[b] > ti - 2:
                        break
                    emit_scores(b)
                    scores_done = b
                    en = encN_pool.tile(
                        [128, NCH * H], dt.bfloat16, name=f"en{b}", tag="en"
                    )
                    i_en = nc.gpsimd.dma_start(
                        en[:],
                        d_encN[b].rearrange("(p n) h -> p (n h)", p=128),
                    )
                    _br.add_dep_helper(
                        i_en.ins, kp_hist[max(0, b - 2)].ins, sync=True,
                        reason="encN paced behind kproj two batches back",
                    )
                    en_tiles.append(en)
                    if b % 4 == 3:
                        emit_wave(b // 4)
            for b in range(scores_done + 1, NB):
                emit_scores(b)
                if b >= len(en_tiles):
                    en = encN_pool.tile(
                        [128, NCH * H], dt.bfloat16, name=f"en{b}", tag="en"
                    )
                    nc.gpsimd.dma_start(
                        en[:],
                        d_encN[b].rearrange("(p n) h -> p (n h)", p=128),
                    )
                    en_tiles.append(en)
                if b % 4 == 3:
                    emit_wave(b // 4)

        # ---------- phase 2: Z totals, 1/Z, ctx scale, G0 ----------
        ct0 = spool.tile([H0, NB], dt.bfloat16)
        ct1 = spool.tile([H1 + 1, NB], dt.bfloat16)  # row 72 = ones (bias row)
        nc.scalar.dma_start(ct1[H1 : H1 + 1, :], d_ones_b[:, :])

        with tc.tile_pool(name="z_psum", bufs=1, space="PSUM") as z_ps:
            # Z per batch broadcast down all 128 partitions in one matmul
            # (lhsT = ones [16, 128] -> out[r, b] = sum_ch zc[ch, b]), then
            # reciprocal straight into SBUF
            zc_sb = spool.tile([NCH, NB], dt.float32)
            nc.vector.tensor_copy(zc_sb[:], zc[:])
            zbc = z_ps.tile([128, NB], dt.float32, tag="zbc")
            nc.tensor.matmul(zbc[:], ones_sq_f[:], zc_sb[:], start=True, stop=True)
            rzb_sb = spool.tile([128, NB], dt.float32)
            nc.vector.reciprocal(rzb_sb[:], zbc[:])
            # normalize: ctxT = ctx_raw * (1/Z) broadcast, cast bf16
            nc.vector.tensor_tensor(ct0[:], ct0_ps[:], rzb_sb[:], op=OP.mult)
            nc.vector.tensor_tensor(
                ct1[0:H1, :], ct1_ps[:], rzb_sb[0:H1, :], op=OP.mult
            )
        att_ctx.close()  # release sc/ct/kp PSUM banks
        g0_pool = ctx.enter_context(tc.tile_pool(name="g0_psum", bufs=1, space="PSUM"))
        gp = g0_pool.tile([NB, G4], dt.float32, tag="g0")
        for n, nsz in [(0, 512), (512, G4 - 512)]:
            nc.tensor.matmul(
                gp[:, n : n + nsz], qt0, whh0[:, n : n + nsz],
                start=True, stop=False,
            )
            nc.tensor.matmul(
                gp[:, n : n + nsz], qt1, whh1[:, n : n + nsz],
                start=False, stop=False,
            )
            nc.tensor.matmul(
                gp[:, n : n + nsz], ct0[:], wihc0[:, n : n + nsz],
                start=False, stop=False,
            )
            nc.tensor.matmul(
                gp[:, n : n + nsz], ct1, wihc1[:, n : n + nsz],
                start=False, stop=True,
            )

        # ---------- phase 3: decoder steps (all bf16, gate order f,i,o|g) ---
        x_sb = spool.tile([NB, 1], dt.float32)
        nc.sync.dma_start(x_sb[:], d_x0[:, :])
        xn_all = spool.tile([NB, NSTEPS], dt.float32)
        ht0 = spool.tile([H0, NB], dt.bfloat16)
        ht1 = spool.tile([H1 + 1, NB], dt.bfloat16)  # row 72 = ones (b1 row)
        nc.scalar.dma_start(ht1[H1 : H1 + 1, :], d_ones_b[:, :])
        o1t = spool.tile([101, NB], dt.bfloat16)  # row 100 = ones (b2 row)
        nc.scalar.dma_start(o1t[100:101, :], d_ones_b[:, :])
        o2t = spool.tile([51, NB], dt.bfloat16)  # row 50 = ones (b3 row)
        nc.scalar.dma_start(o2t[50:51, :], d_ones_b[:, :])

        with (
            tc.tile_pool(name="ls", bufs=2) as ls,
            tc.tile_pool(name="ls_psum", bufs=3, space="PSUM") as lp,
            tc.tile_pool(name="m3_psum", bufs=2, space="PSUM") as mp,
        ):
            xt = x_sb
            pend = []
            for t in range(NSTEPS):
                # gates = g0 + x * wxr, fused on DVE; split f,i,o vs g so
                # the sigmoid can start before the g slice is computed
                gates = ls.tile([NB, G4], dt.bfloat16, tag="gates")
                for glo, ghi in ((0, 2 * H), (3 * H, 4 * H), (2 * H, 3 * H)):
                    nc.vector.scalar_tensor_tensor(
                        gates[:, glo:ghi], wxr_sb[:, glo:ghi], xt[:, 0:1],
                        gp[:, glo:ghi], op0=OP.mult, op1=OP.add,
                    )
                if pend:
                    pt, pm3 = pend.pop()
                    nc.vector.tensor_copy(xn_all[:, pt : pt + 1], pm3[:])
                # f,i sigmoid first (feeds t1/t2), then g tanh, then o
                sfio = ls.tile([NB, 3 * H], dt.bfloat16, tag="sfio")
                nc.scalar.activation(sfio[:, 0 : 2 * H], gates[:, 0 : 2 * H], AF.Sigmoid)
                tg = ls.tile([NB, H], dt.bfloat16, tag="tg")
                nc.scalar.activation(tg[:], gates[:, 3 * H : 4 * H], AF.Tanh)
                nc.scalar.activation(
                    sfio[:, 2 * H : 3 * H], gates[:, 2 * H : 3 * H], AF.Sigmoid
                )
                t1 = ls.tile([NB, H], dt.bfloat16, tag="t1")
                nc.vector.tensor_tensor(t1[:], sfio[:, 0:H], c0_sb, op=OP.mult)
                t2 = ls.tile([NB, H], dt.bfloat16, tag="t2")
                nc.vector.tensor_tensor(t2[:], sfio[:, H : 2 * H], tg[:], op=OP.mult)
                cn = ls.tile([NB, H], dt.bfloat16, tag="cn")
                nc.vector.tensor_tensor(cn[:], t1[:], t2[:], op=OP.add)
                tcn = ls.tile([NB, H], dt.bfloat16, tag="tcn")
                nc.scalar.activation(tcn[:], cn[:], AF.Tanh)
                hh = ls.tile([NB, H], dt.bfloat16, tag="hh")
                nc.vector.tensor_tensor(hh[:], sfio[:, 2 * H : 3 * H], tcn[:], op=OP.mult)
                # transpose h -> ht0/ht1 (feature-major for the MLP); relu
                # folded into the PSUM->SBUF copies (DVE max / ACT relu)
                tp0 = lp.tile([128, NB], dt.bfloat16, tag="lsps")
                nc.tensor.transpose(tp0[:], hh[:, 0:H0], id_bf[0:NB, 0:NB])
                nc.vector.tensor_scalar_max(ht0[:], tp0[:], 0.0)
                tp1 = lp.tile([128, NB], dt.bfloat16, tag="lsps")
                nc.tensor.transpose(tp1[0:H1, :], hh[:, H0:H], id_bf[0:NB, 0:NB])
                nc.scalar.activation(ht1[0:H1, :], tp1[0:H1, :], AF.Relu)
                # MLP in feature-major, biases via ones rows
                m1 = lp.tile([100, NB], dt.float32, tag="lsps")
                nc.tensor.matmul(m1[:], w1t0, ht0[:], start=True, stop=False)
                nc.tensor.matmul(m1[:], w1t1, ht1[:], start=False, stop=True)
                nc.vector.tensor_scalar_max(o1t[0:100, :], m1[:], 0.0)
                m2 = lp.tile([50, NB], dt.float32, tag="lsps")
                nc.tensor.matmul(m2[:], w2t, o1t[:], start=True, stop=True)
                nc.vector.tensor_scalar_max(o2t[0:50, :], m2[:], 0.0)
                # m3 flipped: o2 stationary, w3 moving -> out [NB, 1] is
                # directly the next step's x (read from PSUM as STT scalar)
                m3 = mp.tile([NB, 1], dt.float32, tag="m3")
                nc.tensor.matmul(m3[:], o2t[:], w3t, start=True, stop=True)
                pend.append((t, m3))
                xt = m3
            for t, m3 in pend:
                nc.vector.tensor_copy(xn_all[:, t : t + 1], m3[:])
            nc.sync.dma_start(d_y[:, :], xn_all[:])

    # Bacc lowering: register allocation + wait splitting (<=1 wait/inst on HW)
    nc.compile()
    return nc


def _prep_inputs(x, h0, c0, encoder_output, Wa, ba, Ua, bua, Va, bva,
                 W_ih, W_hh, b_ih, b_hh, W1, b1, W2, b2, W3, b3):
    """Host-side layout prep -> list of per-core input maps."""
    f32 = np.float32
    enc = np.ascontiguousarray(encoder_output, dtype=f32)
    q = np.asarray(h0, dtype=f32)[0]          # [B, H]
    c0f = np.asarray(c0, dtype=f32)[0]        # [B, H]
    x0 = np.asarray(x, dtype=f32).reshape(B, 1)

    # gate permutation: torch order (i,f,g,o) -> device order (f,i,o,g) so
    # one sigmoid instr covers f,i,o and tanh covers g
    gperm = np.concatenate(
        [np.arange(H, 2 * H), np.arange(0, H), np.arange(3 * H, 4 * H),
         np.arange(2 * H, 3 * H)]
    )
    W_ihp = np.asarray(W_ih, f32)[gperm]
    W_hhp = np.asarray(W_hh, f32)[gperm]
    bp = (np.asarray(b_ih, f32) + np.asarray(b_hh, f32))[gperm]

    # Ua scaled x64 into fp8 comfortable range; kernel rescales inside tanh.
    # DoubleRow packing: partition p holds h_in = p (i=0) and 128+p (i=1);
    # M padded 200->208 so the pair stride is 16B-aligned.
    uaT = np.asarray(Ua, f32).T * 64.0  # [h_in, h_out]
    # h_out axis zero-extended left by 96 and right to 352 so "wide" DoubleRow
    # matmuls can compute whole 128-row tiles with garbage rows outside a
    # segment (overwritten by later pieces)
    uaT_pad = np.zeros((256, 352), f32)
    uaT_pad[0:H, 96 : 96 + H] = uaT
    uaT_p = np.stack([uaT_pad[0:128], uaT_pad[128:256]], axis=1)  # [128, 2, 352]

    # packed attention weights
    wpk = np.zeros((128, PK_C), f32)
    waT = np.asarray(Wa, f32).T  # [h_in, h_out]
    wpk[:, PK_WA0 : PK_WA0 + 200] = waT[0:128]
    wpk[0:72, PK_WA1 : PK_WA1 + 200] = waT[128:200]
    va = np.asarray(Va, f32)[0]
    for (bb, ti), cidx in VA_COLS.items():
        r = np.arange(128)
        h = 128 * ti + r - BSTRIDE * bb
        mask = (h >= 0) & (h < H)
        colv = np.zeros(128, f32)
        colv[mask] = va[h[mask]]
        wpk[:, PK_VAPK + cidx] = colv
    qb = np.asarray(ba, f32) + np.asarray(bua, f32)
    wpk[0, PK_QBR : PK_QBR + H] = qb

    # packed decoder weights
    dpk = np.zeros((128, DK_C), f32)
    w_ihcT = W_ihp[:, 1:].T  # [H, G4]
    dpk[:, DK_WIHC0 : DK_WIHC0 + G4] = w_ihcT[0:128]
    dpk[0:72, DK_WIHC1 : DK_WIHC1 + G4] = w_ihcT[128:200]
    dpk[72, DK_WIHC1 : DK_WIHC1 + G4] = bp
    w_hhT = W_hhp.T
    dpk[:, DK_WHH0 : DK_WHH0 + G4] = w_hhT[0:128]
    dpk[0:72, DK_WHH1 : DK_WHH1 + G4] = w_hhT[128:200]
    dpk[0:NB, DK_WXR : DK_WXR + G4] = np.broadcast_to(
        W_ihp[:, 0].reshape(1, G4), (NB, G4)
    )
    w1T = np.asarray(W1, f32).T
    dpk[:, DK_W1T0 : DK_W1T0 + 100] = w1T[0:128]
    dpk[0:72, DK_W1T1 : DK_W1T1 + 100] = w1T[128:200]
    dpk[72, DK_W1T1 : DK_W1T1 + 100] = np.asarray(b1, f32)
    dpk[0:100, DK_W2T : DK_W2T + 50] = np.asarray(W2, f32).T
    dpk[100, DK_W2T : DK_W2T + 50] = np.asarray(b2, f32)
    dpk[0:50, DK_W3T] = np.asarray(W3, f32)[0]
    dpk[50, DK_W3T] = np.asarray(b3, f32)[0]

    shared = {
        "UaTp": np.ascontiguousarray(uaT_p.reshape(128, 2 * 352)).astype(FP8),
        "ones_b": np.ones((1, NB), BF16),
    }

    in_maps = []
    for cix in range(NCORES):
        bs = slice(cix * NB, (cix + 1) * NB)
        enc_c = enc[bs]  # [NB, T, H]
        m = dict(shared)
        encT_c = enc_c.transpose(0, 2, 1)  # [NB, H, T]
        encT_pad = np.concatenate(
            [encT_c, np.zeros((NB, 56, T), f32)], axis=1
        )  # [NB, 256, T]
        encT_p = np.stack([encT_pad[:, 0:128], encT_pad[:, 128:256]], axis=2)
        m["encTp"] = np.ascontiguousarray(encT_p.reshape(NB, 128, 2 * T)).astype(FP8)
        m["encN"] = enc_c.astype(BF16)
        dpk_c = dpk.copy()
        dpk_c[0:NB, DK_C0 : DK_C0 + H] = c0f[bs]
        m["dpk"] = dpk_c.astype(BF16)
        wpk_c = wpk.copy()
        qTc = q[bs].T  # [H, NB]
        wpk_c[:, PK_QT0 : PK_QT0 + NB] = qTc[0:128]
        wpk_c[0:72, PK_QT1 : PK_QT1 + NB] = qTc[128:200]
        m["wpk"] = wpk_c.astype(BF16)
        m["x0s"] = np.ascontiguousarray(x0[bs])
        in_maps.append(m)
    return in_maps


def kernel(**inputs):
    from concourse.bass_utils import run_bass_kernel_spmd

    if "nc" not in _CACHE:
        _CACHE["nc"] = _build_module()
    nc = _CACHE["nc"]

    in_maps = _prep_inputs(**inputs)
    res = run_bass_kernel_spmd(nc, in_maps, core_ids=list(range(NCORES)))
    # y2 per core: [NB, NSTEPS] -> full output [B, NSTEPS]
    out = np.concatenate([r["y2"] for r in res.results], axis=0)
    return np.ascontiguousarray(out.astype(np.float32))
